# revision 1
# baseline (speedup 1.0000x reference)
"""GAT (2-layer) Trainium2 kernel, SPMD across 8 NeuronCores.

Key algebra: segment softmax keyed by row is shift invariant, so the
(h[row] . a_l) term cancels and attention factorizes:
    alpha_e = g[col_e] * u[row_e],
    g[n] = exp(h[n] . a_r),   u[r] = 1 / sum_{e: row=r} g[col_e]
Each GAT layer then needs only two unweighted sparse ops over the fixed
graph:
    z   = A @ g          (segment-sum keyed by row)   -> u = 1/z
    agg = A^T @ (u * h)  (segment-sum keyed by col)
    out = g * agg
Both are done as: dma_gather of table rows per edge (128 edges/block) +
one-hot matmul (lhsT = one-hot of block-relative destination, built by a
DVE is_equal against an iota tile) accumulating into a PSUM window.

Sharding: z-phase edges by row range, aggregation edges by col range (each
core owns its 1250-node output slice). Cross-core: AllGather of u
([10000,H] f32) and of h1^T (5 MB) between the layers.

kernel(**inputs) takes FULL inputs and returns the FULL [10000, 22] output.
"""

import sys

sys.path.insert(0, "/opt/trn_rl_repo")

import numpy as np
import ml_dtypes

from concourse import bacc, mybir, tile
from concourse.bass_utils import run_bass_kernel_spmd

F32 = mybir.dt.float32
BF16 = mybir.dt.bfloat16
I16 = mybir.dt.int16
EXP = mybir.ActivationFunctionType.Exp
EQ = mybir.AluOpType.is_equal
MULT = mybir.AluOpType.mult
ADD = mybir.AluOpType.add
MIN = mybir.AluOpType.min
BYPASS = mybir.AluOpType.bypass

N = 10000
E = 320000
F = 128
H = 4
C = 22
P = 8
SLICE = N // P               # 1250 nodes per core
NWIN = (SLICE + 127) // 128  # 10 windows of <=128 dst/src nodes
NBLK = N // 128 + 1          # 79; always >= 1 pad block so row N is zero
NPAD = NBLK * 128            # 10112; table rows >= N are zero
OW1 = H * F                  # 512
CHUNK = 16                   # layer-1 gather chunk (128-edge blocks)
SKIP = set()                 # debug/timing: {"z", "agg1", "agg2"}


def _configure(n, e, p=8):
    """Shrink sizes for simulator debugging (same program structure)."""
    global N, E, P, SLICE, NWIN, NBLK, NPAD
    N, E, P = n, e, p
    SLICE = N // P
    NWIN = (SLICE + 127) // 128
    NBLK = N // 128 + 1
    NPAD = NBLK * 128


def _cdiv(a, b):
    return (a + b - 1) // b


def _wrap_idxs(idx):
    """dma_gather index layout: logical i at [i%16, i//16], replicated to
    128 partitions."""
    n = idx.shape[0]
    assert n % 16 == 0
    w = idx.reshape(n // 16, 16).T.astype(np.int16)
    return np.tile(w, (8, 1))


def _phase_arrays(key, other, nwin):
    """Group one core's (already core-local) edges by 128-wide key window.
    Returns per-window (rel, other) with rel = key - 128*w."""
    w = key >> 7
    order = np.argsort(w, kind="stable")
    key, other, w = key[order], other[order], w[order]
    out = []
    bounds = np.searchsorted(w, np.arange(nwin + 1))
    for i in range(nwin):
        sl = slice(bounds[i], bounds[i + 1])
        k, o = key[sl] - 128 * i, other[sl]
        so = np.argsort(o, kind="stable")  # sorted gather idx -> HBM locality
        out.append((k[so], o[so]))
    return out


def _build_edge_inputs(row, col):
    zraw, braw = [], []
    for k in range(P):
        base = k * SLICE
        m = (row >= base) & (row < base + SLICE)
        zraw.append(_phase_arrays(row[m] - base, col[m], NWIN))
        m = (col >= base) & (col < base + SLICE)
        braw.append(_phase_arrays(col[m] - base, row[m], NWIN))

    def block_counts(raw):
        return [
            max(_cdiv(max(max(len(raw[k][w][0]) for k in range(P)), 1), 128), 1)
            for w in range(NWIN)
        ]

    zB = block_counts(zraw)
    bB = block_counts(braw)

    def pack(raw, B):
        idx_l, rel_l = [], []
        for w in range(NWIN):
            n = B[w] * 128
            rel = np.zeros(n, np.int32)
            oth = np.full(n, N, np.int32)  # dummy -> zero table row
            r, o = raw[w]
            rel[: len(r)] = r
            oth[: len(o)] = o
            idx_l.append(_wrap_idxs(oth))
            rel_l.append(rel.reshape(B[w], 128).T.astype(np.float32))
        return np.concatenate(idx_l, 1), np.concatenate(rel_l, 1)

    per_core = []
    for k in range(P):
        zidx, zrel = pack(zraw[k], zB)
        bidx, brel = pack(braw[k], bB)
        base = k * SLICE
        gw = []
        for w in range(NWIN):
            nid = base + 128 * w + np.arange(128)
            nid = np.where(nid < base + SLICE, nid, N)
            gw.append(_wrap_idxs(nid))
        per_core.append(
            dict(
                zidx=zidx,
                zrel=zrel,
                bidx=bidx,
                brel_f=brel,
                gwidx=np.concatenate(gw, 1),
            )
        )
    return zB, bB, per_core


def _spmm(nc, tc, B, CH, idx_d, rel_d, tab, elem, rhs_w, psum_w, iof_t,
          name, flush, skip=False, bufs=3):
    """One-hot-matmul SpMM over 128-dst windows with gather chunks that span
    window boundaries. flush(w, po) consumes each window's PSUM result."""
    with (
        tc.tile_pool(name=f"gg{name}", bufs=bufs) as ggp,
        tc.tile_pool(name=f"gi{name}", bufs=bufs) as gip,
        tc.tile_pool(name=f"gr{name}", bufs=bufs) as grp,
        tc.tile_pool(name=f"go{name}", bufs=bufs) as ohp,
        tc.tile_pool(name=f"gp{name}", bufs=2, space="PSUM") as pp,
    ):
        total = sum(B)
        gts, ohs = {}, {}
        gb = 0
        for w, Bw in enumerate(B):
            po = pp.tile([128, psum_w], F32, tag="po")
            if skip:
                nc.vector.memset(po[:], 1.0)
                flush(w, po)
                continue
            for b in range(Bw):
                ch, off = divmod(gb, CH)
                if off == 0:
                    cb = min(CH, total - ch * CH)
                    it = gip.tile([128, CH * 8], I16, tag="gi")
                    nc.sync.dma_start(
                        it[:, : cb * 8],
                        idx_d[:, ch * CH * 8 : (ch * CH + cb) * 8],
                    )
                    gt = ggp.tile([128, CH, elem], BF16, tag="gg")
                    nc.gpsimd.dma_gather(
                        gt[:, :cb, :], tab[:], it[:, : cb * 8],
                        cb * 128, cb * 128, elem, single_packet=False,
                    )
                    rl = grp.tile([128, CH], F32, tag="gr")
                    nc.sync.dma_start(
                        rl[:, :cb], rel_d[:, ch * CH : ch * CH + cb]
                    )
                    oh = ohp.tile([128, CH, 128], BF16, tag="go")
                    nc.vector.tensor_tensor(
                        oh[:, :cb, :],
                        iof_t[:].rearrange("p (x f) -> p x f", x=1)
                        .broadcast_to([128, cb, 128]),
                        rl[:, :cb].rearrange("p (b x) -> p b x", x=1)
                        .broadcast_to([128, cb, 128]),
                        EQ,
                    )
                    gts[ch], ohs[ch] = gt, oh
                nc.tensor.matmul(
                    po[:], ohs[ch][:, off, :], gts[ch][:, off, 0:rhs_w],
                    start=(b == 0), stop=(b == Bw - 1),
                )
                gb += 1
            flush(w, po)


def _declare(nc, zB, bB):
    ZT, BT = sum(zB), sum(bB)
    T = type("T", (), {})()
    T.xT = nc.dram_tensor("xT", [F, NPAD], F32, kind="ExternalInput")
    T.W1 = nc.dram_tensor("W1", [F, OW1], F32, kind="ExternalInput")
    T.W2 = nc.dram_tensor("W2", [F, C], F32, kind="ExternalInput")
    T.a1rc = nc.dram_tensor("a1rc", [F, H], F32, kind="ExternalInput")
    T.a2rc = nc.dram_tensor("a2rc", [F, 1], F32, kind="ExternalInput")
    T.ident = nc.dram_tensor("ident", [128, 128], F32, kind="ExternalInput")
    T.iota_f = nc.dram_tensor("iota_f", [128, 128], F32, kind="ExternalInput")
    T.zidx_d = nc.dram_tensor("zidx", [128, ZT * 8], I16, kind="ExternalInput")
    T.zrel_d = nc.dram_tensor("zrel", [128, ZT], F32, kind="ExternalInput")
    T.bidx_d = nc.dram_tensor("bidx", [128, BT * 8], I16, kind="ExternalInput")
    T.brelf_d = nc.dram_tensor("brel_f", [128, BT], F32, kind="ExternalInput")
    T.gwidx_d = nc.dram_tensor("gwidx", [128, NWIN * 8], I16, kind="ExternalInput")
    T.out_d = nc.dram_tensor("out", [SLICE, C], F32, kind="ExternalOutput")

    T.g1_tab = nc.dram_tensor("g1_tab", [NPAD, 128], BF16)
    T.hh1_tab = nc.dram_tensor("hh1_tab", [NPAD, OW1], BF16)
    T.g2_tab = nc.dram_tensor("g2_tab", [NPAD, 128], BF16)
    T.hh2_tab = nc.dram_tensor("hh2_tab", [NPAD, 128], BF16)
    T.u1_sl = nc.dram_tensor("u1_sl", [SLICE, H], F32)
    T.u2_sl = nc.dram_tensor("u2_sl", [SLICE, 1], F32)
    T.u1_full = nc.dram_tensor("u1_full", [NPAD, H], F32, addr_space="Shared")
    T.u2_full = nc.dram_tensor("u2_full", [NPAD, 1], F32, addr_space="Shared")
    T.h1T_loc = nc.dram_tensor("h1T_loc", [F, SLICE], F32)
    T.h1T_ag = nc.dram_tensor("h1T_ag", [P, F, SLICE], F32, addr_space="Shared")

    return T


def _emit(nc, tc, T, zB, bB, s=""):
        groups = [list(range(P))]
        # ================= layer 1: dense + tables + z1 =================
        with (
            tc.tile_pool(name="persist" + s, bufs=1) as pp,
            tc.tile_pool(name="small" + s, bufs=3) as sp,
        ):
            W1_t = pp.tile([F, OW1], F32)
            nc.sync.dma_start(W1_t[:], T.W1[:])
            id_t = pp.tile([128, 128], F32)
            nc.sync.dma_start(id_t[:], T.ident[:])
            iof_t = pp.tile([128, 128], F32)
            nc.sync.dma_start(iof_t[:], T.iota_f[:])
            a1rc_t = pp.tile([F, H], F32)
            nc.sync.dma_start(a1rc_t[:], T.a1rc[:])
            W1ar_t = pp.tile([F, H], F32)

            with tc.tile_pool(name="ptr" + s, bufs=2, space="PSUM") as ptr:
                for hd in range(H):
                    pt = ptr.tile([128, 128], F32, tag="pt")
                    nc.tensor.transpose(pt[:], W1_t[:, hd * F : (hd + 1) * F], id_t[:])
                    w1t = sp.tile([128, 128], F32, tag="w1t")
                    nc.vector.tensor_copy(w1t[:], pt[:])
                    pv = ptr.tile([128, 1], F32, tag="pv")
                    nc.tensor.matmul(
                        pv[:], w1t[:], a1rc_t[:, hd : hd + 1], start=True, stop=True
                    )
                    nc.vector.tensor_copy(W1ar_t[:, hd : hd + 1], pv[:])

            h_nm = pp.tile([128, NBLK, OW1], F32)  # 20.2 MB
            g1_nm = pp.tile([128, NBLK, H], F32)
            with (
                tc.tile_pool(name="xtp" + s, bufs=3) as xtp,
                tc.tile_pool(name="ph" + s, bufs=2, space="PSUM") as php,
                tc.tile_pool(name="psr" + s, bufs=2, space="PSUM") as psrp,
            ):
                for b in range(NBLK):
                    xt = xtp.tile([128, 128], F32)
                    nc.sync.dma_start(xt[:], T.xT[:, b * 128 : (b + 1) * 128])
                    ph = php.tile([128, OW1], F32)
                    nc.tensor.matmul(ph[:], xt[:], W1_t[:], start=True, stop=True)
                    psr = psrp.tile([128, H], F32)
                    nc.tensor.matmul(psr[:], xt[:], W1ar_t[:], start=True, stop=True)
                    nc.vector.tensor_copy(h_nm[:, b, :], ph[:])
                    nc.scalar.activation(g1_nm[:, b, :], psr[:], EXP)

            with tc.tile_pool(name="stage" + s, bufs=1) as stp:
                st = stp.tile([128, NBLK, 128], BF16, tag="stage")
                nc.vector.memset(st[:], 0.0)
                nc.vector.tensor_copy(
                    st[:, : NBLK - 1, 0:H], g1_nm[:, : NBLK - 1, :]
                )
                nv = N - 128 * (NBLK - 1)
                if nv > 0:
                    nc.vector.tensor_copy(
                        st[0:nv, NBLK - 1, 0:H], g1_nm[0:nv, NBLK - 1, :]
                    )
                nc.sync.dma_start(
                    T.g1_tab.ap().rearrange("(b p) c -> p b c", p=128), st[:]
                )

            with tc.tile_pool(name="zu1" + s, bufs=3) as zup:

                def zflush1(w, po, zup=zup):
                    u_t = zup.tile([128, H], F32, tag="u")
                    nc.vector.reciprocal(u_t[:], po[:, 0:H])
                    rows = min(128, SLICE - 128 * w)
                    nc.sync.dma_start(
                        T.u1_sl[w * 128 : w * 128 + rows, :], u_t[0:rows, :]
                    )

                _spmm(nc, tc, zB, 32, T.zidx_d, T.zrel_d, T.g1_tab, 128, 8, 8,
                      iof_t, "z1" + s, zflush1, skip=("z" in SKIP), bufs=2)

            nc.gpsimd.collective_compute(
                "AllGather", BYPASS, groups,
                ins=[T.u1_sl[:].opt()], outs=[T.u1_full[0:N, :].opt()],
            )
            zt = sp.tile([NPAD - N, H], F32, tag="zpad")
            nc.vector.memset(zt[:], 0.0)
            nc.sync.dma_start(T.u1_full[N:NPAD, :], zt[:])

            u1_nm = pp.tile([128, NBLK, H], F32)
            nc.sync.dma_start(
                u1_nm[:], T.u1_full.ap().rearrange("(b p) c -> p b c", p=128)
            )
            with tc.tile_pool(name="hhp" + s, bufs=3) as hhp:
                for b in range(NBLK):
                    hh = hhp.tile([128, OW1], BF16)
                    for hd in range(H):
                        nc.vector.tensor_scalar(
                            hh[:, hd * F : (hd + 1) * F],
                            h_nm[:, b, hd * F : (hd + 1) * F],
                            u1_nm[:, b, hd : hd + 1],
                            None,
                            MULT,
                        )
                    nc.sync.dma_start(
                        T.hh1_tab.ap().rearrange("(b p) c -> p b c", p=128)[:, b, :],
                        hh[:],
                    )

        # ============ layer 1 aggregation + layer 2 (h_nm freed) ============
        with (
            tc.tile_pool(name="persist2" + s, bufs=1) as pp2,
            tc.tile_pool(name="small2" + s, bufs=3) as sp2,
        ):
            iof2 = pp2.tile([128, 128], F32)
            nc.sync.dma_start(iof2[:], T.iota_f[:])
            id2 = pp2.tile([128, 128], F32)
            nc.sync.dma_start(id2[:], T.ident[:])
            W2cat = pp2.tile([F, C + 1], F32)
            nc.sync.dma_start(W2cat[:, 0:C], T.W2[:])
            with tc.tile_pool(name="ptr2" + s, bufs=2, space="PSUM") as ptr:
                a2rc_t = sp2.tile([F, 1], F32, tag="T.a2rc")
                nc.sync.dma_start(a2rc_t[:], T.a2rc[:])
                pt = ptr.tile([128, 128], F32, tag="pt2")
                nc.tensor.transpose(pt[0:C, :], W2cat[:, 0:C], id2[:])
                w2t = sp2.tile([128, 128], F32, tag="w2t")
                nc.vector.tensor_copy(w2t[0:C, :], pt[0:C, :])
                pv = ptr.tile([128, 1], F32, tag="pv2")
                nc.tensor.matmul(
                    pv[:], w2t[0:C, :], a2rc_t[0:C, :], start=True, stop=True
                )
                nc.vector.tensor_copy(W2cat[:, C : C + 1], pv[:])

            h1T_sb = pp2.tile([128, NWIN * 128], F32)

            with (
                tc.tile_pool(name="gwp" + s, bufs=2) as gwp,
                tc.tile_pool(name="ptw" + s, bufs=2, space="PSUM") as ptw,
                tc.tile_pool(name="flush" + s, bufs=2) as flp,
            ):
                gwi = gwp.tile([128, NWIN * 8], I16, tag="gwi")
                nc.sync.dma_start(gwi[:], T.gwidx_d[:])
                gwb = gwp.tile([128, NWIN, 128], BF16, tag="gwb")
                nc.gpsimd.dma_gather(
                    gwb[:], T.g1_tab[:], gwi[:], NWIN * 128, NWIN * 128, 128,
                    single_packet=False,
                )
                gwf = gwp.tile([128, NWIN, 128], F32, tag="gwf")
                nc.vector.tensor_copy(gwf[:], gwb[:])

                def flush1(w, po):
                    o_t = flp.tile([128, OW1], F32, tag="o")
                    for hd in range(H):
                        nc.vector.tensor_scalar(
                            o_t[:, hd * F : (hd + 1) * F],
                            po[:, hd * F : (hd + 1) * F],
                            gwf[:, w, hd : hd + 1],
                            None, MULT,
                        )
                    # elu(x) = relu(x) + exp(min(x,0)) - 1 ; h1 = mean_heads
                    neg = flp.tile([128, OW1], F32, tag="neg")
                    nc.vector.tensor_scalar(neg[:], o_t[:], 0.0, None, MIN)
                    ex = flp.tile([128, OW1], F32, tag="ex")
                    nc.scalar.activation(ex[:], neg[:], EXP)
                    rl = flp.tile([128, OW1], F32, tag="rl")
                    nc.vector.tensor_relu(rl[:], o_t[:])
                    su = flp.tile([128, OW1], F32, tag="su")
                    nc.vector.tensor_tensor(su[:], rl[:], ex[:], ADD)
                    t01 = flp.tile([128, F], F32, tag="t01")
                    nc.vector.tensor_tensor(t01[:], su[:, 0:F], su[:, F : 2 * F], ADD)
                    t23 = flp.tile([128, F], F32, tag="t23")
                    nc.vector.tensor_tensor(
                        t23[:], su[:, 2 * F : 3 * F], su[:, 3 * F :], ADD
                    )
                    h1_t = flp.tile([128, F], F32, tag="h1")
                    nc.vector.tensor_tensor(h1_t[:], t01[:], t23[:], ADD)
                    nc.vector.tensor_scalar(h1_t[:], h1_t[:], 0.25, -1.0, MULT, ADD)
                    ptt = ptw.tile([128, 128], F32, tag="ptt")
                    nc.tensor.transpose(ptt[:], h1_t[:], id2[:])
                    nc.vector.tensor_copy(h1T_sb[:, w * 128 : (w + 1) * 128], ptt[:])

                _spmm(nc, tc, bB, CHUNK, T.bidx_d, T.brelf_d, T.hh1_tab, OW1,
                      OW1, OW1, iof2, "a1" + s, flush1, skip=("agg1" in SKIP),
                      bufs=3)

            nc.sync.dma_start(T.h1T_loc[:], h1T_sb[:, 0:SLICE])
            nc.gpsimd.collective_compute(
                "AllGather", BYPASS, groups,
                ins=[T.h1T_loc[:].opt()], outs=[T.h1T_ag[:].opt()],
            )
            h1T_full = pp2.tile([128, P, SLICE], F32)
            nc.sync.dma_start(h1T_full[:], T.h1T_ag.ap().rearrange("s f n -> f s n"))
            h1T_flat = h1T_full[:].rearrange("f s n -> f (s n)")

            h2_nm = pp2.tile([128, NBLK, C], F32)
            g2_nm = pp2.tile([128, NBLK, 1], F32)
            with tc.tile_pool(name="ph2" + s, bufs=2, space="PSUM") as ph2p:
                for b in range(NBLK):
                    nv = max(0, min(128, N - b * 128))
                    if nv < 128:
                        nc.vector.memset(h2_nm[:, b, :], 0.0)
                        nc.vector.memset(g2_nm[:, b, :], 0.0)
                    if nv == 0:
                        continue
                    ph2 = ph2p.tile([128, C + 1], F32)
                    nc.tensor.matmul(
                        ph2[0:nv, :],
                        h1T_flat[:, b * 128 : b * 128 + nv],
                        W2cat[:],
                        start=True,
                        stop=True,
                    )
                    nc.vector.tensor_copy(h2_nm[0:nv, b, :], ph2[0:nv, 0:C])
                    nc.scalar.activation(g2_nm[0:nv, b, :], ph2[0:nv, C : C + 1], EXP)

            with tc.tile_pool(name="stage2" + s, bufs=1) as stp:
                st = stp.tile([128, NBLK, 128], BF16, tag="stage2")
                nc.vector.memset(st[:], 0.0)
                nc.vector.tensor_copy(st[:, :, 0:1], g2_nm[:])
                nc.sync.dma_start(
                    T.g2_tab.ap().rearrange("(b p) c -> p b c", p=128), st[:]
                )

            with tc.tile_pool(name="zu2" + s, bufs=3) as zup:

                def zflush2(w, po, zup=zup):
                    u_t = zup.tile([128, 1], F32, tag="u2")
                    nc.vector.reciprocal(u_t[:], po[:, 0:1])
                    rows = min(128, SLICE - 128 * w)
                    nc.sync.dma_start(
                        T.u2_sl[w * 128 : w * 128 + rows, :], u_t[0:rows, :]
                    )

                _spmm(nc, tc, zB, 32, T.zidx_d, T.zrel_d, T.g2_tab, 128, 8, 8,
                      iof2, "z2" + s, zflush2, skip=("z" in SKIP), bufs=3)

            nc.gpsimd.collective_compute(
                "AllGather", BYPASS, groups,
                ins=[T.u2_sl[:].opt()], outs=[T.u2_full[0:N, :].opt()],
            )
            zt2 = sp2.tile([NPAD - N, 1], F32, tag="zpad2")
            nc.vector.memset(zt2[:], 0.0)
            nc.sync.dma_start(T.u2_full[N:NPAD, :], zt2[:])

            u2_nm = pp2.tile([128, NBLK, 1], F32)
            nc.sync.dma_start(
                u2_nm[:], T.u2_full.ap().rearrange("(b p) c -> p b c", p=128)
            )
            with tc.tile_pool(name="stage3" + s, bufs=1) as stp:
                st = stp.tile([128, NBLK, 128], BF16, tag="stage3")
                nc.vector.memset(st[:], 0.0)
                for b in range(NBLK):
                    nc.vector.tensor_scalar(
                        st[:, b, 0:C], h2_nm[:, b, :], u2_nm[:, b, :], None, MULT
                    )
                nc.sync.dma_start(
                    T.hh2_tab.ap().rearrange("(b p) c -> p b c", p=128), st[:]
                )

            with (
                tc.tile_pool(name="gw2" + s, bufs=2) as gwp,
                tc.tile_pool(name="fl2" + s, bufs=2) as flp,
            ):
                gwi = gwp.tile([128, NWIN * 8], I16, tag="gwi2")
                nc.sync.dma_start(gwi[:], T.gwidx_d[:])
                gwb = gwp.tile([128, NWIN, 128], BF16, tag="gwb2")
                nc.gpsimd.dma_gather(
                    gwb[:], T.g2_tab[:], gwi[:], NWIN * 128, NWIN * 128, 128,
                    single_packet=False,
                )
                gwf = gwp.tile([128, NWIN, 128], F32, tag="gwf2")
                nc.vector.tensor_copy(gwf[:], gwb[:])

                def flush2(w, po):
                    o2 = flp.tile([128, C], F32, tag="o2")
                    nc.vector.tensor_scalar(
                        o2[:], po[:, 0:C], gwf[:, w, 0:1], None, MULT
                    )
                    rows = min(128, SLICE - 128 * w)
                    nc.sync.dma_start(
                        T.out_d[w * 128 : w * 128 + rows, :], o2[0:rows, :]
                    )

                _spmm(nc, tc, bB, 32, T.bidx_d, T.brelf_d, T.hh2_tab, 128,
                      C, C, iof2, "a2" + s, flush2, skip=("agg2" in SKIP),
                      bufs=3)



def _build_program(zB, bB, reps=1):
    nc = bacc.Bacc("TRN2", target_bir_lowering=False, debug=False, num_devices=P)
    groups = [list(range(P))]
    T = _declare(nc, zB, bB)
    with tile.TileContext(nc) as tc:
        for r in range(reps):
            _emit(nc, tc, T, zB, bB, s=str(r))
            if reps > 1:
                with tc.tile_critical():
                    nc.all_core_barrier()
    nc.compile()
    return nc


def _host_inputs(x, W1, a1, W2, a2):
    xT = np.zeros((F, NPAD), np.float32)
    xT[:, :N] = np.ascontiguousarray(np.asarray(x, np.float32).T)
    a1 = np.asarray(a1, np.float32)
    a2 = np.asarray(a2, np.float32)
    a1rc = np.ascontiguousarray(a1[:, F : 2 * F].T)  # [128, H]
    a2rc = np.zeros((F, 1), np.float32)
    a2rc[0:C, 0] = a2[0, C : 2 * C]
    iota = np.tile(np.arange(128, dtype=np.float32), (128, 1))
    return dict(
        xT=xT,
        W1=np.asarray(W1, np.float32),
        W2=np.asarray(W2, np.float32),
        a1rc=a1rc,
        a2rc=a2rc,
        ident=np.eye(128, dtype=np.float32),
        iota_f=np.ascontiguousarray(iota),
    )


def build(x, edge_index, W1, a1, W2, a2, reps=1):
    """Build program + per-core input maps. Returns (nc, in_maps)."""
    ei = np.asarray(edge_index)
    row = ei[0].astype(np.int64)
    col = ei[1].astype(np.int64)
    zB, bB, per_core = _build_edge_inputs(row, col)
    nc = _build_program(zB, bB, reps=reps)
    common = _host_inputs(x, W1, a1, W2, a2)
    in_maps = [{**common, **per_core[k]} for k in range(P)]
    return nc, in_maps


def kernel(x, edge_index, W1, a1, W2, a2):
    nc, in_maps = build(x, edge_index, W1, a1, W2, a2)
    res = run_bass_kernel_spmd(nc, in_maps, list(range(P)))
    return np.concatenate([res.results[k]["out"] for k in range(P)], axis=0)



# revision 8
# speedup vs baseline: 2.4124x; 2.4124x over previous
"""GAT (2-layer) Trainium2 kernel, SPMD across 8 NeuronCores.

Key algebra: segment softmax keyed by row is shift invariant, so the
(h[row] . a_l) term cancels and attention factorizes:
    alpha_e = g[col_e] * u[row_e],
    g[n] = exp(h[n] . a_r),   u[r] = 1 / sum_{e: row=r} g[col_e]
Each GAT layer then needs only two unweighted sparse ops over the fixed
graph:
    z   = A @ g          (segment-sum keyed by row)   -> u = 1/z
    agg = A^T @ (u * h)  (segment-sum keyed by col)
    out = g * agg
Both are done as: dma_gather of table rows per edge (128 edges/block) +
one-hot matmul (lhsT = one-hot of block-relative destination, built by a
DVE is_equal against an iota tile) accumulating into a PSUM window.

Sharding: z-phase edges by row range, aggregation edges by col range (each
core owns its 1250-node output slice). Cross-core: AllGather of u
([10000,H] f32) and of h1^T (5 MB) between the layers.

Host->device traffic is minimized (the axon tunnel is the wall-clock
bottleneck): x is uploaded sharded as bf16 and AllGathered on device,
W1 is bf16, the W@a_r reductions are host-precomputed, gather indices are
uploaded unreplicated [16, n] and replicated to 128 partitions on device,
one-hot keys are bf16, and iota/identity constants are generated on device.

kernel(**inputs) takes FULL inputs and returns the FULL [10000, 22] output.
"""

import sys

sys.path.insert(0, "/opt/trn_rl_repo")

import numpy as np
import ml_dtypes

from concourse import bacc, mybir, tile
from concourse.bass_utils import run_bass_kernel_spmd

F32 = mybir.dt.float32
BF16 = mybir.dt.bfloat16
I16 = mybir.dt.int16
EXP = mybir.ActivationFunctionType.Exp
EQ = mybir.AluOpType.is_equal
MULT = mybir.AluOpType.mult
ADD = mybir.AluOpType.add
MIN = mybir.AluOpType.min
BYPASS = mybir.AluOpType.bypass

N = 10000
E = 320000
F = 128
H = 4
C = 22
P = 8
SLICE = N // P               # 1250 nodes per core
NWIN = (SLICE + 127) // 128  # 10 windows of <=128 dst/src nodes
NBLK = N // 128 + 1          # 79; always >= 1 pad block so row N is zero
NPAD = NBLK * 128            # 10112; table rows >= N are zero
XSH = NPAD // P              # 1264 xT columns uploaded per core
OW1 = H * F                  # 512
CHUNK = 16                   # layer-1 gather chunk (128-edge blocks)
SKIP = set()                 # debug/timing: {"z", "agg1", "agg2"}


def _configure(n, e, p=8):
    """Shrink sizes for simulator debugging (same program structure)."""
    global N, E, P, SLICE, NWIN, NBLK, NPAD, XSH
    N, E, P = n, e, p
    SLICE = N // P
    NWIN = (SLICE + 127) // 128
    NBLK = N // 128 + 1
    NPAD = NBLK * 128
    XSH = NPAD // P


def _cdiv(a, b):
    return (a + b - 1) // b


def _wrap_idxs(idx):
    """dma_gather index layout: logical i at [i%16, i//16] (16 partitions,
    replicated to 128 on device)."""
    n = idx.shape[0]
    assert n % 16 == 0
    return np.ascontiguousarray(idx.reshape(n // 16, 16).T.astype(np.int16))


def _phase_arrays(key, other, nwin):
    """Group one core's (already core-local) edges by 128-wide key window.
    Returns per-window (rel, other) with rel = key - 128*w."""
    w = key >> 7
    order = np.argsort(w, kind="stable")
    key, other, w = key[order], other[order], w[order]
    out = []
    bounds = np.searchsorted(w, np.arange(nwin + 1))
    for i in range(nwin):
        sl = slice(bounds[i], bounds[i + 1])
        k, o = key[sl] - 128 * i, other[sl]
        so = np.argsort(o, kind="stable")  # sorted gather idx -> HBM locality
        out.append((k[so], o[so]))
    return out


def _build_edge_inputs(row, col):
    zraw, braw = [], []
    for k in range(P):
        base = k * SLICE
        m = (row >= base) & (row < base + SLICE)
        zraw.append(_phase_arrays(row[m] - base, col[m], NWIN))
        m = (col >= base) & (col < base + SLICE)
        braw.append(_phase_arrays(col[m] - base, row[m], NWIN))

    def block_counts(raw):
        return [
            max(_cdiv(max(max(len(raw[k][w][0]) for k in range(P)), 1), 128), 1)
            for w in range(NWIN)
        ]

    zB = block_counts(zraw)
    bB = block_counts(braw)

    def pack(raw, B):
        idx_l, rel_l = [], []
        for w in range(NWIN):
            n = B[w] * 128
            rel = np.zeros(n, np.int32)
            oth = np.full(n, N, np.int32)  # dummy -> zero table row
            r, o = raw[w]
            rel[: len(r)] = r
            oth[: len(o)] = o
            idx_l.append(_wrap_idxs(oth))
            rel_l.append(
                rel.reshape(B[w], 128).T.astype(ml_dtypes.bfloat16)
            )  # 0..127: exact in bf16
        return np.concatenate(idx_l, 1), np.concatenate(rel_l, 1)

    per_core = []
    for k in range(P):
        zidx, zrel = pack(zraw[k], zB)
        bidx, brel = pack(braw[k], bB)
        base = k * SLICE
        gw = []
        for w in range(NWIN):
            nid = base + 128 * w + np.arange(128)
            nid = np.where(nid < base + SLICE, nid, N)
            gw.append(_wrap_idxs(nid))
        per_core.append(
            dict(
                zidx=zidx,
                zrel=zrel,
                bidx=bidx,
                brel=brel,
                gwidx=np.concatenate(gw, 1),
            )
        )
    return zB, bB, per_core


def _load_idx(nc, idx_sb, idx16_d):
    """Replicate an unreplicated [16, X] int16 DRAM index array to all 128
    SBUF partitions (8 small DMAs)."""
    for g in range(8):
        nc.sync.dma_start(idx_sb[g * 16 : (g + 1) * 16, :], idx16_d[:])


def _spmm(nc, tc, B, CH, idx_d, rel_d, tab, elem, rhs_w, psum_w, iof_t,
          name, flush, skip=False, bufs=3):
    """One-hot-matmul SpMM over 128-dst windows with gather chunks that span
    window boundaries. flush(w, po) consumes each window's PSUM result.
    idx_d: [16, TOT*8] int16 DRAM; rel_d: [128, TOT] bf16 DRAM."""
    total = sum(B)
    with (
        tc.tile_pool(name=f"gg{name}", bufs=bufs) as ggp,
        tc.tile_pool(name=f"gx{name}", bufs=1) as gxp,
        tc.tile_pool(name=f"go{name}", bufs=bufs) as ohp,
        tc.tile_pool(name=f"gp{name}", bufs=2, space="PSUM") as pp,
    ):
        idx_sb = gxp.tile([128, total * 8], I16, tag="gxi")
        _load_idx(nc, idx_sb, idx_d)
        rel_sb = gxp.tile([128, total], BF16, tag="gxr")
        nc.sync.dma_start(rel_sb[:], rel_d[:])

        gts, ohs = {}, {}
        gb = 0
        for w, Bw in enumerate(B):
            po = pp.tile([128, psum_w], F32, tag="po")
            if skip:
                nc.vector.memset(po[:], 1.0)
                flush(w, po)
                continue
            for b in range(Bw):
                ch, off = divmod(gb, CH)
                if off == 0:
                    cb = min(CH, total - ch * CH)
                    gt = ggp.tile([128, CH, elem], BF16, tag="gg")
                    nc.gpsimd.dma_gather(
                        gt[:, :cb, :], tab[:],
                        idx_sb[:, ch * CH * 8 : (ch * CH + cb) * 8],
                        cb * 128, cb * 128, elem, single_packet=False,
                    )
                    oh = ohp.tile([128, CH, 128], BF16, tag="go")
                    nc.vector.tensor_tensor(
                        oh[:, :cb, :],
                        iof_t[:].rearrange("p (x f) -> p x f", x=1)
                        .broadcast_to([128, cb, 128]),
                        rel_sb[:, ch * CH : ch * CH + cb]
                        .rearrange("p (b x) -> p b x", x=1)
                        .broadcast_to([128, cb, 128]),
                        EQ,
                    )
                    gts[ch], ohs[ch] = gt, oh
                nc.tensor.matmul(
                    po[:], ohs[ch][:, off, :], gts[ch][:, off, 0:rhs_w],
                    start=(b == 0), stop=(b == Bw - 1),
                )
                gb += 1
            flush(w, po)


def _declare(nc, zB, bB):
    ZT, BT = sum(zB), sum(bB)
    T = type("T", (), {})()
    T.xTs = nc.dram_tensor("xTs", [F, XSH], BF16, kind="ExternalInput")
    T.W1 = nc.dram_tensor("W1", [F, OW1], BF16, kind="ExternalInput")
    T.w1a = nc.dram_tensor("w1a", [F, H], BF16, kind="ExternalInput")
    T.W2 = nc.dram_tensor("W2", [F, C], F32, kind="ExternalInput")
    T.w2a = nc.dram_tensor("w2a", [F, 1], F32, kind="ExternalInput")
    T.zidx_d = nc.dram_tensor("zidx", [16, ZT * 8], I16, kind="ExternalInput")
    T.zrel_d = nc.dram_tensor("zrel", [128, ZT], BF16, kind="ExternalInput")
    T.bidx_d = nc.dram_tensor("bidx", [16, BT * 8], I16, kind="ExternalInput")
    T.brel_d = nc.dram_tensor("brel", [128, BT], BF16, kind="ExternalInput")
    T.gwidx_d = nc.dram_tensor("gwidx", [16, NWIN * 8], I16, kind="ExternalInput")
    T.out_d = nc.dram_tensor("out", [SLICE, C], F32, kind="ExternalOutput")

    T.xT_loc = nc.dram_tensor("xT_loc", [F, XSH], BF16)
    T.xT_ag = nc.dram_tensor("xT_ag", [P, F, XSH], BF16, addr_space="Shared")
    T.g1_tab = nc.dram_tensor("g1_tab", [NPAD, 128], BF16)
    T.hh1_tab = nc.dram_tensor("hh1_tab", [NPAD, OW1], BF16)
    T.g2_tab = nc.dram_tensor("g2_tab", [NPAD, 128], BF16)
    T.hh2_tab = nc.dram_tensor("hh2_tab", [NPAD, 128], BF16)
    T.u1_sl = nc.dram_tensor("u1_sl", [SLICE, H], F32)
    T.u2_sl = nc.dram_tensor("u2_sl", [SLICE, 1], F32)
    T.u1_full = nc.dram_tensor("u1_full", [NPAD, H], F32, addr_space="Shared")
    T.u2_full = nc.dram_tensor("u2_full", [NPAD, 1], F32, addr_space="Shared")
    T.h1T_loc = nc.dram_tensor("h1T_loc", [F, SLICE], F32)
    T.h1T_ag = nc.dram_tensor("h1T_ag", [P, F, SLICE], F32, addr_space="Shared")

    return T


def _emit(nc, tc, T, zB, bB, s=""):
        groups = [list(range(P))]
        # ======= constants generated on device (iota / identity) =======
        with tc.tile_pool(name="const" + s, bufs=1) as cp:
            iof_f = cp.tile([128, 128], F32)     # row-iota 0..127, f32
            nc.gpsimd.iota(iof_f[:], [[1, 128]], channel_multiplier=0,
                           allow_small_or_imprecise_dtypes=True)
            pid_f = cp.tile([128, 128], F32)     # partition index, f32
            nc.gpsimd.iota(pid_f[:], [[0, 128]], channel_multiplier=1,
                           allow_small_or_imprecise_dtypes=True)
            id_t = cp.tile([128, 128], F32)      # identity
            nc.vector.tensor_tensor(id_t[:], iof_f[:], pid_f[:], EQ)
            iof_b = cp.tile([128, 128], BF16)    # row-iota, bf16 (one-hot key)
            nc.vector.tensor_copy(iof_b[:], iof_f[:])

            # broadcast x across cores on NeuronLink (upload is sharded);
            # collectives cannot read IO tensors, so bounce through DRAM
            nc.sync.dma_start(T.xT_loc[:], T.xTs[:])
            nc.gpsimd.collective_compute(
                "AllGather", BYPASS, groups,
                ins=[T.xT_loc[:].opt()], outs=[T.xT_ag[:].opt()],
            )

            # ================= layer 1: dense + tables + z1 =================
            with (
                tc.tile_pool(name="persist" + s, bufs=1) as pp,
                tc.tile_pool(name="small" + s, bufs=3) as sp,
            ):
                x_sb = pp.tile([128, P, XSH], BF16)  # full xT, 20.2KB/par
                nc.sync.dma_start(
                    x_sb[:], T.xT_ag.ap().rearrange("s f n -> f s n")
                )
                x_flat = x_sb[:].rearrange("f s n -> f (s n)")
                W1_t = pp.tile([F, OW1], BF16)
                nc.sync.dma_start(W1_t[:], T.W1[:])
                W1ar_t = pp.tile([F, H], BF16)
                nc.sync.dma_start(W1ar_t[:], T.w1a[:])

                h_nm = pp.tile([128, NBLK, OW1], BF16)  # 79x512 bf16/par
                g1_nm = pp.tile([128, NBLK, H], F32)
                with (
                    tc.tile_pool(name="ph" + s, bufs=2, space="PSUM") as php,
                    tc.tile_pool(name="psr" + s, bufs=2, space="PSUM") as psrp,
                ):
                    for b in range(NBLK):
                        xt = x_flat[:, b * 128 : (b + 1) * 128]
                        ph = php.tile([128, OW1], F32)
                        nc.tensor.matmul(ph[:], xt, W1_t[:], start=True, stop=True)
                        psr = psrp.tile([128, H], F32)
                        nc.tensor.matmul(psr[:], xt, W1ar_t[:], start=True, stop=True)
                        nc.vector.tensor_copy(h_nm[:, b, :], ph[:])
                        nc.scalar.activation(g1_nm[:, b, :], psr[:], EXP)

                with tc.tile_pool(name="stage" + s, bufs=1) as stp:
                    st = stp.tile([128, NBLK, 128], BF16, tag="stage")
                    nc.vector.memset(st[:], 0.0)
                    nc.vector.tensor_copy(
                        st[:, : NBLK - 1, 0:H], g1_nm[:, : NBLK - 1, :]
                    )
                    nv = N - 128 * (NBLK - 1)
                    if nv > 0:
                        nc.vector.tensor_copy(
                            st[0:nv, NBLK - 1, 0:H], g1_nm[0:nv, NBLK - 1, :]
                        )
                    nc.sync.dma_start(
                        T.g1_tab.ap().rearrange("(b p) c -> p b c", p=128), st[:]
                    )

                with tc.tile_pool(name="zu1" + s, bufs=3) as zup:

                    def zflush1(w, po, zup=zup):
                        u_t = zup.tile([128, H], F32, tag="u")
                        nc.vector.reciprocal(u_t[:], po[:, 0:H])
                        rows = min(128, SLICE - 128 * w)
                        nc.sync.dma_start(
                            T.u1_sl[w * 128 : w * 128 + rows, :], u_t[0:rows, :]
                        )

                    _spmm(nc, tc, zB, 32, T.zidx_d, T.zrel_d, T.g1_tab, 128, 8, 8,
                          iof_b, "z1" + s, zflush1, skip=("z" in SKIP), bufs=2)

                nc.gpsimd.collective_compute(
                    "AllGather", BYPASS, groups,
                    ins=[T.u1_sl[:].opt()], outs=[T.u1_full[0:N, :].opt()],
                )
                zt = sp.tile([NPAD - N, H], F32, tag="zpad")
                nc.vector.memset(zt[:], 0.0)
                nc.sync.dma_start(T.u1_full[N:NPAD, :], zt[:])

                u1_nm = pp.tile([128, NBLK, H], F32)
                nc.sync.dma_start(
                    u1_nm[:], T.u1_full.ap().rearrange("(b p) c -> p b c", p=128)
                )
                with tc.tile_pool(name="hhp" + s, bufs=3) as hhp:
                    for b in range(NBLK):
                        hh = hhp.tile([128, OW1], BF16)
                        for hd in range(H):
                            nc.vector.tensor_scalar(
                                hh[:, hd * F : (hd + 1) * F],
                                h_nm[:, b, hd * F : (hd + 1) * F],
                                u1_nm[:, b, hd : hd + 1],
                                None,
                                MULT,
                            )
                        nc.sync.dma_start(
                            T.hh1_tab.ap().rearrange("(b p) c -> p b c", p=128)[:, b, :],
                            hh[:],
                        )

            # ============ layer 1 aggregation + layer 2 (h_nm freed) ============
            with (
                tc.tile_pool(name="persist2" + s, bufs=1) as pp2,
                tc.tile_pool(name="small2" + s, bufs=3) as sp2,
            ):
                W2cat = pp2.tile([F, C + 1], F32)
                nc.sync.dma_start(W2cat[:, 0:C], T.W2[:])
                nc.sync.dma_start(W2cat[:, C : C + 1], T.w2a[:])

                h1T_sb = pp2.tile([128, NWIN * 128], F32)

                with (
                    tc.tile_pool(name="gwp" + s, bufs=2) as gwp,
                    tc.tile_pool(name="ptw" + s, bufs=2, space="PSUM") as ptw,
                    tc.tile_pool(name="flush" + s, bufs=2) as flp,
                ):
                    gwi = gwp.tile([128, NWIN * 8], I16, tag="gwi")
                    _load_idx(nc, gwi, T.gwidx_d)
                    gwb = gwp.tile([128, NWIN, 128], BF16, tag="gwb")
                    nc.gpsimd.dma_gather(
                        gwb[:], T.g1_tab[:], gwi[:], NWIN * 128, NWIN * 128, 128,
                        single_packet=False,
                    )
                    gwf = gwp.tile([128, NWIN, 128], F32, tag="gwf")
                    nc.vector.tensor_copy(gwf[:], gwb[:])

                    def flush1(w, po):
                        o_t = flp.tile([128, OW1], F32, tag="o")
                        for hd in range(H):
                            nc.vector.tensor_scalar(
                                o_t[:, hd * F : (hd + 1) * F],
                                po[:, hd * F : (hd + 1) * F],
                                gwf[:, w, hd : hd + 1],
                                None, MULT,
                            )
                        # elu(x) = relu(x) + exp(min(x,0)) - 1 ; h1 = mean_heads
                        neg = flp.tile([128, OW1], F32, tag="neg")
                        nc.vector.tensor_scalar(neg[:], o_t[:], 0.0, None, MIN)
                        ex = flp.tile([128, OW1], F32, tag="ex")
                        nc.scalar.activation(ex[:], neg[:], EXP)
                        rl = flp.tile([128, OW1], F32, tag="rl")
                        nc.vector.tensor_relu(rl[:], o_t[:])
                        su = flp.tile([128, OW1], F32, tag="su")
                        nc.vector.tensor_tensor(su[:], rl[:], ex[:], ADD)
                        t01 = flp.tile([128, F], F32, tag="t01")
                        nc.vector.tensor_tensor(t01[:], su[:, 0:F], su[:, F : 2 * F], ADD)
                        t23 = flp.tile([128, F], F32, tag="t23")
                        nc.vector.tensor_tensor(
                            t23[:], su[:, 2 * F : 3 * F], su[:, 3 * F :], ADD
                        )
                        h1_t = flp.tile([128, F], F32, tag="h1")
                        nc.vector.tensor_tensor(h1_t[:], t01[:], t23[:], ADD)
                        nc.vector.tensor_scalar(h1_t[:], h1_t[:], 0.25, -1.0, MULT, ADD)
                        ptt = ptw.tile([128, 128], F32, tag="ptt")
                        nc.tensor.transpose(ptt[:], h1_t[:], id_t[:])
                        nc.vector.tensor_copy(h1T_sb[:, w * 128 : (w + 1) * 128], ptt[:])

                    _spmm(nc, tc, bB, CHUNK, T.bidx_d, T.brel_d, T.hh1_tab, OW1,
                          OW1, OW1, iof_b, "a1" + s, flush1, skip=("agg1" in SKIP),
                          bufs=3)

                nc.sync.dma_start(T.h1T_loc[:], h1T_sb[:, 0:SLICE])
                nc.gpsimd.collective_compute(
                    "AllGather", BYPASS, groups,
                    ins=[T.h1T_loc[:].opt()], outs=[T.h1T_ag[:].opt()],
                )
                h1T_full = pp2.tile([128, P, SLICE], F32)
                nc.sync.dma_start(h1T_full[:], T.h1T_ag.ap().rearrange("s f n -> f s n"))
                h1T_flat = h1T_full[:].rearrange("f s n -> f (s n)")

                h2_nm = pp2.tile([128, NBLK, C], F32)
                g2_nm = pp2.tile([128, NBLK, 1], F32)
                with tc.tile_pool(name="ph2" + s, bufs=2, space="PSUM") as ph2p:
                    for b in range(NBLK):
                        nv = max(0, min(128, N - b * 128))
                        if nv < 128:
                            nc.vector.memset(h2_nm[:, b, :], 0.0)
                            nc.vector.memset(g2_nm[:, b, :], 0.0)
                        if nv == 0:
                            continue
                        ph2 = ph2p.tile([128, C + 1], F32)
                        nc.tensor.matmul(
                            ph2[0:nv, :],
                            h1T_flat[:, b * 128 : b * 128 + nv],
                            W2cat[:],
                            start=True,
                            stop=True,
                        )
                        nc.vector.tensor_copy(h2_nm[0:nv, b, :], ph2[0:nv, 0:C])
                        nc.scalar.activation(g2_nm[0:nv, b, :], ph2[0:nv, C : C + 1], EXP)

                with tc.tile_pool(name="stage2" + s, bufs=1) as stp:
                    st = stp.tile([128, NBLK, 128], BF16, tag="stage2")
                    nc.vector.memset(st[:], 0.0)
                    nc.vector.tensor_copy(st[:, :, 0:1], g2_nm[:])
                    nc.sync.dma_start(
                        T.g2_tab.ap().rearrange("(b p) c -> p b c", p=128), st[:]
                    )

                with tc.tile_pool(name="zu2" + s, bufs=3) as zup:

                    def zflush2(w, po, zup=zup):
                        u_t = zup.tile([128, 1], F32, tag="u2")
                        nc.vector.reciprocal(u_t[:], po[:, 0:1])
                        rows = min(128, SLICE - 128 * w)
                        nc.sync.dma_start(
                            T.u2_sl[w * 128 : w * 128 + rows, :], u_t[0:rows, :]
                        )

                    _spmm(nc, tc, zB, 32, T.zidx_d, T.zrel_d, T.g2_tab, 128, 8, 8,
                          iof_b, "z2" + s, zflush2, skip=("z" in SKIP), bufs=3)

                nc.gpsimd.collective_compute(
                    "AllGather", BYPASS, groups,
                    ins=[T.u2_sl[:].opt()], outs=[T.u2_full[0:N, :].opt()],
                )
                zt2 = sp2.tile([NPAD - N, 1], F32, tag="zpad2")
                nc.vector.memset(zt2[:], 0.0)
                nc.sync.dma_start(T.u2_full[N:NPAD, :], zt2[:])

                u2_nm = pp2.tile([128, NBLK, 1], F32)
                nc.sync.dma_start(
                    u2_nm[:], T.u2_full.ap().rearrange("(b p) c -> p b c", p=128)
                )
                with tc.tile_pool(name="stage3" + s, bufs=1) as stp:
                    st = stp.tile([128, NBLK, 128], BF16, tag="stage3")
                    nc.vector.memset(st[:], 0.0)
                    for b in range(NBLK):
                        nc.vector.tensor_scalar(
                            st[:, b, 0:C], h2_nm[:, b, :], u2_nm[:, b, :], None, MULT
                        )
                    nc.sync.dma_start(
                        T.hh2_tab.ap().rearrange("(b p) c -> p b c", p=128), st[:]
                    )

                with (
                    tc.tile_pool(name="gw2" + s, bufs=2) as gwp,
                    tc.tile_pool(name="fl2" + s, bufs=2) as flp,
                ):
                    gwi = gwp.tile([128, NWIN * 8], I16, tag="gwi2")
                    _load_idx(nc, gwi, T.gwidx_d)
                    gwb = gwp.tile([128, NWIN, 128], BF16, tag="gwb2")
                    nc.gpsimd.dma_gather(
                        gwb[:], T.g2_tab[:], gwi[:], NWIN * 128, NWIN * 128, 128,
                        single_packet=False,
                    )
                    gwf = gwp.tile([128, NWIN, 128], F32, tag="gwf2")
                    nc.vector.tensor_copy(gwf[:], gwb[:])

                    def flush2(w, po):
                        o2 = flp.tile([128, C], F32, tag="o2")
                        nc.vector.tensor_scalar(
                            o2[:], po[:, 0:C], gwf[:, w, 0:1], None, MULT
                        )
                        rows = min(128, SLICE - 128 * w)
                        nc.sync.dma_start(
                            T.out_d[w * 128 : w * 128 + rows, :], o2[0:rows, :]
                        )

                    _spmm(nc, tc, bB, 32, T.bidx_d, T.brel_d, T.hh2_tab, 128,
                          C, C, iof_b, "a2" + s, flush2, skip=("agg2" in SKIP),
                          bufs=3)


def _build_program(zB, bB, reps=1):
    nc = bacc.Bacc("TRN2", target_bir_lowering=False, debug=False, num_devices=P)
    T = _declare(nc, zB, bB)
    with tile.TileContext(nc) as tc:
        for r in range(reps):
            _emit(nc, tc, T, zB, bB, s=str(r))
            if reps > 1:
                with tc.tile_critical():
                    nc.all_core_barrier()
    nc.compile()
    return nc


def _host_inputs(x, W1, a1, W2, a2):
    x = np.asarray(x, np.float64)
    W1 = np.asarray(W1, np.float64)
    a1 = np.asarray(a1, np.float64)
    W2 = np.asarray(W2, np.float64)
    a2 = np.asarray(a2, np.float64)
    xT = np.zeros((F, NPAD), np.float64)
    xT[:, :N] = x.T
    # host-precomputed per-head W @ a_r reductions
    w1a = np.einsum("fhg,hg->fh", W1.reshape(F, H, F), a1[:, F:])  # [F, H]
    w2a = (W2 @ a2[0, C:]).reshape(F, 1)                           # [F, 1]
    return dict(
        W1=W1.astype(ml_dtypes.bfloat16),
        w1a=w1a.astype(ml_dtypes.bfloat16),
        W2=W2.astype(np.float32),
        w2a=w2a.astype(np.float32),
    ), xT.astype(ml_dtypes.bfloat16)


def build(x, edge_index, W1, a1, W2, a2, reps=1):
    """Build program + per-core input maps. Returns (nc, in_maps)."""
    ei = np.asarray(edge_index)
    row = ei[0].astype(np.int64)
    col = ei[1].astype(np.int64)
    zB, bB, per_core = _build_edge_inputs(row, col)
    nc = _build_program(zB, bB, reps=reps)
    common, xT = _host_inputs(x, W1, a1, W2, a2)
    in_maps = [
        {
            **common,
            **per_core[k],
            "xTs": np.ascontiguousarray(xT[:, k * XSH : (k + 1) * XSH]),
        }
        for k in range(P)
    ]
    return nc, in_maps


def kernel(x, edge_index, W1, a1, W2, a2):
    nc, in_maps = build(x, edge_index, W1, a1, W2, a2)
    res = run_bass_kernel_spmd(nc, in_maps, list(range(P)))
    return np.concatenate([res.results[k]["out"] for k in range(P)], axis=0)


# revision 9
# speedup vs baseline: 5.2981x; 2.1962x over previous
"""GAT (2-layer) Trainium2 kernel, SPMD across 8 NeuronCores.

Key algebra: segment softmax keyed by row is shift invariant, so the
(h[row] . a_l) term cancels and attention factorizes:
    alpha_e = g[col_e] * u[row_e],
    g[n] = exp(h[n] . a_r),   u[r] = 1 / sum_{e: row=r} g[col_e]
Each GAT layer then needs only two unweighted sparse ops over the fixed
graph:
    z   = A @ g          (segment-sum keyed by row)   -> u = 1/z
    agg = A^T @ (u * h)  (segment-sum keyed by col)
    out = g * agg
Both are done as: dma_gather of table rows per edge (128 edges/block) +
one-hot matmul (lhsT = one-hot of block-relative destination, built by a
DVE is_equal against an iota tile) accumulating into a PSUM window.

Sharding: z-phase edges by row range, aggregation edges by col range (each
core owns its 1250-node output slice). Cross-core: AllGather of u
([10000,H] f32) and of h1^T (5 MB) between the layers.

Host->device traffic is minimized (the axon tunnel is the wall-clock
bottleneck): x is uploaded sharded as bf16 and AllGathered on device,
W1 is bf16, the W@a_r reductions are host-precomputed, gather indices are
uploaded unreplicated [16, n] and replicated to 128 partitions on device,
one-hot keys are bf16, and iota/identity constants are generated on device.

kernel(**inputs) takes FULL inputs and returns the FULL [10000, 22] output.
"""

import sys

sys.path.insert(0, "/opt/trn_rl_repo")

import numpy as np
import ml_dtypes

# Persistent XLA compilation cache: run_bass_kernel_spmd re-jits a fresh
# closure every call; with the cache enabled the per-call backend compile
# becomes a ~20ms disk hit instead of ~1s.
try:
    import os
    import tempfile

    import jax

    _cache_dir = os.path.join(tempfile.gettempdir(), "jax_comp_cache")
    jax.config.update("jax_compilation_cache_dir", _cache_dir)
    jax.config.update("jax_persistent_cache_min_compile_time_secs", 0)
    jax.config.update("jax_persistent_cache_min_entry_size_bytes", 0)
except Exception:
    pass

from concourse import bacc, mybir, tile
from concourse.bass_utils import run_bass_kernel_spmd

F32 = mybir.dt.float32
BF16 = mybir.dt.bfloat16
I16 = mybir.dt.int16
EXP = mybir.ActivationFunctionType.Exp
EQ = mybir.AluOpType.is_equal
MULT = mybir.AluOpType.mult
ADD = mybir.AluOpType.add
MIN = mybir.AluOpType.min
BYPASS = mybir.AluOpType.bypass

N = 10000
E = 320000
F = 128
H = 4
C = 22
P = 8
SLICE = N // P               # 1250 nodes per core
NWIN = (SLICE + 127) // 128  # 10 windows of <=128 dst/src nodes
NBLK = N // 128 + 1          # 79; always >= 1 pad block so row N is zero
NPAD = NBLK * 128            # 10112; table rows >= N are zero
XSH = NPAD // P              # 1264 xT columns uploaded per core
OW1 = H * F                  # 512
CHUNK = 16                   # layer-1 gather chunk (128-edge blocks)
SKIP = set()                 # debug/timing: {"z", "agg1", "agg2"}


def _configure(n, e, p=8):
    """Shrink sizes for simulator debugging (same program structure)."""
    global N, E, P, SLICE, NWIN, NBLK, NPAD, XSH
    N, E, P = n, e, p
    SLICE = N // P
    NWIN = (SLICE + 127) // 128
    NBLK = N // 128 + 1
    NPAD = NBLK * 128
    XSH = NPAD // P


def _cdiv(a, b):
    return (a + b - 1) // b


def _wrap_idxs(idx):
    """dma_gather index layout: logical i at [i%16, i//16] (16 partitions,
    replicated to 128 on device)."""
    n = idx.shape[0]
    assert n % 16 == 0
    return np.ascontiguousarray(idx.reshape(n // 16, 16).T.astype(np.int16))


def _phase_arrays(key, other, nwin):
    """Group one core's (already core-local) edges by 128-wide key window.
    Returns per-window (rel, other) with rel = key - 128*w."""
    w = key >> 7
    order = np.argsort(w, kind="stable")
    key, other, w = key[order], other[order], w[order]
    out = []
    bounds = np.searchsorted(w, np.arange(nwin + 1))
    for i in range(nwin):
        sl = slice(bounds[i], bounds[i + 1])
        k, o = key[sl] - 128 * i, other[sl]
        so = np.argsort(o, kind="stable")  # sorted gather idx -> HBM locality
        out.append((k[so], o[so]))
    return out


def _build_edge_inputs(row, col):
    zraw, braw = [], []
    for k in range(P):
        base = k * SLICE
        m = (row >= base) & (row < base + SLICE)
        zraw.append(_phase_arrays(row[m] - base, col[m], NWIN))
        m = (col >= base) & (col < base + SLICE)
        braw.append(_phase_arrays(col[m] - base, row[m], NWIN))

    def block_counts(raw):
        return [
            max(_cdiv(max(max(len(raw[k][w][0]) for k in range(P)), 1), 128), 1)
            for w in range(NWIN)
        ]

    zB = block_counts(zraw)
    bB = block_counts(braw)

    def pack(raw, B):
        idx_l, rel_l = [], []
        for w in range(NWIN):
            n = B[w] * 128
            rel = np.zeros(n, np.int32)
            oth = np.full(n, N, np.int32)  # dummy -> zero table row
            r, o = raw[w]
            rel[: len(r)] = r
            oth[: len(o)] = o
            idx_l.append(_wrap_idxs(oth))
            rel_l.append(
                rel.reshape(B[w], 128).T.astype(ml_dtypes.bfloat16)
            )  # 0..127: exact in bf16
        return np.concatenate(idx_l, 1), np.concatenate(rel_l, 1)

    per_core = []
    for k in range(P):
        zidx, zrel = pack(zraw[k], zB)
        bidx, brel = pack(braw[k], bB)
        base = k * SLICE
        gw = []
        for w in range(NWIN):
            nid = base + 128 * w + np.arange(128)
            nid = np.where(nid < base + SLICE, nid, N)
            gw.append(_wrap_idxs(nid))
        per_core.append(
            dict(
                zidx=zidx,
                zrel=zrel,
                bidx=bidx,
                brel=brel,
                gwidx=np.concatenate(gw, 1),
            )
        )
    return zB, bB, per_core


def _load_idx(nc, idx_sb, idx16_d):
    """Replicate an unreplicated [16, X] int16 DRAM index array to all 128
    SBUF partitions (8 small DMAs)."""
    for g in range(8):
        nc.sync.dma_start(idx_sb[g * 16 : (g + 1) * 16, :], idx16_d[:])


def _spmm(nc, tc, B, CH, idx_d, rel_d, tab, elem, rhs_w, psum_w, iof_t,
          name, flush, skip=False, bufs=3):
    """One-hot-matmul SpMM over 128-dst windows with gather chunks that span
    window boundaries. flush(w, po) consumes each window's PSUM result.
    idx_d: [16, TOT*8] int16 DRAM; rel_d: [128, TOT] bf16 DRAM."""
    total = sum(B)
    with (
        tc.tile_pool(name=f"gg{name}", bufs=bufs) as ggp,
        tc.tile_pool(name=f"gx{name}", bufs=1) as gxp,
        tc.tile_pool(name=f"go{name}", bufs=bufs) as ohp,
        tc.tile_pool(name=f"gp{name}", bufs=2, space="PSUM") as pp,
    ):
        idx_sb = gxp.tile([128, total * 8], I16, tag="gxi")
        _load_idx(nc, idx_sb, idx_d)
        rel_sb = gxp.tile([128, total], BF16, tag="gxr")
        nc.sync.dma_start(rel_sb[:], rel_d[:])

        gts, ohs = {}, {}
        gb = 0
        for w, Bw in enumerate(B):
            po = pp.tile([128, psum_w], F32, tag="po")
            if skip:
                nc.vector.memset(po[:], 1.0)
                flush(w, po)
                continue
            for b in range(Bw):
                ch, off = divmod(gb, CH)
                if off == 0:
                    cb = min(CH, total - ch * CH)
                    gt = ggp.tile([128, CH, elem], BF16, tag="gg")
                    nc.gpsimd.dma_gather(
                        gt[:, :cb, :], tab[:],
                        idx_sb[:, ch * CH * 8 : (ch * CH + cb) * 8],
                        cb * 128, cb * 128, elem, single_packet=False,
                    )
                    oh = ohp.tile([128, CH, 128], BF16, tag="go")
                    nc.vector.tensor_tensor(
                        oh[:, :cb, :],
                        iof_t[:].rearrange("p (x f) -> p x f", x=1)
                        .broadcast_to([128, cb, 128]),
                        rel_sb[:, ch * CH : ch * CH + cb]
                        .rearrange("p (b x) -> p b x", x=1)
                        .broadcast_to([128, cb, 128]),
                        EQ,
                    )
                    gts[ch], ohs[ch] = gt, oh
                nc.tensor.matmul(
                    po[:], ohs[ch][:, off, :], gts[ch][:, off, 0:rhs_w],
                    start=(b == 0), stop=(b == Bw - 1),
                )
                gb += 1
            flush(w, po)


def _declare(nc, zB, bB):
    ZT, BT = sum(zB), sum(bB)
    T = type("T", (), {})()
    T.xTs = nc.dram_tensor("xTs", [F, XSH], BF16, kind="ExternalInput")
    T.W1 = nc.dram_tensor("W1", [F, OW1], BF16, kind="ExternalInput")
    T.w1a = nc.dram_tensor("w1a", [F, H], BF16, kind="ExternalInput")
    T.W2 = nc.dram_tensor("W2", [F, C], F32, kind="ExternalInput")
    T.w2a = nc.dram_tensor("w2a", [F, 1], F32, kind="ExternalInput")
    T.zidx_d = nc.dram_tensor("zidx", [16, ZT * 8], I16, kind="ExternalInput")
    T.zrel_d = nc.dram_tensor("zrel", [128, ZT], BF16, kind="ExternalInput")
    T.bidx_d = nc.dram_tensor("bidx", [16, BT * 8], I16, kind="ExternalInput")
    T.brel_d = nc.dram_tensor("brel", [128, BT], BF16, kind="ExternalInput")
    T.gwidx_d = nc.dram_tensor("gwidx", [16, NWIN * 8], I16, kind="ExternalInput")
    T.out_d = nc.dram_tensor("out", [SLICE, C], F32, kind="ExternalOutput")

    T.xT_loc = nc.dram_tensor("xT_loc", [F, XSH], BF16)
    T.xT_ag = nc.dram_tensor("xT_ag", [P, F, XSH], BF16, addr_space="Shared")
    T.g1_tab = nc.dram_tensor("g1_tab", [NPAD, 128], BF16)
    T.hh1_tab = nc.dram_tensor("hh1_tab", [NPAD, OW1], BF16)
    T.g2_tab = nc.dram_tensor("g2_tab", [NPAD, 128], BF16)
    T.hh2_tab = nc.dram_tensor("hh2_tab", [NPAD, 128], BF16)
    T.u1_sl = nc.dram_tensor("u1_sl", [SLICE, H], F32)
    T.u2_sl = nc.dram_tensor("u2_sl", [SLICE, 1], F32)
    T.u1_full = nc.dram_tensor("u1_full", [NPAD, H], F32, addr_space="Shared")
    T.u2_full = nc.dram_tensor("u2_full", [NPAD, 1], F32, addr_space="Shared")
    T.h1T_loc = nc.dram_tensor("h1T_loc", [F, SLICE], F32)
    T.h1T_ag = nc.dram_tensor("h1T_ag", [P, F, SLICE], F32, addr_space="Shared")

    return T


def _emit(nc, tc, T, zB, bB, s=""):
        groups = [list(range(P))]
        # ======= constants generated on device (iota / identity) =======
        with tc.tile_pool(name="const" + s, bufs=1) as cp:
            iof_f = cp.tile([128, 128], F32)     # row-iota 0..127, f32
            nc.gpsimd.iota(iof_f[:], [[1, 128]], channel_multiplier=0,
                           allow_small_or_imprecise_dtypes=True)
            pid_f = cp.tile([128, 128], F32)     # partition index, f32
            nc.gpsimd.iota(pid_f[:], [[0, 128]], channel_multiplier=1,
                           allow_small_or_imprecise_dtypes=True)
            id_t = cp.tile([128, 128], F32)      # identity
            nc.vector.tensor_tensor(id_t[:], iof_f[:], pid_f[:], EQ)
            iof_b = cp.tile([128, 128], BF16)    # row-iota, bf16 (one-hot key)
            nc.vector.tensor_copy(iof_b[:], iof_f[:])

            # broadcast x across cores on NeuronLink (upload is sharded);
            # collectives cannot read IO tensors, so bounce through DRAM
            nc.sync.dma_start(T.xT_loc[:], T.xTs[:])
            nc.gpsimd.collective_compute(
                "AllGather", BYPASS, groups,
                ins=[T.xT_loc[:].opt()], outs=[T.xT_ag[:].opt()],
            )

            # ================= layer 1: dense + tables + z1 =================
            with (
                tc.tile_pool(name="persist" + s, bufs=1) as pp,
                tc.tile_pool(name="small" + s, bufs=3) as sp,
            ):
                x_sb = pp.tile([128, P, XSH], BF16)  # full xT, 20.2KB/par
                nc.sync.dma_start(
                    x_sb[:], T.xT_ag.ap().rearrange("s f n -> f s n")
                )
                x_flat = x_sb[:].rearrange("f s n -> f (s n)")
                W1_t = pp.tile([F, OW1], BF16)
                nc.sync.dma_start(W1_t[:], T.W1[:])
                W1ar_t = pp.tile([F, H], BF16)
                nc.sync.dma_start(W1ar_t[:], T.w1a[:])

                h_nm = pp.tile([128, NBLK, OW1], BF16)  # 79x512 bf16/par
                g1_nm = pp.tile([128, NBLK, H], F32)
                with (
                    tc.tile_pool(name="ph" + s, bufs=2, space="PSUM") as php,
                    tc.tile_pool(name="psr" + s, bufs=2, space="PSUM") as psrp,
                ):
                    for b in range(NBLK):
                        xt = x_flat[:, b * 128 : (b + 1) * 128]
                        ph = php.tile([128, OW1], F32)
                        nc.tensor.matmul(ph[:], xt, W1_t[:], start=True, stop=True)
                        psr = psrp.tile([128, H], F32)
                        nc.tensor.matmul(psr[:], xt, W1ar_t[:], start=True, stop=True)
                        nc.vector.tensor_copy(h_nm[:, b, :], ph[:])
                        nc.scalar.activation(g1_nm[:, b, :], psr[:], EXP)

                with tc.tile_pool(name="stage" + s, bufs=1) as stp:
                    st = stp.tile([128, NBLK, 128], BF16, tag="stage")
                    nc.vector.memset(st[:], 0.0)
                    nc.vector.tensor_copy(
                        st[:, : NBLK - 1, 0:H], g1_nm[:, : NBLK - 1, :]
                    )
                    nv = N - 128 * (NBLK - 1)
                    if nv > 0:
                        nc.vector.tensor_copy(
                            st[0:nv, NBLK - 1, 0:H], g1_nm[0:nv, NBLK - 1, :]
                        )
                    nc.sync.dma_start(
                        T.g1_tab.ap().rearrange("(b p) c -> p b c", p=128), st[:]
                    )

                with tc.tile_pool(name="zu1" + s, bufs=3) as zup:

                    def zflush1(w, po, zup=zup):
                        u_t = zup.tile([128, H], F32, tag="u")
                        nc.vector.reciprocal(u_t[:], po[:, 0:H])
                        rows = min(128, SLICE - 128 * w)
                        nc.sync.dma_start(
                            T.u1_sl[w * 128 : w * 128 + rows, :], u_t[0:rows, :]
                        )

                    _spmm(nc, tc, zB, 32, T.zidx_d, T.zrel_d, T.g1_tab, 128, 8, 8,
                          iof_b, "z1" + s, zflush1, skip=("z" in SKIP), bufs=2)

                nc.gpsimd.collective_compute(
                    "AllGather", BYPASS, groups,
                    ins=[T.u1_sl[:].opt()], outs=[T.u1_full[0:N, :].opt()],
                )
                zt = sp.tile([NPAD - N, H], F32, tag="zpad")
                nc.vector.memset(zt[:], 0.0)
                nc.sync.dma_start(T.u1_full[N:NPAD, :], zt[:])

                u1_nm = pp.tile([128, NBLK, H], F32)
                nc.sync.dma_start(
                    u1_nm[:], T.u1_full.ap().rearrange("(b p) c -> p b c", p=128)
                )
                with tc.tile_pool(name="hhp" + s, bufs=3) as hhp:
                    for b in range(NBLK):
                        hh = hhp.tile([128, OW1], BF16)
                        for hd in range(H):
                            nc.vector.tensor_scalar(
                                hh[:, hd * F : (hd + 1) * F],
                                h_nm[:, b, hd * F : (hd + 1) * F],
                                u1_nm[:, b, hd : hd + 1],
                                None,
                                MULT,
                            )
                        nc.sync.dma_start(
                            T.hh1_tab.ap().rearrange("(b p) c -> p b c", p=128)[:, b, :],
                            hh[:],
                        )

            # ============ layer 1 aggregation + layer 2 (h_nm freed) ============
            with (
                tc.tile_pool(name="persist2" + s, bufs=1) as pp2,
                tc.tile_pool(name="small2" + s, bufs=3) as sp2,
            ):
                W2cat = pp2.tile([F, C + 1], F32)
                nc.sync.dma_start(W2cat[:, 0:C], T.W2[:])
                nc.sync.dma_start(W2cat[:, C : C + 1], T.w2a[:])

                h1T_sb = pp2.tile([128, NWIN * 128], F32)

                with (
                    tc.tile_pool(name="gwp" + s, bufs=2) as gwp,
                    tc.tile_pool(name="ptw" + s, bufs=2, space="PSUM") as ptw,
                    tc.tile_pool(name="flush" + s, bufs=2) as flp,
                ):
                    gwi = gwp.tile([128, NWIN * 8], I16, tag="gwi")
                    _load_idx(nc, gwi, T.gwidx_d)
                    gwb = gwp.tile([128, NWIN, 128], BF16, tag="gwb")
                    nc.gpsimd.dma_gather(
                        gwb[:], T.g1_tab[:], gwi[:], NWIN * 128, NWIN * 128, 128,
                        single_packet=False,
                    )
                    gwf = gwp.tile([128, NWIN, 128], F32, tag="gwf")
                    nc.vector.tensor_copy(gwf[:], gwb[:])

                    def flush1(w, po):
                        o_t = flp.tile([128, OW1], F32, tag="o")
                        for hd in range(H):
                            nc.vector.tensor_scalar(
                                o_t[:, hd * F : (hd + 1) * F],
                                po[:, hd * F : (hd + 1) * F],
                                gwf[:, w, hd : hd + 1],
                                None, MULT,
                            )
                        # elu(x) = relu(x) + exp(min(x,0)) - 1 ; h1 = mean_heads
                        neg = flp.tile([128, OW1], F32, tag="neg")
                        nc.vector.tensor_scalar(neg[:], o_t[:], 0.0, None, MIN)
                        ex = flp.tile([128, OW1], F32, tag="ex")
                        nc.scalar.activation(ex[:], neg[:], EXP)
                        rl = flp.tile([128, OW1], F32, tag="rl")
                        nc.vector.tensor_relu(rl[:], o_t[:])
                        su = flp.tile([128, OW1], F32, tag="su")
                        nc.vector.tensor_tensor(su[:], rl[:], ex[:], ADD)
                        t01 = flp.tile([128, F], F32, tag="t01")
                        nc.vector.tensor_tensor(t01[:], su[:, 0:F], su[:, F : 2 * F], ADD)
                        t23 = flp.tile([128, F], F32, tag="t23")
                        nc.vector.tensor_tensor(
                            t23[:], su[:, 2 * F : 3 * F], su[:, 3 * F :], ADD
                        )
                        h1_t = flp.tile([128, F], F32, tag="h1")
                        nc.vector.tensor_tensor(h1_t[:], t01[:], t23[:], ADD)
                        nc.vector.tensor_scalar(h1_t[:], h1_t[:], 0.25, -1.0, MULT, ADD)
                        ptt = ptw.tile([128, 128], F32, tag="ptt")
                        nc.tensor.transpose(ptt[:], h1_t[:], id_t[:])
                        nc.vector.tensor_copy(h1T_sb[:, w * 128 : (w + 1) * 128], ptt[:])

                    _spmm(nc, tc, bB, CHUNK, T.bidx_d, T.brel_d, T.hh1_tab, OW1,
                          OW1, OW1, iof_b, "a1" + s, flush1, skip=("agg1" in SKIP),
                          bufs=3)

                nc.sync.dma_start(T.h1T_loc[:], h1T_sb[:, 0:SLICE])
                nc.gpsimd.collective_compute(
                    "AllGather", BYPASS, groups,
                    ins=[T.h1T_loc[:].opt()], outs=[T.h1T_ag[:].opt()],
                )
                h1T_full = pp2.tile([128, P, SLICE], F32)
                nc.sync.dma_start(h1T_full[:], T.h1T_ag.ap().rearrange("s f n -> f s n"))
                h1T_flat = h1T_full[:].rearrange("f s n -> f (s n)")

                h2_nm = pp2.tile([128, NBLK, C], F32)
                g2_nm = pp2.tile([128, NBLK, 1], F32)
                with tc.tile_pool(name="ph2" + s, bufs=2, space="PSUM") as ph2p:
                    for b in range(NBLK):
                        nv = max(0, min(128, N - b * 128))
                        if nv < 128:
                            nc.vector.memset(h2_nm[:, b, :], 0.0)
                            nc.vector.memset(g2_nm[:, b, :], 0.0)
                        if nv == 0:
                            continue
                        ph2 = ph2p.tile([128, C + 1], F32)
                        nc.tensor.matmul(
                            ph2[0:nv, :],
                            h1T_flat[:, b * 128 : b * 128 + nv],
                            W2cat[:],
                            start=True,
                            stop=True,
                        )
                        nc.vector.tensor_copy(h2_nm[0:nv, b, :], ph2[0:nv, 0:C])
                        nc.scalar.activation(g2_nm[0:nv, b, :], ph2[0:nv, C : C + 1], EXP)

                with tc.tile_pool(name="stage2" + s, bufs=1) as stp:
                    st = stp.tile([128, NBLK, 128], BF16, tag="stage2")
                    nc.vector.memset(st[:], 0.0)
                    nc.vector.tensor_copy(st[:, :, 0:1], g2_nm[:])
                    nc.sync.dma_start(
                        T.g2_tab.ap().rearrange("(b p) c -> p b c", p=128), st[:]
                    )

                with tc.tile_pool(name="zu2" + s, bufs=3) as zup:

                    def zflush2(w, po, zup=zup):
                        u_t = zup.tile([128, 1], F32, tag="u2")
                        nc.vector.reciprocal(u_t[:], po[:, 0:1])
                        rows = min(128, SLICE - 128 * w)
                        nc.sync.dma_start(
                            T.u2_sl[w * 128 : w * 128 + rows, :], u_t[0:rows, :]
                        )

                    _spmm(nc, tc, zB, 32, T.zidx_d, T.zrel_d, T.g2_tab, 128, 8, 8,
                          iof_b, "z2" + s, zflush2, skip=("z" in SKIP), bufs=3)

                nc.gpsimd.collective_compute(
                    "AllGather", BYPASS, groups,
                    ins=[T.u2_sl[:].opt()], outs=[T.u2_full[0:N, :].opt()],
                )
                zt2 = sp2.tile([NPAD - N, 1], F32, tag="zpad2")
                nc.vector.memset(zt2[:], 0.0)
                nc.sync.dma_start(T.u2_full[N:NPAD, :], zt2[:])

                u2_nm = pp2.tile([128, NBLK, 1], F32)
                nc.sync.dma_start(
                    u2_nm[:], T.u2_full.ap().rearrange("(b p) c -> p b c", p=128)
                )
                with tc.tile_pool(name="stage3" + s, bufs=1) as stp:
                    st = stp.tile([128, NBLK, 128], BF16, tag="stage3")
                    nc.vector.memset(st[:], 0.0)
                    for b in range(NBLK):
                        nc.vector.tensor_scalar(
                            st[:, b, 0:C], h2_nm[:, b, :], u2_nm[:, b, :], None, MULT
                        )
                    nc.sync.dma_start(
                        T.hh2_tab.ap().rearrange("(b p) c -> p b c", p=128), st[:]
                    )

                with (
                    tc.tile_pool(name="gw2" + s, bufs=2) as gwp,
                    tc.tile_pool(name="fl2" + s, bufs=2) as flp,
                ):
                    gwi = gwp.tile([128, NWIN * 8], I16, tag="gwi2")
                    _load_idx(nc, gwi, T.gwidx_d)
                    gwb = gwp.tile([128, NWIN, 128], BF16, tag="gwb2")
                    nc.gpsimd.dma_gather(
                        gwb[:], T.g2_tab[:], gwi[:], NWIN * 128, NWIN * 128, 128,
                        single_packet=False,
                    )
                    gwf = gwp.tile([128, NWIN, 128], F32, tag="gwf2")
                    nc.vector.tensor_copy(gwf[:], gwb[:])

                    def flush2(w, po):
                        o2 = flp.tile([128, C], F32, tag="o2")
                        nc.vector.tensor_scalar(
                            o2[:], po[:, 0:C], gwf[:, w, 0:1], None, MULT
                        )
                        rows = min(128, SLICE - 128 * w)
                        nc.sync.dma_start(
                            T.out_d[w * 128 : w * 128 + rows, :], o2[0:rows, :]
                        )

                    _spmm(nc, tc, bB, 32, T.bidx_d, T.brel_d, T.hh2_tab, 128,
                          C, C, iof_b, "a2" + s, flush2, skip=("agg2" in SKIP),
                          bufs=3)


def _build_program(zB, bB, reps=1):
    nc = bacc.Bacc("TRN2", target_bir_lowering=False, debug=False, num_devices=P)
    T = _declare(nc, zB, bB)
    with tile.TileContext(nc) as tc:
        for r in range(reps):
            _emit(nc, tc, T, zB, bB, s=str(r))
            if reps > 1:
                with tc.tile_critical():
                    nc.all_core_barrier()
    nc.compile()
    return nc


def _host_inputs(x, W1, a1, W2, a2):
    x = np.asarray(x, np.float64)
    W1 = np.asarray(W1, np.float64)
    a1 = np.asarray(a1, np.float64)
    W2 = np.asarray(W2, np.float64)
    a2 = np.asarray(a2, np.float64)
    xT = np.zeros((F, NPAD), np.float64)
    xT[:, :N] = x.T
    # host-precomputed per-head W @ a_r reductions
    w1a = np.einsum("fhg,hg->fh", W1.reshape(F, H, F), a1[:, F:])  # [F, H]
    w2a = (W2 @ a2[0, C:]).reshape(F, 1)                           # [F, 1]
    return dict(
        W1=W1.astype(ml_dtypes.bfloat16),
        w1a=w1a.astype(ml_dtypes.bfloat16),
        W2=W2.astype(np.float32),
        w2a=w2a.astype(np.float32),
    ), xT.astype(ml_dtypes.bfloat16)


def build(x, edge_index, W1, a1, W2, a2, reps=1):
    """Build program + per-core input maps. Returns (nc, in_maps)."""
    ei = np.asarray(edge_index)
    row = ei[0].astype(np.int64)
    col = ei[1].astype(np.int64)
    zB, bB, per_core = _build_edge_inputs(row, col)
    nc = _build_program(zB, bB, reps=reps)
    common, xT = _host_inputs(x, W1, a1, W2, a2)
    in_maps = [
        {
            **common,
            **per_core[k],
            "xTs": np.ascontiguousarray(xT[:, k * XSH : (k + 1) * XSH]),
        }
        for k in range(P)
    ]
    return nc, in_maps


def kernel(x, edge_index, W1, a1, W2, a2):
    nc, in_maps = build(x, edge_index, W1, a1, W2, a2)
    res = run_bass_kernel_spmd(nc, in_maps, list(range(P)))
    return np.concatenate([res.results[k]["out"] for k in range(P)], axis=0)


# revision 26
# speedup vs baseline: 6.7698x; 1.2778x over previous
"""GAT (2-layer) Trainium2 kernel, SPMD across 8 NeuronCores.

Key algebra: segment softmax keyed by row is shift invariant, so the
(h[row] . a_l) term cancels and attention factorizes:
    alpha_e = g[col_e] * u[row_e],
    g[n] = exp(h[n] . a_r),   u[r] = 1 / sum_{e: row=r} g[col_e]
Each GAT layer then needs only two unweighted sparse ops over the fixed
graph:
    z   = A @ g          (segment-sum keyed by row)   -> u = 1/z
    agg = A^T @ (u * h)  (segment-sum keyed by col)
    out = g * agg
Both are done as: dma_gather of table rows per edge (128 edges/block) +
one-hot matmul (lhsT = one-hot of block-relative destination, built by a
DVE is_equal against an iota tile) accumulating into a PSUM window.

Sharding: z-phase edges by row range, aggregation edges by col range (each
core owns its 1250-node output slice). Cross-core: AllGather of u
([10000,H] f32) and of h1^T (5 MB) between the layers.

Host->device traffic is minimized (the axon tunnel is the wall-clock
bottleneck): x is uploaded sharded as bf16 and AllGathered on device,
W1 is bf16, the W@a_r reductions are host-precomputed, gather indices are
uploaded unreplicated [16, n] and replicated to 128 partitions on device,
one-hot keys are bf16, and iota/identity constants are generated on device.

kernel(**inputs) takes FULL inputs and returns the FULL [10000, 22] output.
"""

import sys

sys.path.insert(0, "/opt/trn_rl_repo")

import numpy as np
import ml_dtypes

# Persistent XLA compilation cache: run_bass_kernel_spmd re-jits a fresh
# closure every call; with the cache enabled the per-call backend compile
# becomes a ~20ms disk hit instead of ~1s.
try:
    import os
    import tempfile

    import jax

    _cache_dir = os.path.join(tempfile.gettempdir(), "jax_comp_cache")
    jax.config.update("jax_compilation_cache_dir", _cache_dir)
    jax.config.update("jax_persistent_cache_min_compile_time_secs", 0)
    jax.config.update("jax_persistent_cache_min_entry_size_bytes", 0)
except Exception:
    pass

from concourse import bacc, mybir, tile
from concourse.bass_utils import run_bass_kernel_spmd

F32 = mybir.dt.float32
BF16 = mybir.dt.bfloat16
I16 = mybir.dt.int16
EXP = mybir.ActivationFunctionType.Exp
EQ = mybir.AluOpType.is_equal
MULT = mybir.AluOpType.mult
ADD = mybir.AluOpType.add
MIN = mybir.AluOpType.min
BYPASS = mybir.AluOpType.bypass

N = 10000
E = 320000
F = 128
H = 4
C = 22
P = 8
SLICE = N // P               # 1250 nodes per core
NWIN = (SLICE + 127) // 128  # 10 windows of <=128 dst/src nodes
NBLK = N // 128 + 1          # 79; always >= 1 pad block so row N is zero
NPAD = NBLK * 128            # 10112; table rows >= N are zero
XSH = NPAD // P              # 1264 xT columns uploaded per core
OW1 = H * F                  # 512
W1PAD = 520                  # W1 (512) + w1a (4) + pad (4), divisible by P
W1SH = W1PAD // P            # 65 W1cat columns uploaded per core
XW = XSH + W1SH              # 1329 combined xT+W1 shard columns
CHUNK = 16                   # layer-1 gather chunk (128-edge blocks)
SKIP = set()                 # debug/timing: {"z", "agg1", "agg2"}


def _configure(n, e, p=8):
    """Shrink sizes for simulator debugging (same program structure)."""
    global N, E, P, SLICE, NWIN, NBLK, NPAD, XSH, XW
    N, E, P = n, e, p
    SLICE = N // P
    NWIN = (SLICE + 127) // 128
    NBLK = N // 128 + 1
    NPAD = NBLK * 128
    XSH = NPAD // P
    XW = XSH + W1SH


def _cdiv(a, b):
    return (a + b - 1) // b


def _wrap_idxs(idx):
    """dma_gather index layout: logical i at [i%16, i//16] (16 partitions,
    replicated to 128 on device)."""
    n = idx.shape[0]
    assert n % 16 == 0
    return np.ascontiguousarray(idx.reshape(n // 16, 16).T.astype(np.int16))


def _phase_arrays(key, other, nwin):
    """Group one core's (already core-local) edges by 128-wide key window.
    Returns per-window (rel, other) with rel = key - 128*w."""
    w = key >> 7
    order = np.argsort(w, kind="stable")
    key, other, w = key[order], other[order], w[order]
    out = []
    bounds = np.searchsorted(w, np.arange(nwin + 1))
    for i in range(nwin):
        sl = slice(bounds[i], bounds[i + 1])
        k, o = key[sl] - 128 * i, other[sl]
        so = np.argsort(o, kind="stable")  # sorted gather idx -> HBM locality
        out.append((k[so], o[so]))
    return out


def _build_edge_inputs(row, col):
    zraw, braw = [], []
    for k in range(P):
        base = k * SLICE
        m = (row >= base) & (row < base + SLICE)
        zraw.append(_phase_arrays(row[m] - base, col[m], NWIN))
        m = (col >= base) & (col < base + SLICE)
        braw.append(_phase_arrays(col[m] - base, row[m], NWIN))

    def block_counts(raw):
        return [
            max(_cdiv(max(max(len(raw[k][w][0]) for k in range(P)), 1), 128), 1)
            for w in range(NWIN)
        ]

    zB = block_counts(zraw)
    bB = block_counts(braw)

    def pack(raw, B):
        idx_l, rel_l = [], []
        for w in range(NWIN):
            n = B[w] * 128
            rel = np.zeros(n, np.int32)
            oth = np.full(n, N, np.int32)  # dummy -> zero table row
            r, o = raw[w]
            rel[: len(r)] = r
            oth[: len(o)] = o
            idx_l.append(_wrap_idxs(oth))
            rel_l.append(
                rel.reshape(B[w], 128).T.astype(ml_dtypes.bfloat16)
            )  # 0..127: exact in bf16
        return np.concatenate(idx_l, 1), np.concatenate(rel_l, 1)

    per_core = []
    for k in range(P):
        zidx, zrel = pack(zraw[k], zB)
        bidx, brel = pack(braw[k], bB)
        base = k * SLICE
        gw = []
        for w in range(NWIN):
            nid = base + 128 * w + np.arange(128)
            nid = np.where(nid < base + SLICE, nid, N)
            gw.append(_wrap_idxs(nid))
        per_core.append(
            dict(
                idx=np.ascontiguousarray(
                    np.concatenate([zidx, bidx] + gw, 1)
                ),
                rel=np.ascontiguousarray(np.concatenate([zrel, brel], 1)),
            )
        )
    return zB, bB, per_core


def _load_idx(nc, idx_sb, idx_ap):
    """Replicate an unreplicated [16, X] int16 DRAM index AP to all 128
    SBUF partitions (8 small DMAs)."""
    for g in range(8):
        nc.sync.dma_start(idx_sb[g * 16 : (g + 1) * 16, :], idx_ap)


def _spmm(nc, tc, B, CH, idx_ap, rel_ap, tab, elem, rhs_w, psum_w, iof_t,
          name, flush, skip=False, bufs=3):
    """One-hot-matmul SpMM over 128-dst windows with gather chunks that span
    window boundaries. flush(w, po) consumes each window's PSUM result.
    idx_ap: [16, TOT*8] int16 DRAM AP; rel_ap: [128, TOT] bf16 DRAM AP."""
    total = sum(B)
    with (
        tc.tile_pool(name=f"gg{name}", bufs=bufs) as ggp,
        tc.tile_pool(name=f"gx{name}", bufs=1) as gxp,
        tc.tile_pool(name=f"go{name}", bufs=bufs) as ohp,
        tc.tile_pool(name=f"gp{name}", bufs=2, space="PSUM") as pp,
    ):
        idx_sb = gxp.tile([128, total * 8], I16, tag="gxi")
        _load_idx(nc, idx_sb, idx_ap)
        rel_sb = gxp.tile([128, total], BF16, tag="gxr")
        nc.sync.dma_start(rel_sb[:], rel_ap)

        gts, ohs = {}, {}
        gb = 0
        for w, Bw in enumerate(B):
            po = pp.tile([128, psum_w], F32, tag="po")
            if skip:
                nc.vector.memset(po[:], 1.0)
                flush(w, po)
                continue
            for b in range(Bw):
                ch, off = divmod(gb, CH)
                if off == 0:
                    cb = min(CH, total - ch * CH)
                    gt = ggp.tile([128, CH, elem], BF16, tag="gg")
                    nc.gpsimd.dma_gather(
                        gt[:, :cb, :], tab[:],
                        idx_sb[:, ch * CH * 8 : (ch * CH + cb) * 8],
                        cb * 128, cb * 128, elem, single_packet=False,
                    )
                    oh = ohp.tile([128, CH, 128], BF16, tag="go")
                    nc.vector.tensor_tensor(
                        oh[:, :cb, :],
                        iof_t[:].rearrange("p (x f) -> p x f", x=1)
                        .broadcast_to([128, cb, 128]),
                        rel_sb[:, ch * CH : ch * CH + cb]
                        .rearrange("p (b x) -> p b x", x=1)
                        .broadcast_to([128, cb, 128]),
                        EQ,
                    )
                    gts[ch], ohs[ch] = gt, oh
                nc.tensor.matmul(
                    po[:], ohs[ch][:, off, :], gts[ch][:, off, 0:rhs_w],
                    start=(b == 0), stop=(b == Bw - 1),
                )
                gb += 1
            flush(w, po)


def _declare(nc, zB, bB):
    ZT, BT = sum(zB), sum(bB)
    T = type("T", (), {})()
    T.xw = nc.dram_tensor("xw", [F, XW], BF16, kind="ExternalInput")
    T.W2cat = nc.dram_tensor("W2cat", [F, C + 1], F32, kind="ExternalInput")
    T.idx_d = nc.dram_tensor(
        "idx", [16, (ZT + BT + NWIN) * 8], I16, kind="ExternalInput"
    )
    T.rel_d = nc.dram_tensor("rel", [128, ZT + BT], BF16, kind="ExternalInput")
    T.out_d = nc.dram_tensor("out", [SLICE, C], BF16, kind="ExternalOutput")
    # column offsets into idx/rel for the three index groups
    T.zoff, T.boff, T.gwoff = 0, ZT, ZT + BT

    T.xw_loc = nc.dram_tensor("xw_loc", [F, XW], BF16)
    T.xw_ag = nc.dram_tensor("xw_ag", [P, F, XW], BF16, addr_space="Shared")
    T.g1_tab = nc.dram_tensor("g1_tab", [NPAD, 128], BF16)
    T.hh1_tab = nc.dram_tensor("hh1_tab", [NPAD, OW1], BF16)
    T.g2_tab = nc.dram_tensor("g2_tab", [NPAD, 128], BF16)
    T.hh2_tab = nc.dram_tensor("hh2_tab", [NPAD, 128], BF16)
    T.u1_sl = nc.dram_tensor("u1_sl", [SLICE, H], F32)
    T.u2_sl = nc.dram_tensor("u2_sl", [SLICE, 1], F32)
    T.u1_full = nc.dram_tensor("u1_full", [NPAD, H], F32, addr_space="Shared")
    T.u2_full = nc.dram_tensor("u2_full", [NPAD, 1], F32, addr_space="Shared")
    T.h1T_loc = nc.dram_tensor("h1T_loc", [F, SLICE], F32)
    T.h1T_ag = nc.dram_tensor("h1T_ag", [P, F, SLICE], F32, addr_space="Shared")

    return T


def _emit(nc, tc, T, zB, bB, s=""):
        groups = [list(range(P))]
        # ======= constants generated on device (iota / identity) =======
        with tc.tile_pool(name="const" + s, bufs=1) as cp:
            iof_f = cp.tile([128, 128], F32)     # row-iota 0..127, f32
            nc.gpsimd.iota(iof_f[:], [[1, 128]], channel_multiplier=0,
                           allow_small_or_imprecise_dtypes=True)
            pid_f = cp.tile([128, 128], F32)     # partition index, f32
            nc.gpsimd.iota(pid_f[:], [[0, 128]], channel_multiplier=1,
                           allow_small_or_imprecise_dtypes=True)
            id_t = cp.tile([128, 128], F32)      # identity
            nc.vector.tensor_tensor(id_t[:], iof_f[:], pid_f[:], EQ)
            iof_b = cp.tile([128, 128], BF16)    # row-iota, bf16 (one-hot key)
            nc.vector.tensor_copy(iof_b[:], iof_f[:])

            # broadcast x+W1 across cores on NeuronLink (upload is sharded);
            # collectives cannot read IO tensors, so bounce through DRAM
            nc.sync.dma_start(T.xw_loc[:], T.xw[:])
            nc.gpsimd.collective_compute(
                "AllGather", BYPASS, groups,
                ins=[T.xw_loc[:].opt()], outs=[T.xw_ag[:].opt()],
            )

            # ================= layer 1: dense + tables + z1 =================
            with (
                tc.tile_pool(name="persist" + s, bufs=1) as pp,
                tc.tile_pool(name="small" + s, bufs=3) as sp,
            ):
                xw_view = T.xw_ag.ap().rearrange("s f n -> f s n")
                x_sb = pp.tile([128, P, XSH], BF16)  # full xT, 20.2KB/par
                nc.sync.dma_start(x_sb[:], xw_view[:, :, 0:XSH])
                x_flat = x_sb[:].rearrange("f s n -> f (s n)")
                w1_sb = pp.tile([128, P, W1SH], BF16)
                nc.sync.dma_start(w1_sb[:], xw_view[:, :, XSH:XW])
                w1_flat = w1_sb[:].rearrange("f s n -> f (s n)")
                W1_t = w1_flat[:, 0:OW1]
                W1ar_t = w1_flat[:, OW1 : OW1 + H]

                h_nm = pp.tile([128, NBLK, OW1], BF16)  # 79x512 bf16/par
                g1_nm = pp.tile([128, NBLK, H], F32)
                with (
                    tc.tile_pool(name="ph" + s, bufs=2, space="PSUM") as php,
                    tc.tile_pool(name="psr" + s, bufs=2, space="PSUM") as psrp,
                ):
                    for b in range(NBLK):
                        xt = x_flat[:, b * 128 : (b + 1) * 128]
                        ph = php.tile([128, OW1], F32)
                        nc.tensor.matmul(ph[:], xt, W1_t, start=True, stop=True)
                        psr = psrp.tile([128, H], F32)
                        nc.tensor.matmul(psr[:], xt, W1ar_t, start=True, stop=True)
                        nc.vector.tensor_copy(h_nm[:, b, :], ph[:])
                        nc.scalar.activation(g1_nm[:, b, :], psr[:], EXP)

                with tc.tile_pool(name="stage" + s, bufs=1) as stp:
                    st = stp.tile([128, NBLK, 128], BF16, tag="stage")
                    nc.vector.memset(st[:], 0.0)
                    nc.vector.tensor_copy(
                        st[:, : NBLK - 1, 0:H], g1_nm[:, : NBLK - 1, :]
                    )
                    nv = N - 128 * (NBLK - 1)
                    if nv > 0:
                        nc.vector.tensor_copy(
                            st[0:nv, NBLK - 1, 0:H], g1_nm[0:nv, NBLK - 1, :]
                        )
                    nc.sync.dma_start(
                        T.g1_tab.ap().rearrange("(b p) c -> p b c", p=128), st[:]
                    )

                with tc.tile_pool(name="zu1" + s, bufs=3) as zup:

                    def zflush1(w, po, zup=zup):
                        u_t = zup.tile([128, H], F32, tag="u")
                        nc.vector.reciprocal(u_t[:], po[:, 0:H])
                        rows = min(128, SLICE - 128 * w)
                        nc.sync.dma_start(
                            T.u1_sl[w * 128 : w * 128 + rows, :], u_t[0:rows, :]
                        )

                    _spmm(nc, tc, zB, 32,
                          T.idx_d[:, T.zoff * 8 : (T.zoff + sum(zB)) * 8],
                          T.rel_d[:, T.zoff : T.zoff + sum(zB)],
                          T.g1_tab, 128, 8, 8,
                          iof_b, "z1" + s, zflush1, skip=("z" in SKIP), bufs=2)

                nc.gpsimd.collective_compute(
                    "AllGather", BYPASS, groups,
                    ins=[T.u1_sl[:].opt()], outs=[T.u1_full[0:N, :].opt()],
                )
                zt = sp.tile([NPAD - N, H], F32, tag="zpad")
                nc.vector.memset(zt[:], 0.0)
                nc.sync.dma_start(T.u1_full[N:NPAD, :], zt[:])

                u1_nm = pp.tile([128, NBLK, H], F32)
                nc.sync.dma_start(
                    u1_nm[:], T.u1_full.ap().rearrange("(b p) c -> p b c", p=128)
                )
                with tc.tile_pool(name="hhp" + s, bufs=3) as hhp:
                    for b in range(NBLK):
                        hh = hhp.tile([128, OW1], BF16)
                        for hd in range(H):
                            nc.vector.tensor_scalar(
                                hh[:, hd * F : (hd + 1) * F],
                                h_nm[:, b, hd * F : (hd + 1) * F],
                                u1_nm[:, b, hd : hd + 1],
                                None,
                                MULT,
                            )
                        nc.sync.dma_start(
                            T.hh1_tab.ap().rearrange("(b p) c -> p b c", p=128)[:, b, :],
                            hh[:],
                        )

            # ============ layer 1 aggregation + layer 2 (h_nm freed) ============
            with (
                tc.tile_pool(name="persist2" + s, bufs=1) as pp2,
                tc.tile_pool(name="small2" + s, bufs=3) as sp2,
            ):
                W2cat = pp2.tile([F, C + 1], F32)
                nc.sync.dma_start(W2cat[:], T.W2cat[:])

                h1T_sb = pp2.tile([128, NWIN * 128], F32)

                with (
                    tc.tile_pool(name="gwp" + s, bufs=2) as gwp,
                    tc.tile_pool(name="ptw" + s, bufs=2, space="PSUM") as ptw,
                    tc.tile_pool(name="flush" + s, bufs=2) as flp,
                ):
                    gwi = gwp.tile([128, NWIN * 8], I16, tag="gwi")
                    _load_idx(nc, gwi, T.idx_d[:, T.gwoff * 8 :])
                    gwb = gwp.tile([128, NWIN, 128], BF16, tag="gwb")
                    nc.gpsimd.dma_gather(
                        gwb[:], T.g1_tab[:], gwi[:], NWIN * 128, NWIN * 128, 128,
                        single_packet=False,
                    )
                    gwf = gwp.tile([128, NWIN, 128], F32, tag="gwf")
                    nc.vector.tensor_copy(gwf[:], gwb[:])

                    def flush1(w, po):
                        o_t = flp.tile([128, OW1], F32, tag="o")
                        for hd in range(H):
                            nc.vector.tensor_scalar(
                                o_t[:, hd * F : (hd + 1) * F],
                                po[:, hd * F : (hd + 1) * F],
                                gwf[:, w, hd : hd + 1],
                                None, MULT,
                            )
                        # elu(x) = relu(x) + exp(min(x,0)) - 1 ; h1 = mean_heads
                        neg = flp.tile([128, OW1], F32, tag="neg")
                        nc.vector.tensor_scalar(neg[:], o_t[:], 0.0, None, MIN)
                        ex = flp.tile([128, OW1], F32, tag="ex")
                        nc.scalar.activation(ex[:], neg[:], EXP)
                        rl = flp.tile([128, OW1], F32, tag="rl")
                        nc.vector.tensor_relu(rl[:], o_t[:])
                        su = flp.tile([128, OW1], F32, tag="su")
                        nc.vector.tensor_tensor(su[:], rl[:], ex[:], ADD)
                        t01 = flp.tile([128, F], F32, tag="t01")
                        nc.vector.tensor_tensor(t01[:], su[:, 0:F], su[:, F : 2 * F], ADD)
                        t23 = flp.tile([128, F], F32, tag="t23")
                        nc.vector.tensor_tensor(
                            t23[:], su[:, 2 * F : 3 * F], su[:, 3 * F :], ADD
                        )
                        h1_t = flp.tile([128, F], F32, tag="h1")
                        nc.vector.tensor_tensor(h1_t[:], t01[:], t23[:], ADD)
                        nc.vector.tensor_scalar(h1_t[:], h1_t[:], 0.25, -1.0, MULT, ADD)
                        ptt = ptw.tile([128, 128], F32, tag="ptt")
                        nc.tensor.transpose(ptt[:], h1_t[:], id_t[:])
                        nc.vector.tensor_copy(h1T_sb[:, w * 128 : (w + 1) * 128], ptt[:])

                    _spmm(nc, tc, bB, CHUNK,
                          T.idx_d[:, T.boff * 8 : (T.boff + sum(bB)) * 8],
                          T.rel_d[:, T.boff : T.boff + sum(bB)],
                          T.hh1_tab, OW1,
                          OW1, OW1, iof_b, "a1" + s, flush1, skip=("agg1" in SKIP),
                          bufs=3)

                nc.sync.dma_start(T.h1T_loc[:], h1T_sb[:, 0:SLICE])
                nc.gpsimd.collective_compute(
                    "AllGather", BYPASS, groups,
                    ins=[T.h1T_loc[:].opt()], outs=[T.h1T_ag[:].opt()],
                )
                h1T_full = pp2.tile([128, P, SLICE], F32)
                nc.sync.dma_start(h1T_full[:], T.h1T_ag.ap().rearrange("s f n -> f s n"))
                h1T_flat = h1T_full[:].rearrange("f s n -> f (s n)")

                h2_nm = pp2.tile([128, NBLK, C], F32)
                g2_nm = pp2.tile([128, NBLK, 1], F32)
                with tc.tile_pool(name="ph2" + s, bufs=2, space="PSUM") as ph2p:
                    for b in range(NBLK):
                        nv = max(0, min(128, N - b * 128))
                        if nv < 128:
                            nc.vector.memset(h2_nm[:, b, :], 0.0)
                            nc.vector.memset(g2_nm[:, b, :], 0.0)
                        if nv == 0:
                            continue
                        ph2 = ph2p.tile([128, C + 1], F32)
                        nc.tensor.matmul(
                            ph2[0:nv, :],
                            h1T_flat[:, b * 128 : b * 128 + nv],
                            W2cat[:],
                            start=True,
                            stop=True,
                        )
                        nc.vector.tensor_copy(h2_nm[0:nv, b, :], ph2[0:nv, 0:C])
                        nc.scalar.activation(g2_nm[0:nv, b, :], ph2[0:nv, C : C + 1], EXP)

                with tc.tile_pool(name="stage2" + s, bufs=1) as stp:
                    st = stp.tile([128, NBLK, 128], BF16, tag="stage2")
                    nc.vector.memset(st[:], 0.0)
                    nc.vector.tensor_copy(st[:, :, 0:1], g2_nm[:])
                    nc.sync.dma_start(
                        T.g2_tab.ap().rearrange("(b p) c -> p b c", p=128), st[:]
                    )

                with tc.tile_pool(name="zu2" + s, bufs=3) as zup:

                    def zflush2(w, po, zup=zup):
                        u_t = zup.tile([128, 1], F32, tag="u2")
                        nc.vector.reciprocal(u_t[:], po[:, 0:1])
                        rows = min(128, SLICE - 128 * w)
                        nc.sync.dma_start(
                            T.u2_sl[w * 128 : w * 128 + rows, :], u_t[0:rows, :]
                        )

                    _spmm(nc, tc, zB, 32,
                          T.idx_d[:, T.zoff * 8 : (T.zoff + sum(zB)) * 8],
                          T.rel_d[:, T.zoff : T.zoff + sum(zB)],
                          T.g2_tab, 128, 8, 8,
                          iof_b, "z2" + s, zflush2, skip=("z" in SKIP), bufs=3)

                nc.gpsimd.collective_compute(
                    "AllGather", BYPASS, groups,
                    ins=[T.u2_sl[:].opt()], outs=[T.u2_full[0:N, :].opt()],
                )
                zt2 = sp2.tile([NPAD - N, 1], F32, tag="zpad2")
                nc.vector.memset(zt2[:], 0.0)
                nc.sync.dma_start(T.u2_full[N:NPAD, :], zt2[:])

                u2_nm = pp2.tile([128, NBLK, 1], F32)
                nc.sync.dma_start(
                    u2_nm[:], T.u2_full.ap().rearrange("(b p) c -> p b c", p=128)
                )
                with tc.tile_pool(name="stage3" + s, bufs=1) as stp:
                    st = stp.tile([128, NBLK, 128], BF16, tag="stage3")
                    nc.vector.memset(st[:], 0.0)
                    for b in range(NBLK):
                        nc.vector.tensor_scalar(
                            st[:, b, 0:C], h2_nm[:, b, :], u2_nm[:, b, :], None, MULT
                        )
                    nc.sync.dma_start(
                        T.hh2_tab.ap().rearrange("(b p) c -> p b c", p=128), st[:]
                    )

                with (
                    tc.tile_pool(name="gw2" + s, bufs=2) as gwp,
                    tc.tile_pool(name="fl2" + s, bufs=2) as flp,
                ):
                    gwi = gwp.tile([128, NWIN * 8], I16, tag="gwi2")
                    _load_idx(nc, gwi, T.idx_d[:, T.gwoff * 8 :])
                    gwb = gwp.tile([128, NWIN, 128], BF16, tag="gwb2")
                    nc.gpsimd.dma_gather(
                        gwb[:], T.g2_tab[:], gwi[:], NWIN * 128, NWIN * 128, 128,
                        single_packet=False,
                    )
                    gwf = gwp.tile([128, NWIN, 128], F32, tag="gwf2")
                    nc.vector.tensor_copy(gwf[:], gwb[:])

                    def flush2(w, po):
                        o2 = flp.tile([128, C], BF16, tag="o2")
                        nc.vector.tensor_scalar(
                            o2[:], po[:, 0:C], gwf[:, w, 0:1], None, MULT
                        )
                        rows = min(128, SLICE - 128 * w)
                        nc.sync.dma_start(
                            T.out_d[w * 128 : w * 128 + rows, :], o2[0:rows, :]
                        )

                    _spmm(nc, tc, bB, 32,
                          T.idx_d[:, T.boff * 8 : (T.boff + sum(bB)) * 8],
                          T.rel_d[:, T.boff : T.boff + sum(bB)],
                          T.hh2_tab, 128,
                          C, C, iof_b, "a2" + s, flush2, skip=("agg2" in SKIP),
                          bufs=3)


def _build_program(zB, bB, reps=1):
    nc = bacc.Bacc("TRN2", target_bir_lowering=False, debug=False, num_devices=P)
    T = _declare(nc, zB, bB)
    with tile.TileContext(nc) as tc:
        for r in range(reps):
            _emit(nc, tc, T, zB, bB, s=str(r))
            if reps > 1:
                with tc.tile_critical():
                    nc.all_core_barrier()
    nc.compile()
    return nc


def _host_inputs(x, W1, a1, W2, a2):
    x = np.asarray(x, np.float64)
    W1 = np.asarray(W1, np.float64)
    a1 = np.asarray(a1, np.float64)
    W2 = np.asarray(W2, np.float64)
    a2 = np.asarray(a2, np.float64)
    # combined bf16 [F, NPAD + W1PAD]: xT | W1 | w1a | pad, sharded by column
    xw = np.zeros((F, NPAD + W1PAD), np.float64)
    xw[:, :N] = x.T
    xw[:, NPAD : NPAD + OW1] = W1
    # host-precomputed per-head W @ a_r reductions
    xw[:, NPAD + OW1 : NPAD + OW1 + H] = np.einsum(
        "fhg,hg->fh", W1.reshape(F, H, F), a1[:, F:]
    )
    W2cat = np.concatenate([W2, (W2 @ a2[0, C:]).reshape(F, 1)], axis=1)
    return xw.astype(ml_dtypes.bfloat16), W2cat.astype(np.float32)


def build(x, edge_index, W1, a1, W2, a2, reps=1):
    """Build program + per-core input maps. Returns (nc, in_maps)."""
    ei = np.asarray(edge_index)
    row = ei[0].astype(np.int64)
    col = ei[1].astype(np.int64)
    zB, bB, per_core = _build_edge_inputs(row, col)
    nc = _build_program(zB, bB, reps=reps)
    xw, W2cat = _host_inputs(x, W1, a1, W2, a2)
    # shard columns: core k gets xT cols [k*XSH,(k+1)*XSH) + W1cat cols
    # [k*W1SH,(k+1)*W1SH); AllGather + on-device views restore both
    in_maps = [
        {
            **per_core[k],
            "W2cat": W2cat,
            "xw": np.ascontiguousarray(
                np.concatenate(
                    [
                        xw[:, k * XSH : (k + 1) * XSH],
                        xw[:, NPAD + k * W1SH : NPAD + (k + 1) * W1SH],
                    ],
                    axis=1,
                )
            ),
        }
        for k in range(P)
    ]
    return nc, in_maps


def kernel(x, edge_index, W1, a1, W2, a2):
    nc, in_maps = build(x, edge_index, W1, a1, W2, a2)
    res = run_bass_kernel_spmd(nc, in_maps, list(range(P)))
    out = np.concatenate([res.results[k]["out"] for k in range(P)], axis=0)
    return out.astype(np.float32)


# revision 32
# speedup vs baseline: 6.9289x; 1.0235x over previous
"""GAT (2-layer) Trainium2 kernel, SPMD across 8 NeuronCores.

Key algebra: segment softmax keyed by row is shift invariant, so the
(h[row] . a_l) term cancels and attention factorizes:
    alpha_e = g[col_e] * u[row_e],
    g[n] = exp(h[n] . a_r),   u[r] = 1 / sum_{e: row=r} g[col_e]
Each GAT layer then needs only two unweighted sparse ops over the fixed
graph:
    z   = A @ g          (segment-sum keyed by row)   -> u = 1/z
    agg = A^T @ (u * h)  (segment-sum keyed by col)
    out = g * agg
Both are done as: dma_gather of table rows per edge (128 edges/block) +
one-hot matmul (lhsT = one-hot of block-relative destination, built by a
DVE is_equal against an iota tile) accumulating into a PSUM window.

Sharding: z-phase edges by row range, aggregation edges by col range (each
core owns its 1250-node output slice). Cross-core: AllGather of u
([10000,H] f32) and of h1^T (5 MB) between the layers.

Host->device traffic is minimized (the axon tunnel is the wall-clock
bottleneck): x is uploaded sharded as bf16 and AllGathered on device,
W1 is bf16, the W@a_r reductions are host-precomputed, gather indices are
uploaded unreplicated [16, n] and replicated to 128 partitions on device,
one-hot keys are bf16, and iota/identity constants are generated on device.

kernel(**inputs) takes FULL inputs and returns the FULL [10000, 22] output.
"""

import sys

sys.path.insert(0, "/opt/trn_rl_repo")

import numpy as np
import ml_dtypes

# Persistent XLA compilation cache: run_bass_kernel_spmd re-jits a fresh
# closure every call; with the cache enabled the per-call backend compile
# becomes a ~20ms disk hit instead of ~1s.
try:
    import os
    import tempfile

    import jax

    _cache_dir = os.path.join(tempfile.gettempdir(), "jax_comp_cache")
    jax.config.update("jax_compilation_cache_dir", _cache_dir)
    jax.config.update("jax_persistent_cache_min_compile_time_secs", 0)
    jax.config.update("jax_persistent_cache_min_entry_size_bytes", 0)
except Exception:
    pass

from concourse import bacc, mybir, tile
from concourse.bass_utils import run_bass_kernel_spmd

F32 = mybir.dt.float32
BF16 = mybir.dt.bfloat16
I16 = mybir.dt.int16
I8 = mybir.dt.int8
EXP = mybir.ActivationFunctionType.Exp
EQ = mybir.AluOpType.is_equal
MULT = mybir.AluOpType.mult
ADD = mybir.AluOpType.add
MIN = mybir.AluOpType.min
BYPASS = mybir.AluOpType.bypass

N = 10000
E = 320000
F = 128
H = 4
C = 22
P = 8
SLICE = N // P               # 1250 nodes per core
NWIN = (SLICE + 127) // 128  # 10 windows of <=128 dst/src nodes
NBLK = N // 128 + 1          # 79; always >= 1 pad block so row N is zero
NPAD = NBLK * 128            # 10112; table rows >= N are zero
XSH = NPAD // P              # 1264 xT columns uploaded per core
OW1 = H * F                  # 512
W1PAD = 520                  # W1 (512) + w1a (4) + pad (4), divisible by P
W1SH = W1PAD // P            # 65 W1cat columns uploaded per core
XW = XSH + W1SH              # 1329 combined xT+W1 shard columns
CHUNK = 16                   # layer-1 gather chunk (128-edge blocks)
SKIP = set()                 # debug/timing: {"z", "agg1", "agg2"}


def _configure(n, e, p=8):
    """Shrink sizes for simulator debugging (same program structure)."""
    global N, E, P, SLICE, NWIN, NBLK, NPAD, XSH, XW
    N, E, P = n, e, p
    SLICE = N // P
    NWIN = (SLICE + 127) // 128
    NBLK = N // 128 + 1
    NPAD = NBLK * 128
    XSH = NPAD // P
    XW = XSH + W1SH


def _cdiv(a, b):
    return (a + b - 1) // b


def _wrap_idxs(idx):
    """dma_gather index layout: logical i at [i%16, i//16] (16 partitions,
    replicated to 128 on device)."""
    n = idx.shape[0]
    assert n % 16 == 0
    return np.ascontiguousarray(idx.reshape(n // 16, 16).T.astype(np.int16))


def _phase_arrays(key, other, nwin):
    """Group one core's (already core-local) edges by 128-wide key window.
    Returns per-window (rel, other) with rel = key - 128*w."""
    w = key >> 7
    order = np.argsort(w, kind="stable")
    key, other, w = key[order], other[order], w[order]
    out = []
    bounds = np.searchsorted(w, np.arange(nwin + 1))
    for i in range(nwin):
        sl = slice(bounds[i], bounds[i + 1])
        k, o = key[sl] - 128 * i, other[sl]
        so = np.argsort(o, kind="stable")  # sorted gather idx -> HBM locality
        out.append((k[so], o[so]))
    return out


def _build_edge_inputs(row, col):
    zraw, braw = [], []
    for k in range(P):
        base = k * SLICE
        m = (row >= base) & (row < base + SLICE)
        zraw.append(_phase_arrays(row[m] - base, col[m], NWIN))
        m = (col >= base) & (col < base + SLICE)
        braw.append(_phase_arrays(col[m] - base, row[m], NWIN))

    def block_counts(raw):
        return [
            max(_cdiv(max(max(len(raw[k][w][0]) for k in range(P)), 1), 128), 1)
            for w in range(NWIN)
        ]

    zB = block_counts(zraw)
    bB = block_counts(braw)

    def pack(raw, B):
        idx_l, rel_l = [], []
        for w in range(NWIN):
            n = B[w] * 128
            rel = np.zeros(n, np.int32)
            oth = np.full(n, N, np.int32)  # dummy -> zero table row
            r, o = raw[w]
            rel[: len(r)] = r
            oth[: len(o)] = o
            idx_l.append(_wrap_idxs(oth))
            rel_l.append(rel.reshape(B[w], 128).T.astype(np.int8))
        return np.concatenate(idx_l, 1), np.concatenate(rel_l, 1)

    per_core = []
    for k in range(P):
        zidx, zrel = pack(zraw[k], zB)
        bidx, brel = pack(braw[k], bB)
        base = k * SLICE
        gw = []
        for w in range(NWIN):
            nid = base + 128 * w + np.arange(128)
            nid = np.where(nid < base + SLICE, nid, N)
            gw.append(_wrap_idxs(nid))
        per_core.append(
            dict(
                idx=np.ascontiguousarray(
                    np.concatenate([zidx, bidx] + gw, 1)
                ),
                rel=np.ascontiguousarray(np.concatenate([zrel, brel], 1)),
            )
        )
    return zB, bB, per_core


def _load_idx(nc, idx_sb, idx_ap):
    """Replicate an unreplicated [16, X] int16 DRAM index AP to all 128
    SBUF partitions (8 small DMAs)."""
    for g in range(8):
        nc.sync.dma_start(idx_sb[g * 16 : (g + 1) * 16, :], idx_ap)


def _spmm(nc, tc, B, CH, idx_ap, rel_ap, tab, elem, rhs_w, psum_w, iof_t,
          name, flush, skip=False, bufs=3):
    """One-hot-matmul SpMM over 128-dst windows with gather chunks that span
    window boundaries. flush(w, po) consumes each window's PSUM result.
    idx_ap: [16, TOT*8] int16 DRAM AP; rel_ap: [128, TOT] bf16 DRAM AP."""
    total = sum(B)
    with (
        tc.tile_pool(name=f"gg{name}", bufs=bufs) as ggp,
        tc.tile_pool(name=f"gx{name}", bufs=1) as gxp,
        tc.tile_pool(name=f"go{name}", bufs=bufs) as ohp,
        tc.tile_pool(name=f"gp{name}", bufs=2, space="PSUM") as pp,
    ):
        idx_sb = gxp.tile([128, total * 8], I16, tag="gxi")
        _load_idx(nc, idx_sb, idx_ap)
        rel8_sb = gxp.tile([128, total], I8, tag="gxr8")
        nc.sync.dma_start(rel8_sb[:], rel_ap)
        rel_sb = gxp.tile([128, total], BF16, tag="gxr")
        nc.vector.tensor_copy(rel_sb[:], rel8_sb[:])

        gts, ohs = {}, {}
        gb = 0
        for w, Bw in enumerate(B):
            po = pp.tile([128, psum_w], F32, tag="po")
            if skip:
                nc.vector.memset(po[:], 1.0)
                flush(w, po)
                continue
            for b in range(Bw):
                ch, off = divmod(gb, CH)
                if off == 0:
                    cb = min(CH, total - ch * CH)
                    gt = ggp.tile([128, CH, elem], BF16, tag="gg")
                    nc.gpsimd.dma_gather(
                        gt[:, :cb, :], tab[:],
                        idx_sb[:, ch * CH * 8 : (ch * CH + cb) * 8],
                        cb * 128, cb * 128, elem, single_packet=False,
                    )
                    oh = ohp.tile([128, CH, 128], BF16, tag="go")
                    nc.vector.tensor_tensor(
                        oh[:, :cb, :],
                        iof_t[:].rearrange("p (x f) -> p x f", x=1)
                        .broadcast_to([128, cb, 128]),
                        rel_sb[:, ch * CH : ch * CH + cb]
                        .rearrange("p (b x) -> p b x", x=1)
                        .broadcast_to([128, cb, 128]),
                        EQ,
                    )
                    gts[ch], ohs[ch] = gt, oh
                nc.tensor.matmul(
                    po[:], ohs[ch][:, off, :], gts[ch][:, off, 0:rhs_w],
                    start=(b == 0), stop=(b == Bw - 1),
                )
                gb += 1
            flush(w, po)


def _declare(nc, zB, bB):
    ZT, BT = sum(zB), sum(bB)
    T = type("T", (), {})()
    T.xw = nc.dram_tensor("xw", [F, XW], BF16, kind="ExternalInput")
    T.W2cat = nc.dram_tensor("W2cat", [F, C + 1], F32, kind="ExternalInput")
    T.idx_d = nc.dram_tensor(
        "idx", [16, (ZT + BT + NWIN) * 8], I16, kind="ExternalInput"
    )
    T.rel_d = nc.dram_tensor("rel", [128, ZT + BT], I8, kind="ExternalInput")
    T.out_d = nc.dram_tensor("out", [SLICE, C], BF16, kind="ExternalOutput")
    # column offsets into idx/rel for the three index groups
    T.zoff, T.boff, T.gwoff = 0, ZT, ZT + BT

    T.xw_loc = nc.dram_tensor("xw_loc", [F, XW], BF16)
    T.xw_ag = nc.dram_tensor("xw_ag", [P, F, XW], BF16, addr_space="Shared")
    T.g1_tab = nc.dram_tensor("g1_tab", [NPAD, 128], BF16)
    T.hh1_tab = nc.dram_tensor("hh1_tab", [NPAD, OW1], BF16)
    T.g2_tab = nc.dram_tensor("g2_tab", [NPAD, 128], BF16)
    T.hh2_tab = nc.dram_tensor("hh2_tab", [NPAD, 128], BF16)
    T.u1_sl = nc.dram_tensor("u1_sl", [SLICE, H], F32)
    T.u2_sl = nc.dram_tensor("u2_sl", [SLICE, 1], F32)
    T.u1_full = nc.dram_tensor("u1_full", [NPAD, H], F32, addr_space="Shared")
    T.u2_full = nc.dram_tensor("u2_full", [NPAD, 1], F32, addr_space="Shared")
    T.h1T_loc = nc.dram_tensor("h1T_loc", [F, SLICE], F32)
    T.h1T_ag = nc.dram_tensor("h1T_ag", [P, F, SLICE], F32, addr_space="Shared")

    return T


def _emit(nc, tc, T, zB, bB, s=""):
        groups = [list(range(P))]
        # ======= constants generated on device (iota / identity) =======
        with tc.tile_pool(name="const" + s, bufs=1) as cp:
            iof_f = cp.tile([128, 128], F32)     # row-iota 0..127, f32
            nc.gpsimd.iota(iof_f[:], [[1, 128]], channel_multiplier=0,
                           allow_small_or_imprecise_dtypes=True)
            pid_f = cp.tile([128, 128], F32)     # partition index, f32
            nc.gpsimd.iota(pid_f[:], [[0, 128]], channel_multiplier=1,
                           allow_small_or_imprecise_dtypes=True)
            id_t = cp.tile([128, 128], F32)      # identity
            nc.vector.tensor_tensor(id_t[:], iof_f[:], pid_f[:], EQ)
            iof_b = cp.tile([128, 128], BF16)    # row-iota, bf16 (one-hot key)
            nc.vector.tensor_copy(iof_b[:], iof_f[:])

            # broadcast x+W1 across cores on NeuronLink (upload is sharded);
            # collectives cannot read IO tensors, so bounce through DRAM
            nc.sync.dma_start(T.xw_loc[:], T.xw[:])
            nc.gpsimd.collective_compute(
                "AllGather", BYPASS, groups,
                ins=[T.xw_loc[:].opt()], outs=[T.xw_ag[:].opt()],
            )

            # ================= layer 1: dense + tables + z1 =================
            with (
                tc.tile_pool(name="persist" + s, bufs=1) as pp,
                tc.tile_pool(name="small" + s, bufs=3) as sp,
            ):
                xw_view = T.xw_ag.ap().rearrange("s f n -> f s n")
                x_sb = pp.tile([128, P, XSH], BF16)  # full xT, 20.2KB/par
                nc.sync.dma_start(x_sb[:], xw_view[:, :, 0:XSH])
                x_flat = x_sb[:].rearrange("f s n -> f (s n)")
                w1_sb = pp.tile([128, P, W1SH], BF16)
                nc.sync.dma_start(w1_sb[:], xw_view[:, :, XSH:XW])
                w1_flat = w1_sb[:].rearrange("f s n -> f (s n)")
                W1_t = w1_flat[:, 0:OW1]
                W1ar_t = w1_flat[:, OW1 : OW1 + H]

                h_nm = pp.tile([128, NBLK, OW1], BF16)  # 79x512 bf16/par
                g1_nm = pp.tile([128, NBLK, H], F32)
                with (
                    tc.tile_pool(name="ph" + s, bufs=2, space="PSUM") as php,
                    tc.tile_pool(name="psr" + s, bufs=2, space="PSUM") as psrp,
                ):
                    for b in range(NBLK):
                        xt = x_flat[:, b * 128 : (b + 1) * 128]
                        ph = php.tile([128, OW1], F32)
                        nc.tensor.matmul(ph[:], xt, W1_t, start=True, stop=True)
                        psr = psrp.tile([128, H], F32)
                        nc.tensor.matmul(psr[:], xt, W1ar_t, start=True, stop=True)
                        nc.vector.tensor_copy(h_nm[:, b, :], ph[:])
                        nc.scalar.activation(g1_nm[:, b, :], psr[:], EXP)

                with tc.tile_pool(name="stage" + s, bufs=1) as stp:
                    st = stp.tile([128, NBLK, 128], BF16, tag="stage")
                    nc.vector.memset(st[:], 0.0)
                    nc.vector.tensor_copy(
                        st[:, : NBLK - 1, 0:H], g1_nm[:, : NBLK - 1, :]
                    )
                    nv = N - 128 * (NBLK - 1)
                    if nv > 0:
                        nc.vector.tensor_copy(
                            st[0:nv, NBLK - 1, 0:H], g1_nm[0:nv, NBLK - 1, :]
                        )
                    nc.sync.dma_start(
                        T.g1_tab.ap().rearrange("(b p) c -> p b c", p=128), st[:]
                    )

                with tc.tile_pool(name="zu1" + s, bufs=3) as zup:

                    def zflush1(w, po, zup=zup):
                        u_t = zup.tile([128, H], F32, tag="u")
                        nc.vector.reciprocal(u_t[:], po[:, 0:H])
                        rows = min(128, SLICE - 128 * w)
                        nc.sync.dma_start(
                            T.u1_sl[w * 128 : w * 128 + rows, :], u_t[0:rows, :]
                        )

                    _spmm(nc, tc, zB, 32,
                          T.idx_d[:, T.zoff * 8 : (T.zoff + sum(zB)) * 8],
                          T.rel_d[:, T.zoff : T.zoff + sum(zB)],
                          T.g1_tab, 128, 8, 8,
                          iof_b, "z1" + s, zflush1, skip=("z" in SKIP), bufs=2)

                nc.gpsimd.collective_compute(
                    "AllGather", BYPASS, groups,
                    ins=[T.u1_sl[:].opt()], outs=[T.u1_full[0:N, :].opt()],
                )
                zt = sp.tile([NPAD - N, H], F32, tag="zpad")
                nc.vector.memset(zt[:], 0.0)
                nc.sync.dma_start(T.u1_full[N:NPAD, :], zt[:])

                u1_nm = pp.tile([128, NBLK, H], F32)
                nc.sync.dma_start(
                    u1_nm[:], T.u1_full.ap().rearrange("(b p) c -> p b c", p=128)
                )
                with tc.tile_pool(name="hhp" + s, bufs=3) as hhp:
                    for b in range(NBLK):
                        hh = hhp.tile([128, OW1], BF16)
                        for hd in range(H):
                            nc.vector.tensor_scalar(
                                hh[:, hd * F : (hd + 1) * F],
                                h_nm[:, b, hd * F : (hd + 1) * F],
                                u1_nm[:, b, hd : hd + 1],
                                None,
                                MULT,
                            )
                        nc.sync.dma_start(
                            T.hh1_tab.ap().rearrange("(b p) c -> p b c", p=128)[:, b, :],
                            hh[:],
                        )

            # ============ layer 1 aggregation + layer 2 (h_nm freed) ============
            with (
                tc.tile_pool(name="persist2" + s, bufs=1) as pp2,
                tc.tile_pool(name="small2" + s, bufs=3) as sp2,
            ):
                W2cat = pp2.tile([F, C + 1], F32)
                nc.sync.dma_start(W2cat[:], T.W2cat[:])

                h1T_sb = pp2.tile([128, NWIN * 128], F32)

                with (
                    tc.tile_pool(name="gwp" + s, bufs=2) as gwp,
                    tc.tile_pool(name="ptw" + s, bufs=2, space="PSUM") as ptw,
                    tc.tile_pool(name="flush" + s, bufs=2) as flp,
                ):
                    gwi = gwp.tile([128, NWIN * 8], I16, tag="gwi")
                    _load_idx(nc, gwi, T.idx_d[:, T.gwoff * 8 :])
                    gwb = gwp.tile([128, NWIN, 128], BF16, tag="gwb")
                    nc.gpsimd.dma_gather(
                        gwb[:], T.g1_tab[:], gwi[:], NWIN * 128, NWIN * 128, 128,
                        single_packet=False,
                    )
                    gwf = gwp.tile([128, NWIN, 128], F32, tag="gwf")
                    nc.vector.tensor_copy(gwf[:], gwb[:])

                    def flush1(w, po):
                        o_t = flp.tile([128, OW1], F32, tag="o")
                        for hd in range(H):
                            nc.vector.tensor_scalar(
                                o_t[:, hd * F : (hd + 1) * F],
                                po[:, hd * F : (hd + 1) * F],
                                gwf[:, w, hd : hd + 1],
                                None, MULT,
                            )
                        # elu(x) = relu(x) + exp(min(x,0)) - 1 ; h1 = mean_heads
                        neg = flp.tile([128, OW1], F32, tag="neg")
                        nc.vector.tensor_scalar(neg[:], o_t[:], 0.0, None, MIN)
                        ex = flp.tile([128, OW1], F32, tag="ex")
                        nc.scalar.activation(ex[:], neg[:], EXP)
                        rl = flp.tile([128, OW1], F32, tag="rl")
                        nc.vector.tensor_relu(rl[:], o_t[:])
                        su = flp.tile([128, OW1], F32, tag="su")
                        nc.vector.tensor_tensor(su[:], rl[:], ex[:], ADD)
                        t01 = flp.tile([128, F], F32, tag="t01")
                        nc.vector.tensor_tensor(t01[:], su[:, 0:F], su[:, F : 2 * F], ADD)
                        t23 = flp.tile([128, F], F32, tag="t23")
                        nc.vector.tensor_tensor(
                            t23[:], su[:, 2 * F : 3 * F], su[:, 3 * F :], ADD
                        )
                        h1_t = flp.tile([128, F], F32, tag="h1")
                        nc.vector.tensor_tensor(h1_t[:], t01[:], t23[:], ADD)
                        nc.vector.tensor_scalar(h1_t[:], h1_t[:], 0.25, -1.0, MULT, ADD)
                        ptt = ptw.tile([128, 128], F32, tag="ptt")
                        nc.tensor.transpose(ptt[:], h1_t[:], id_t[:])
                        nc.vector.tensor_copy(h1T_sb[:, w * 128 : (w + 1) * 128], ptt[:])

                    _spmm(nc, tc, bB, CHUNK,
                          T.idx_d[:, T.boff * 8 : (T.boff + sum(bB)) * 8],
                          T.rel_d[:, T.boff : T.boff + sum(bB)],
                          T.hh1_tab, OW1,
                          OW1, OW1, iof_b, "a1" + s, flush1, skip=("agg1" in SKIP),
                          bufs=3)

                nc.sync.dma_start(T.h1T_loc[:], h1T_sb[:, 0:SLICE])
                nc.gpsimd.collective_compute(
                    "AllGather", BYPASS, groups,
                    ins=[T.h1T_loc[:].opt()], outs=[T.h1T_ag[:].opt()],
                )
                h1T_full = pp2.tile([128, P, SLICE], F32)
                nc.sync.dma_start(h1T_full[:], T.h1T_ag.ap().rearrange("s f n -> f s n"))
                h1T_flat = h1T_full[:].rearrange("f s n -> f (s n)")

                h2_nm = pp2.tile([128, NBLK, C], F32)
                g2_nm = pp2.tile([128, NBLK, 1], F32)
                with tc.tile_pool(name="ph2" + s, bufs=2, space="PSUM") as ph2p:
                    for b in range(NBLK):
                        nv = max(0, min(128, N - b * 128))
                        if nv < 128:
                            nc.vector.memset(h2_nm[:, b, :], 0.0)
                            nc.vector.memset(g2_nm[:, b, :], 0.0)
                        if nv == 0:
                            continue
                        ph2 = ph2p.tile([128, C + 1], F32)
                        nc.tensor.matmul(
                            ph2[0:nv, :],
                            h1T_flat[:, b * 128 : b * 128 + nv],
                            W2cat[:],
                            start=True,
                            stop=True,
                        )
                        nc.vector.tensor_copy(h2_nm[0:nv, b, :], ph2[0:nv, 0:C])
                        nc.scalar.activation(g2_nm[0:nv, b, :], ph2[0:nv, C : C + 1], EXP)

                with tc.tile_pool(name="stage2" + s, bufs=1) as stp:
                    st = stp.tile([128, NBLK, 128], BF16, tag="stage2")
                    nc.vector.memset(st[:], 0.0)
                    nc.vector.tensor_copy(st[:, :, 0:1], g2_nm[:])
                    nc.sync.dma_start(
                        T.g2_tab.ap().rearrange("(b p) c -> p b c", p=128), st[:]
                    )

                with tc.tile_pool(name="zu2" + s, bufs=3) as zup:

                    def zflush2(w, po, zup=zup):
                        u_t = zup.tile([128, 1], F32, tag="u2")
                        nc.vector.reciprocal(u_t[:], po[:, 0:1])
                        rows = min(128, SLICE - 128 * w)
                        nc.sync.dma_start(
                            T.u2_sl[w * 128 : w * 128 + rows, :], u_t[0:rows, :]
                        )

                    _spmm(nc, tc, zB, 32,
                          T.idx_d[:, T.zoff * 8 : (T.zoff + sum(zB)) * 8],
                          T.rel_d[:, T.zoff : T.zoff + sum(zB)],
                          T.g2_tab, 128, 8, 8,
                          iof_b, "z2" + s, zflush2, skip=("z" in SKIP), bufs=3)

                nc.gpsimd.collective_compute(
                    "AllGather", BYPASS, groups,
                    ins=[T.u2_sl[:].opt()], outs=[T.u2_full[0:N, :].opt()],
                )
                zt2 = sp2.tile([NPAD - N, 1], F32, tag="zpad2")
                nc.vector.memset(zt2[:], 0.0)
                nc.sync.dma_start(T.u2_full[N:NPAD, :], zt2[:])

                u2_nm = pp2.tile([128, NBLK, 1], F32)
                nc.sync.dma_start(
                    u2_nm[:], T.u2_full.ap().rearrange("(b p) c -> p b c", p=128)
                )
                with tc.tile_pool(name="stage3" + s, bufs=1) as stp:
                    st = stp.tile([128, NBLK, 128], BF16, tag="stage3")
                    nc.vector.memset(st[:], 0.0)
                    for b in range(NBLK):
                        nc.vector.tensor_scalar(
                            st[:, b, 0:C], h2_nm[:, b, :], u2_nm[:, b, :], None, MULT
                        )
                    nc.sync.dma_start(
                        T.hh2_tab.ap().rearrange("(b p) c -> p b c", p=128), st[:]
                    )

                with (
                    tc.tile_pool(name="gw2" + s, bufs=2) as gwp,
                    tc.tile_pool(name="fl2" + s, bufs=2) as flp,
                ):
                    gwi = gwp.tile([128, NWIN * 8], I16, tag="gwi2")
                    _load_idx(nc, gwi, T.idx_d[:, T.gwoff * 8 :])
                    gwb = gwp.tile([128, NWIN, 128], BF16, tag="gwb2")
                    nc.gpsimd.dma_gather(
                        gwb[:], T.g2_tab[:], gwi[:], NWIN * 128, NWIN * 128, 128,
                        single_packet=False,
                    )
                    gwf = gwp.tile([128, NWIN, 128], F32, tag="gwf2")
                    nc.vector.tensor_copy(gwf[:], gwb[:])

                    def flush2(w, po):
                        o2 = flp.tile([128, C], BF16, tag="o2")
                        nc.vector.tensor_scalar(
                            o2[:], po[:, 0:C], gwf[:, w, 0:1], None, MULT
                        )
                        rows = min(128, SLICE - 128 * w)
                        nc.sync.dma_start(
                            T.out_d[w * 128 : w * 128 + rows, :], o2[0:rows, :]
                        )

                    _spmm(nc, tc, bB, 32,
                          T.idx_d[:, T.boff * 8 : (T.boff + sum(bB)) * 8],
                          T.rel_d[:, T.boff : T.boff + sum(bB)],
                          T.hh2_tab, 128,
                          C, C, iof_b, "a2" + s, flush2, skip=("agg2" in SKIP),
                          bufs=3)


def _build_program(zB, bB, reps=1):
    nc = bacc.Bacc("TRN2", target_bir_lowering=False, debug=False, num_devices=P)
    T = _declare(nc, zB, bB)
    with tile.TileContext(nc) as tc:
        for r in range(reps):
            _emit(nc, tc, T, zB, bB, s=str(r))
            if reps > 1:
                with tc.tile_critical():
                    nc.all_core_barrier()
    nc.compile()
    return nc


def _host_inputs(x, W1, a1, W2, a2):
    x = np.asarray(x, np.float64)
    W1 = np.asarray(W1, np.float64)
    a1 = np.asarray(a1, np.float64)
    W2 = np.asarray(W2, np.float64)
    a2 = np.asarray(a2, np.float64)
    # combined bf16 [F, NPAD + W1PAD]: xT | W1 | w1a | pad, sharded by column
    xw = np.zeros((F, NPAD + W1PAD), np.float64)
    xw[:, :N] = x.T
    xw[:, NPAD : NPAD + OW1] = W1
    # host-precomputed per-head W @ a_r reductions
    xw[:, NPAD + OW1 : NPAD + OW1 + H] = np.einsum(
        "fhg,hg->fh", W1.reshape(F, H, F), a1[:, F:]
    )
    W2cat = np.concatenate([W2, (W2 @ a2[0, C:]).reshape(F, 1)], axis=1)
    return xw.astype(ml_dtypes.bfloat16), W2cat.astype(np.float32)


def build(x, edge_index, W1, a1, W2, a2, reps=1):
    """Build program + per-core input maps. Returns (nc, in_maps)."""
    ei = np.asarray(edge_index)
    row = ei[0].astype(np.int64)
    col = ei[1].astype(np.int64)
    zB, bB, per_core = _build_edge_inputs(row, col)
    nc = _build_program(zB, bB, reps=reps)
    xw, W2cat = _host_inputs(x, W1, a1, W2, a2)
    # shard columns: core k gets xT cols [k*XSH,(k+1)*XSH) + W1cat cols
    # [k*W1SH,(k+1)*W1SH); AllGather + on-device views restore both
    in_maps = [
        {
            **per_core[k],
            "W2cat": W2cat,
            "xw": np.ascontiguousarray(
                np.concatenate(
                    [
                        xw[:, k * XSH : (k + 1) * XSH],
                        xw[:, NPAD + k * W1SH : NPAD + (k + 1) * W1SH],
                    ],
                    axis=1,
                )
            ),
        }
        for k in range(P)
    ]
    return nc, in_maps


def kernel(x, edge_index, W1, a1, W2, a2):
    nc, in_maps = build(x, edge_index, W1, a1, W2, a2)
    res = run_bass_kernel_spmd(nc, in_maps, list(range(P)))
    out = np.concatenate([res.results[k]["out"] for k in range(P)], axis=0)
    return out.astype(np.float32)


# revision 33
# speedup vs baseline: 7.0373x; 1.0157x over previous
"""GAT (2-layer) Trainium2 kernel, SPMD across 8 NeuronCores.

Key algebra: segment softmax keyed by row is shift invariant, so the
(h[row] . a_l) term cancels and attention factorizes:
    alpha_e = g[col_e] * u[row_e],
    g[n] = exp(h[n] . a_r),   u[r] = 1 / sum_{e: row=r} g[col_e]
Each GAT layer then needs only two unweighted sparse ops over the fixed
graph:
    z   = A @ g          (segment-sum keyed by row)   -> u = 1/z
    agg = A^T @ (u * h)  (segment-sum keyed by col)
    out = g * agg
Both are done as: dma_gather of table rows per edge (128 edges/block) +
one-hot matmul (lhsT = one-hot of block-relative destination, built by a
DVE is_equal against an iota tile) accumulating into a PSUM window.

Sharding: z-phase edges by row range, aggregation edges by col range (each
core owns its 1250-node output slice). Cross-core: AllGather of u
([10000,H] f32) and of h1^T (5 MB) between the layers.

Host->device traffic is minimized (the axon tunnel is the wall-clock
bottleneck): x is uploaded sharded as bf16 and AllGathered on device,
W1 is bf16, the W@a_r reductions are host-precomputed, gather indices are
uploaded unreplicated [16, n] and replicated to 128 partitions on device,
one-hot keys are bf16, and iota/identity constants are generated on device.

kernel(**inputs) takes FULL inputs and returns the FULL [10000, 22] output.
"""

import sys

sys.path.insert(0, "/opt/trn_rl_repo")

import numpy as np
import ml_dtypes

# Persistent XLA compilation cache: run_bass_kernel_spmd re-jits a fresh
# closure every call; with the cache enabled the per-call backend compile
# becomes a ~20ms disk hit instead of ~1s.
try:
    import os
    import tempfile

    import jax

    _cache_dir = os.path.join(tempfile.gettempdir(), "jax_comp_cache")
    jax.config.update("jax_compilation_cache_dir", _cache_dir)
    jax.config.update("jax_persistent_cache_min_compile_time_secs", 0)
    jax.config.update("jax_persistent_cache_min_entry_size_bytes", 0)
except Exception:
    pass
try:
    import concourse.bass2jax  # registers the bass_fast_dispatch config state

    jax.config.update("bass_fast_dispatch", True)
except Exception:
    pass

from concourse import bacc, mybir, tile
from concourse.bass_utils import run_bass_kernel_spmd

F32 = mybir.dt.float32
BF16 = mybir.dt.bfloat16
I16 = mybir.dt.int16
I8 = mybir.dt.int8
EXP = mybir.ActivationFunctionType.Exp
EQ = mybir.AluOpType.is_equal
MULT = mybir.AluOpType.mult
ADD = mybir.AluOpType.add
MIN = mybir.AluOpType.min
BYPASS = mybir.AluOpType.bypass

N = 10000
E = 320000
F = 128
H = 4
C = 22
P = 8
SLICE = N // P               # 1250 nodes per core
NWIN = (SLICE + 127) // 128  # 10 windows of <=128 dst/src nodes
NBLK = N // 128 + 1          # 79; always >= 1 pad block so row N is zero
NPAD = NBLK * 128            # 10112; table rows >= N are zero
XSH = NPAD // P              # 1264 xT columns uploaded per core
OW1 = H * F                  # 512
W1PAD = 520                  # W1 (512) + w1a (4) + pad (4), divisible by P
W1SH = W1PAD // P            # 65 W1cat columns uploaded per core
XW = XSH + W1SH              # 1329 combined xT+W1 shard columns
CHUNK = 16                   # layer-1 gather chunk (128-edge blocks)
SKIP = set()                 # debug/timing: {"z", "agg1", "agg2"}


def _configure(n, e, p=8):
    """Shrink sizes for simulator debugging (same program structure)."""
    global N, E, P, SLICE, NWIN, NBLK, NPAD, XSH, XW
    N, E, P = n, e, p
    SLICE = N // P
    NWIN = (SLICE + 127) // 128
    NBLK = N // 128 + 1
    NPAD = NBLK * 128
    XSH = NPAD // P
    XW = XSH + W1SH


def _cdiv(a, b):
    return (a + b - 1) // b


def _wrap_idxs(idx):
    """dma_gather index layout: logical i at [i%16, i//16] (16 partitions,
    replicated to 128 on device)."""
    n = idx.shape[0]
    assert n % 16 == 0
    return np.ascontiguousarray(idx.reshape(n // 16, 16).T.astype(np.int16))


def _phase_arrays(key, other, nwin):
    """Group one core's (already core-local) edges by 128-wide key window.
    Returns per-window (rel, other) with rel = key - 128*w."""
    w = key >> 7
    order = np.argsort(w, kind="stable")
    key, other, w = key[order], other[order], w[order]
    out = []
    bounds = np.searchsorted(w, np.arange(nwin + 1))
    for i in range(nwin):
        sl = slice(bounds[i], bounds[i + 1])
        k, o = key[sl] - 128 * i, other[sl]
        so = np.argsort(o, kind="stable")  # sorted gather idx -> HBM locality
        out.append((k[so], o[so]))
    return out


def _build_edge_inputs(row, col):
    zraw, braw = [], []
    for k in range(P):
        base = k * SLICE
        m = (row >= base) & (row < base + SLICE)
        zraw.append(_phase_arrays(row[m] - base, col[m], NWIN))
        m = (col >= base) & (col < base + SLICE)
        braw.append(_phase_arrays(col[m] - base, row[m], NWIN))

    def block_counts(raw):
        return [
            max(_cdiv(max(max(len(raw[k][w][0]) for k in range(P)), 1), 128), 1)
            for w in range(NWIN)
        ]

    zB = block_counts(zraw)
    bB = block_counts(braw)

    def pack(raw, B):
        idx_l, rel_l = [], []
        for w in range(NWIN):
            n = B[w] * 128
            rel = np.zeros(n, np.int32)
            oth = np.full(n, N, np.int32)  # dummy -> zero table row
            r, o = raw[w]
            rel[: len(r)] = r
            oth[: len(o)] = o
            idx_l.append(_wrap_idxs(oth))
            rel_l.append(rel.reshape(B[w], 128).T.astype(np.int8))
        return np.concatenate(idx_l, 1), np.concatenate(rel_l, 1)

    per_core = []
    for k in range(P):
        zidx, zrel = pack(zraw[k], zB)
        bidx, brel = pack(braw[k], bB)
        base = k * SLICE
        gw = []
        for w in range(NWIN):
            nid = base + 128 * w + np.arange(128)
            nid = np.where(nid < base + SLICE, nid, N)
            gw.append(_wrap_idxs(nid))
        per_core.append(
            dict(
                idx=np.ascontiguousarray(
                    np.concatenate([zidx, bidx] + gw, 1)
                ),
                rel=np.ascontiguousarray(np.concatenate([zrel, brel], 1)),
            )
        )
    return zB, bB, per_core


def _load_idx(nc, idx_sb, idx_ap):
    """Replicate an unreplicated [16, X] int16 DRAM index AP to all 128
    SBUF partitions (8 small DMAs)."""
    for g in range(8):
        nc.sync.dma_start(idx_sb[g * 16 : (g + 1) * 16, :], idx_ap)


def _spmm(nc, tc, B, CH, idx_ap, rel_ap, tab, elem, rhs_w, psum_w, iof_t,
          name, flush, skip=False, bufs=3):
    """One-hot-matmul SpMM over 128-dst windows with gather chunks that span
    window boundaries. flush(w, po) consumes each window's PSUM result.
    idx_ap: [16, TOT*8] int16 DRAM AP; rel_ap: [128, TOT] bf16 DRAM AP."""
    total = sum(B)
    with (
        tc.tile_pool(name=f"gg{name}", bufs=bufs) as ggp,
        tc.tile_pool(name=f"gx{name}", bufs=1) as gxp,
        tc.tile_pool(name=f"go{name}", bufs=bufs) as ohp,
        tc.tile_pool(name=f"gp{name}", bufs=2, space="PSUM") as pp,
    ):
        idx_sb = gxp.tile([128, total * 8], I16, tag="gxi")
        _load_idx(nc, idx_sb, idx_ap)
        rel8_sb = gxp.tile([128, total], I8, tag="gxr8")
        nc.sync.dma_start(rel8_sb[:], rel_ap)
        rel_sb = gxp.tile([128, total], BF16, tag="gxr")
        nc.vector.tensor_copy(rel_sb[:], rel8_sb[:])

        gts, ohs = {}, {}
        gb = 0
        for w, Bw in enumerate(B):
            po = pp.tile([128, psum_w], F32, tag="po")
            if skip:
                nc.vector.memset(po[:], 1.0)
                flush(w, po)
                continue
            for b in range(Bw):
                ch, off = divmod(gb, CH)
                if off == 0:
                    cb = min(CH, total - ch * CH)
                    gt = ggp.tile([128, CH, elem], BF16, tag="gg")
                    nc.gpsimd.dma_gather(
                        gt[:, :cb, :], tab[:],
                        idx_sb[:, ch * CH * 8 : (ch * CH + cb) * 8],
                        cb * 128, cb * 128, elem, single_packet=False,
                    )
                    oh = ohp.tile([128, CH, 128], BF16, tag="go")
                    nc.vector.tensor_tensor(
                        oh[:, :cb, :],
                        iof_t[:].rearrange("p (x f) -> p x f", x=1)
                        .broadcast_to([128, cb, 128]),
                        rel_sb[:, ch * CH : ch * CH + cb]
                        .rearrange("p (b x) -> p b x", x=1)
                        .broadcast_to([128, cb, 128]),
                        EQ,
                    )
                    gts[ch], ohs[ch] = gt, oh
                nc.tensor.matmul(
                    po[:], ohs[ch][:, off, :], gts[ch][:, off, 0:rhs_w],
                    start=(b == 0), stop=(b == Bw - 1),
                )
                gb += 1
            flush(w, po)


def _declare(nc, zB, bB):
    ZT, BT = sum(zB), sum(bB)
    T = type("T", (), {})()
    T.xw = nc.dram_tensor("xw", [F, XW], BF16, kind="ExternalInput")
    T.W2cat = nc.dram_tensor("W2cat", [F, C + 1], F32, kind="ExternalInput")
    T.idx_d = nc.dram_tensor(
        "idx", [16, (ZT + BT + NWIN) * 8], I16, kind="ExternalInput"
    )
    T.rel_d = nc.dram_tensor("rel", [128, ZT + BT], I8, kind="ExternalInput")
    T.out_d = nc.dram_tensor("out", [SLICE, C], BF16, kind="ExternalOutput")
    # column offsets into idx/rel for the three index groups
    T.zoff, T.boff, T.gwoff = 0, ZT, ZT + BT

    T.xw_loc = nc.dram_tensor("xw_loc", [F, XW], BF16)
    T.xw_ag = nc.dram_tensor("xw_ag", [P, F, XW], BF16, addr_space="Shared")
    T.g1_tab = nc.dram_tensor("g1_tab", [NPAD, 128], BF16)
    T.hh1_tab = nc.dram_tensor("hh1_tab", [NPAD, OW1], BF16)
    T.g2_tab = nc.dram_tensor("g2_tab", [NPAD, 128], BF16)
    T.hh2_tab = nc.dram_tensor("hh2_tab", [NPAD, 128], BF16)
    T.u1_sl = nc.dram_tensor("u1_sl", [SLICE, H], F32)
    T.u2_sl = nc.dram_tensor("u2_sl", [SLICE, 1], F32)
    T.u1_full = nc.dram_tensor("u1_full", [NPAD, H], F32, addr_space="Shared")
    T.u2_full = nc.dram_tensor("u2_full", [NPAD, 1], F32, addr_space="Shared")
    T.h1T_loc = nc.dram_tensor("h1T_loc", [F, SLICE], F32)
    T.h1T_ag = nc.dram_tensor("h1T_ag", [P, F, SLICE], F32, addr_space="Shared")

    return T


def _emit(nc, tc, T, zB, bB, s=""):
        groups = [list(range(P))]
        # ======= constants generated on device (iota / identity) =======
        with tc.tile_pool(name="const" + s, bufs=1) as cp:
            iof_f = cp.tile([128, 128], F32)     # row-iota 0..127, f32
            nc.gpsimd.iota(iof_f[:], [[1, 128]], channel_multiplier=0,
                           allow_small_or_imprecise_dtypes=True)
            pid_f = cp.tile([128, 128], F32)     # partition index, f32
            nc.gpsimd.iota(pid_f[:], [[0, 128]], channel_multiplier=1,
                           allow_small_or_imprecise_dtypes=True)
            id_t = cp.tile([128, 128], F32)      # identity
            nc.vector.tensor_tensor(id_t[:], iof_f[:], pid_f[:], EQ)
            iof_b = cp.tile([128, 128], BF16)    # row-iota, bf16 (one-hot key)
            nc.vector.tensor_copy(iof_b[:], iof_f[:])

            # broadcast x+W1 across cores on NeuronLink (upload is sharded);
            # collectives cannot read IO tensors, so bounce through DRAM
            nc.sync.dma_start(T.xw_loc[:], T.xw[:])
            nc.gpsimd.collective_compute(
                "AllGather", BYPASS, groups,
                ins=[T.xw_loc[:].opt()], outs=[T.xw_ag[:].opt()],
            )

            # ================= layer 1: dense + tables + z1 =================
            with (
                tc.tile_pool(name="persist" + s, bufs=1) as pp,
                tc.tile_pool(name="small" + s, bufs=3) as sp,
            ):
                xw_view = T.xw_ag.ap().rearrange("s f n -> f s n")
                x_sb = pp.tile([128, P, XSH], BF16)  # full xT, 20.2KB/par
                nc.sync.dma_start(x_sb[:], xw_view[:, :, 0:XSH])
                x_flat = x_sb[:].rearrange("f s n -> f (s n)")
                w1_sb = pp.tile([128, P, W1SH], BF16)
                nc.sync.dma_start(w1_sb[:], xw_view[:, :, XSH:XW])
                w1_flat = w1_sb[:].rearrange("f s n -> f (s n)")
                W1_t = w1_flat[:, 0:OW1]
                W1ar_t = w1_flat[:, OW1 : OW1 + H]

                h_nm = pp.tile([128, NBLK, OW1], BF16)  # 79x512 bf16/par
                g1_nm = pp.tile([128, NBLK, H], F32)
                with (
                    tc.tile_pool(name="ph" + s, bufs=2, space="PSUM") as php,
                    tc.tile_pool(name="psr" + s, bufs=2, space="PSUM") as psrp,
                ):
                    for b in range(NBLK):
                        xt = x_flat[:, b * 128 : (b + 1) * 128]
                        ph = php.tile([128, OW1], F32)
                        nc.tensor.matmul(ph[:], xt, W1_t, start=True, stop=True)
                        psr = psrp.tile([128, H], F32)
                        nc.tensor.matmul(psr[:], xt, W1ar_t, start=True, stop=True)
                        nc.vector.tensor_copy(h_nm[:, b, :], ph[:])
                        nc.scalar.activation(g1_nm[:, b, :], psr[:], EXP)

                with tc.tile_pool(name="stage" + s, bufs=1) as stp:
                    st = stp.tile([128, NBLK, 128], BF16, tag="stage")
                    nc.vector.memset(st[:], 0.0)
                    nc.vector.tensor_copy(
                        st[:, : NBLK - 1, 0:H], g1_nm[:, : NBLK - 1, :]
                    )
                    nv = N - 128 * (NBLK - 1)
                    if nv > 0:
                        nc.vector.tensor_copy(
                            st[0:nv, NBLK - 1, 0:H], g1_nm[0:nv, NBLK - 1, :]
                        )
                    nc.sync.dma_start(
                        T.g1_tab.ap().rearrange("(b p) c -> p b c", p=128), st[:]
                    )

                with tc.tile_pool(name="zu1" + s, bufs=3) as zup:

                    def zflush1(w, po, zup=zup):
                        u_t = zup.tile([128, H], F32, tag="u")
                        nc.vector.reciprocal(u_t[:], po[:, 0:H])
                        rows = min(128, SLICE - 128 * w)
                        nc.sync.dma_start(
                            T.u1_sl[w * 128 : w * 128 + rows, :], u_t[0:rows, :]
                        )

                    _spmm(nc, tc, zB, 32,
                          T.idx_d[:, T.zoff * 8 : (T.zoff + sum(zB)) * 8],
                          T.rel_d[:, T.zoff : T.zoff + sum(zB)],
                          T.g1_tab, 128, 8, 8,
                          iof_b, "z1" + s, zflush1, skip=("z" in SKIP), bufs=2)

                nc.gpsimd.collective_compute(
                    "AllGather", BYPASS, groups,
                    ins=[T.u1_sl[:].opt()], outs=[T.u1_full[0:N, :].opt()],
                )
                zt = sp.tile([NPAD - N, H], F32, tag="zpad")
                nc.vector.memset(zt[:], 0.0)
                nc.sync.dma_start(T.u1_full[N:NPAD, :], zt[:])

                u1_nm = pp.tile([128, NBLK, H], F32)
                nc.sync.dma_start(
                    u1_nm[:], T.u1_full.ap().rearrange("(b p) c -> p b c", p=128)
                )
                with tc.tile_pool(name="hhp" + s, bufs=3) as hhp:
                    for b in range(NBLK):
                        hh = hhp.tile([128, OW1], BF16)
                        for hd in range(H):
                            nc.vector.tensor_scalar(
                                hh[:, hd * F : (hd + 1) * F],
                                h_nm[:, b, hd * F : (hd + 1) * F],
                                u1_nm[:, b, hd : hd + 1],
                                None,
                                MULT,
                            )
                        nc.sync.dma_start(
                            T.hh1_tab.ap().rearrange("(b p) c -> p b c", p=128)[:, b, :],
                            hh[:],
                        )

            # ============ layer 1 aggregation + layer 2 (h_nm freed) ============
            with (
                tc.tile_pool(name="persist2" + s, bufs=1) as pp2,
                tc.tile_pool(name="small2" + s, bufs=3) as sp2,
            ):
                W2cat = pp2.tile([F, C + 1], F32)
                nc.sync.dma_start(W2cat[:], T.W2cat[:])

                h1T_sb = pp2.tile([128, NWIN * 128], F32)

                with (
                    tc.tile_pool(name="gwp" + s, bufs=2) as gwp,
                    tc.tile_pool(name="ptw" + s, bufs=2, space="PSUM") as ptw,
                    tc.tile_pool(name="flush" + s, bufs=2) as flp,
                ):
                    gwi = gwp.tile([128, NWIN * 8], I16, tag="gwi")
                    _load_idx(nc, gwi, T.idx_d[:, T.gwoff * 8 :])
                    gwb = gwp.tile([128, NWIN, 128], BF16, tag="gwb")
                    nc.gpsimd.dma_gather(
                        gwb[:], T.g1_tab[:], gwi[:], NWIN * 128, NWIN * 128, 128,
                        single_packet=False,
                    )
                    gwf = gwp.tile([128, NWIN, 128], F32, tag="gwf")
                    nc.vector.tensor_copy(gwf[:], gwb[:])

                    def flush1(w, po):
                        o_t = flp.tile([128, OW1], F32, tag="o")
                        for hd in range(H):
                            nc.vector.tensor_scalar(
                                o_t[:, hd * F : (hd + 1) * F],
                                po[:, hd * F : (hd + 1) * F],
                                gwf[:, w, hd : hd + 1],
                                None, MULT,
                            )
                        # elu(x) = relu(x) + exp(min(x,0)) - 1 ; h1 = mean_heads
                        neg = flp.tile([128, OW1], F32, tag="neg")
                        nc.vector.tensor_scalar(neg[:], o_t[:], 0.0, None, MIN)
                        ex = flp.tile([128, OW1], F32, tag="ex")
                        nc.scalar.activation(ex[:], neg[:], EXP)
                        rl = flp.tile([128, OW1], F32, tag="rl")
                        nc.vector.tensor_relu(rl[:], o_t[:])
                        su = flp.tile([128, OW1], F32, tag="su")
                        nc.vector.tensor_tensor(su[:], rl[:], ex[:], ADD)
                        t01 = flp.tile([128, F], F32, tag="t01")
                        nc.vector.tensor_tensor(t01[:], su[:, 0:F], su[:, F : 2 * F], ADD)
                        t23 = flp.tile([128, F], F32, tag="t23")
                        nc.vector.tensor_tensor(
                            t23[:], su[:, 2 * F : 3 * F], su[:, 3 * F :], ADD
                        )
                        h1_t = flp.tile([128, F], F32, tag="h1")
                        nc.vector.tensor_tensor(h1_t[:], t01[:], t23[:], ADD)
                        nc.vector.tensor_scalar(h1_t[:], h1_t[:], 0.25, -1.0, MULT, ADD)
                        ptt = ptw.tile([128, 128], F32, tag="ptt")
                        nc.tensor.transpose(ptt[:], h1_t[:], id_t[:])
                        nc.vector.tensor_copy(h1T_sb[:, w * 128 : (w + 1) * 128], ptt[:])

                    _spmm(nc, tc, bB, CHUNK,
                          T.idx_d[:, T.boff * 8 : (T.boff + sum(bB)) * 8],
                          T.rel_d[:, T.boff : T.boff + sum(bB)],
                          T.hh1_tab, OW1,
                          OW1, OW1, iof_b, "a1" + s, flush1, skip=("agg1" in SKIP),
                          bufs=3)

                nc.sync.dma_start(T.h1T_loc[:], h1T_sb[:, 0:SLICE])
                nc.gpsimd.collective_compute(
                    "AllGather", BYPASS, groups,
                    ins=[T.h1T_loc[:].opt()], outs=[T.h1T_ag[:].opt()],
                )
                h1T_full = pp2.tile([128, P, SLICE], F32)
                nc.sync.dma_start(h1T_full[:], T.h1T_ag.ap().rearrange("s f n -> f s n"))
                h1T_flat = h1T_full[:].rearrange("f s n -> f (s n)")

                h2_nm = pp2.tile([128, NBLK, C], F32)
                g2_nm = pp2.tile([128, NBLK, 1], F32)
                with tc.tile_pool(name="ph2" + s, bufs=2, space="PSUM") as ph2p:
                    for b in range(NBLK):
                        nv = max(0, min(128, N - b * 128))
                        if nv < 128:
                            nc.vector.memset(h2_nm[:, b, :], 0.0)
                            nc.vector.memset(g2_nm[:, b, :], 0.0)
                        if nv == 0:
                            continue
                        ph2 = ph2p.tile([128, C + 1], F32)
                        nc.tensor.matmul(
                            ph2[0:nv, :],
                            h1T_flat[:, b * 128 : b * 128 + nv],
                            W2cat[:],
                            start=True,
                            stop=True,
                        )
                        nc.vector.tensor_copy(h2_nm[0:nv, b, :], ph2[0:nv, 0:C])
                        nc.scalar.activation(g2_nm[0:nv, b, :], ph2[0:nv, C : C + 1], EXP)

                with tc.tile_pool(name="stage2" + s, bufs=1) as stp:
                    st = stp.tile([128, NBLK, 128], BF16, tag="stage2")
                    nc.vector.memset(st[:], 0.0)
                    nc.vector.tensor_copy(st[:, :, 0:1], g2_nm[:])
                    nc.sync.dma_start(
                        T.g2_tab.ap().rearrange("(b p) c -> p b c", p=128), st[:]
                    )

                with tc.tile_pool(name="zu2" + s, bufs=3) as zup:

                    def zflush2(w, po, zup=zup):
                        u_t = zup.tile([128, 1], F32, tag="u2")
                        nc.vector.reciprocal(u_t[:], po[:, 0:1])
                        rows = min(128, SLICE - 128 * w)
                        nc.sync.dma_start(
                            T.u2_sl[w * 128 : w * 128 + rows, :], u_t[0:rows, :]
                        )

                    _spmm(nc, tc, zB, 32,
                          T.idx_d[:, T.zoff * 8 : (T.zoff + sum(zB)) * 8],
                          T.rel_d[:, T.zoff : T.zoff + sum(zB)],
                          T.g2_tab, 128, 8, 8,
                          iof_b, "z2" + s, zflush2, skip=("z" in SKIP), bufs=3)

                nc.gpsimd.collective_compute(
                    "AllGather", BYPASS, groups,
                    ins=[T.u2_sl[:].opt()], outs=[T.u2_full[0:N, :].opt()],
                )
                zt2 = sp2.tile([NPAD - N, 1], F32, tag="zpad2")
                nc.vector.memset(zt2[:], 0.0)
                nc.sync.dma_start(T.u2_full[N:NPAD, :], zt2[:])

                u2_nm = pp2.tile([128, NBLK, 1], F32)
                nc.sync.dma_start(
                    u2_nm[:], T.u2_full.ap().rearrange("(b p) c -> p b c", p=128)
                )
                with tc.tile_pool(name="stage3" + s, bufs=1) as stp:
                    st = stp.tile([128, NBLK, 128], BF16, tag="stage3")
                    nc.vector.memset(st[:], 0.0)
                    for b in range(NBLK):
                        nc.vector.tensor_scalar(
                            st[:, b, 0:C], h2_nm[:, b, :], u2_nm[:, b, :], None, MULT
                        )
                    nc.sync.dma_start(
                        T.hh2_tab.ap().rearrange("(b p) c -> p b c", p=128), st[:]
                    )

                with (
                    tc.tile_pool(name="gw2" + s, bufs=2) as gwp,
                    tc.tile_pool(name="fl2" + s, bufs=2) as flp,
                ):
                    gwi = gwp.tile([128, NWIN * 8], I16, tag="gwi2")
                    _load_idx(nc, gwi, T.idx_d[:, T.gwoff * 8 :])
                    gwb = gwp.tile([128, NWIN, 128], BF16, tag="gwb2")
                    nc.gpsimd.dma_gather(
                        gwb[:], T.g2_tab[:], gwi[:], NWIN * 128, NWIN * 128, 128,
                        single_packet=False,
                    )
                    gwf = gwp.tile([128, NWIN, 128], F32, tag="gwf2")
                    nc.vector.tensor_copy(gwf[:], gwb[:])

                    def flush2(w, po):
                        o2 = flp.tile([128, C], BF16, tag="o2")
                        nc.vector.tensor_scalar(
                            o2[:], po[:, 0:C], gwf[:, w, 0:1], None, MULT
                        )
                        rows = min(128, SLICE - 128 * w)
                        nc.sync.dma_start(
                            T.out_d[w * 128 : w * 128 + rows, :], o2[0:rows, :]
                        )

                    _spmm(nc, tc, bB, 32,
                          T.idx_d[:, T.boff * 8 : (T.boff + sum(bB)) * 8],
                          T.rel_d[:, T.boff : T.boff + sum(bB)],
                          T.hh2_tab, 128,
                          C, C, iof_b, "a2" + s, flush2, skip=("agg2" in SKIP),
                          bufs=3)


def _build_program(zB, bB, reps=1):
    nc = bacc.Bacc("TRN2", target_bir_lowering=False, debug=False, num_devices=P)
    T = _declare(nc, zB, bB)
    with tile.TileContext(nc) as tc:
        for r in range(reps):
            _emit(nc, tc, T, zB, bB, s=str(r))
            if reps > 1:
                with tc.tile_critical():
                    nc.all_core_barrier()
    nc.compile()
    return nc


def _host_inputs(x, W1, a1, W2, a2):
    x = np.asarray(x, np.float64)
    W1 = np.asarray(W1, np.float64)
    a1 = np.asarray(a1, np.float64)
    W2 = np.asarray(W2, np.float64)
    a2 = np.asarray(a2, np.float64)
    # combined bf16 [F, NPAD + W1PAD]: xT | W1 | w1a | pad, sharded by column
    xw = np.zeros((F, NPAD + W1PAD), np.float64)
    xw[:, :N] = x.T
    xw[:, NPAD : NPAD + OW1] = W1
    # host-precomputed per-head W @ a_r reductions
    xw[:, NPAD + OW1 : NPAD + OW1 + H] = np.einsum(
        "fhg,hg->fh", W1.reshape(F, H, F), a1[:, F:]
    )
    W2cat = np.concatenate([W2, (W2 @ a2[0, C:]).reshape(F, 1)], axis=1)
    return xw.astype(ml_dtypes.bfloat16), W2cat.astype(np.float32)


def build(x, edge_index, W1, a1, W2, a2, reps=1):
    """Build program + per-core input maps. Returns (nc, in_maps)."""
    ei = np.asarray(edge_index)
    row = ei[0].astype(np.int64)
    col = ei[1].astype(np.int64)
    zB, bB, per_core = _build_edge_inputs(row, col)
    nc = _build_program(zB, bB, reps=reps)
    xw, W2cat = _host_inputs(x, W1, a1, W2, a2)
    # shard columns: core k gets xT cols [k*XSH,(k+1)*XSH) + W1cat cols
    # [k*W1SH,(k+1)*W1SH); AllGather + on-device views restore both
    in_maps = [
        {
            **per_core[k],
            "W2cat": W2cat,
            "xw": np.ascontiguousarray(
                np.concatenate(
                    [
                        xw[:, k * XSH : (k + 1) * XSH],
                        xw[:, NPAD + k * W1SH : NPAD + (k + 1) * W1SH],
                    ],
                    axis=1,
                )
            ),
        }
        for k in range(P)
    ]
    return nc, in_maps


def kernel(x, edge_index, W1, a1, W2, a2):
    nc, in_maps = build(x, edge_index, W1, a1, W2, a2)
    res = run_bass_kernel_spmd(nc, in_maps, list(range(P)))
    out = np.concatenate([res.results[k]["out"] for k in range(P)], axis=0)
    return out.astype(np.float32)


# revision 34
# speedup vs baseline: 7.7909x; 1.1071x over previous
"""GAT (2-layer) Trainium2 kernel, SPMD across 8 NeuronCores.

Key algebra: segment softmax keyed by row is shift invariant, so the
(h[row] . a_l) term cancels and attention factorizes:
    alpha_e = g[col_e] * u[row_e],
    g[n] = exp(h[n] . a_r),   u[r] = 1 / sum_{e: row=r} g[col_e]
Each GAT layer then needs only two unweighted sparse ops over the fixed
graph:
    z   = A @ g          (segment-sum keyed by row)   -> u = 1/z
    agg = A^T @ (u * h)  (segment-sum keyed by col)
    out = g * agg
Both are done as: dma_gather of table rows per edge (128 edges/block) +
one-hot matmul (lhsT = one-hot of block-relative destination, built by a
DVE is_equal against an iota tile) accumulating into a PSUM window.

Sharding: z-phase edges by row range, aggregation edges by col range (each
core owns its 1250-node output slice). Cross-core: AllGather of u
([10000,H] f32) and of h1^T (5 MB) between the layers.

Host->device traffic is minimized (the axon tunnel is the wall-clock
bottleneck): x is uploaded sharded as bf16 and AllGathered on device,
W1 is bf16, the W@a_r reductions are host-precomputed, gather indices are
uploaded unreplicated [16, n] and replicated to 128 partitions on device,
one-hot keys are bf16, and iota/identity constants are generated on device.

kernel(**inputs) takes FULL inputs and returns the FULL [10000, 22] output.
"""

import sys

sys.path.insert(0, "/opt/trn_rl_repo")

import numpy as np
import ml_dtypes

# Persistent XLA compilation cache: run_bass_kernel_spmd re-jits a fresh
# closure every call; with the cache enabled the per-call backend compile
# becomes a ~20ms disk hit instead of ~1s.
try:
    import os
    import tempfile

    import jax

    _cache_dir = os.path.join(tempfile.gettempdir(), "jax_comp_cache")
    jax.config.update("jax_compilation_cache_dir", _cache_dir)
    jax.config.update("jax_persistent_cache_min_compile_time_secs", 0)
    jax.config.update("jax_persistent_cache_min_entry_size_bytes", 0)
except Exception:
    pass
try:
    import concourse.bass2jax  # registers the bass_fast_dispatch config state

    jax.config.update("bass_fast_dispatch", True)
except Exception:
    pass

from concourse import bacc, mybir, tile
from concourse.bass_utils import run_bass_kernel_spmd

F32 = mybir.dt.float32
BF16 = mybir.dt.bfloat16
I16 = mybir.dt.int16
I8 = mybir.dt.int8
EXP = mybir.ActivationFunctionType.Exp
EQ = mybir.AluOpType.is_equal
MULT = mybir.AluOpType.mult
ADD = mybir.AluOpType.add
MIN = mybir.AluOpType.min
BYPASS = mybir.AluOpType.bypass

N = 10000
E = 320000
F = 128
H = 4
C = 22
P = 8
SLICE = N // P               # 1250 nodes per core
NWIN = (SLICE + 127) // 128  # 10 windows of <=128 dst/src nodes
NBLK = N // 128 + 1          # 79; always >= 1 pad block so row N is zero
NPAD = NBLK * 128            # 10112; table rows >= N are zero
XSH = NPAD // P              # 1264 xT columns uploaded per core
OW1 = H * F                  # 512
W1PAD = 520                  # W1 (512) + w1a (4) + pad (4), divisible by P
W1SH = W1PAD // P            # 65 W1cat columns uploaded per core
XW = XSH + W1SH              # 1329 combined xT+W1 shard columns
CHUNK = 16                   # layer-1 gather chunk (128-edge blocks)
SKIP = set()                 # debug/timing: {"z", "agg1", "agg2"}


def _configure(n, e, p=8):
    """Shrink sizes for simulator debugging (same program structure)."""
    global N, E, P, SLICE, NWIN, NBLK, NPAD, XSH, XW
    N, E, P = n, e, p
    SLICE = N // P
    NWIN = (SLICE + 127) // 128
    NBLK = N // 128 + 1
    NPAD = NBLK * 128
    XSH = NPAD // P
    XW = XSH + W1SH


def _cdiv(a, b):
    return (a + b - 1) // b


def _wrap_idxs(idx):
    """dma_gather index layout: logical i at [i%16, i//16] (16 partitions,
    replicated to 128 on device)."""
    n = idx.shape[0]
    assert n % 16 == 0
    return np.ascontiguousarray(idx.reshape(n // 16, 16).T.astype(np.int16))


def _phase_arrays(key, other, nwin):
    """Group one core's (already core-local) edges by 128-wide key window.
    Returns per-window (rel, other) with rel = key - 128*w."""
    w = key >> 7
    order = np.argsort(w, kind="stable")
    key, other, w = key[order], other[order], w[order]
    out = []
    bounds = np.searchsorted(w, np.arange(nwin + 1))
    for i in range(nwin):
        sl = slice(bounds[i], bounds[i + 1])
        k, o = key[sl] - 128 * i, other[sl]
        so = np.argsort(o, kind="stable")  # sorted gather idx -> HBM locality
        out.append((k[so], o[so]))
    return out


def _build_edge_inputs(row, col):
    zraw, braw = [], []
    for k in range(P):
        base = k * SLICE
        m = (row >= base) & (row < base + SLICE)
        zraw.append(_phase_arrays(row[m] - base, col[m], NWIN))
        m = (col >= base) & (col < base + SLICE)
        braw.append(_phase_arrays(col[m] - base, row[m], NWIN))

    def block_counts(raw):
        return [
            max(_cdiv(max(max(len(raw[k][w][0]) for k in range(P)), 1), 128), 1)
            for w in range(NWIN)
        ]

    zB = block_counts(zraw)
    bB = block_counts(braw)

    def pack(raw, B):
        idx_l, rel_l = [], []
        for w in range(NWIN):
            n = B[w] * 128
            rel = np.zeros(n, np.int32)
            oth = np.full(n, N, np.int32)  # dummy -> zero table row
            r, o = raw[w]
            rel[: len(r)] = r
            oth[: len(o)] = o
            idx_l.append(_wrap_idxs(oth))
            rel_l.append(rel.reshape(B[w], 128).T.astype(np.int8))
        return np.concatenate(idx_l, 1), np.concatenate(rel_l, 1)

    per_core = []
    for k in range(P):
        zidx, zrel = pack(zraw[k], zB)
        bidx, brel = pack(braw[k], bB)
        base = k * SLICE
        gw = []
        for w in range(NWIN):
            nid = base + 128 * w + np.arange(128)
            nid = np.where(nid < base + SLICE, nid, N)
            gw.append(_wrap_idxs(nid))
        per_core.append(
            dict(
                idx=np.ascontiguousarray(
                    np.concatenate([zidx, bidx] + gw, 1)
                ),
                rel=np.ascontiguousarray(np.concatenate([zrel, brel], 1)),
            )
        )
    return zB, bB, per_core


def _load_idx(nc, idx_sb, idx_ap):
    """Replicate an unreplicated [16, X] int16 DRAM index AP to all 128
    SBUF partitions (8 small DMAs)."""
    for g in range(8):
        nc.sync.dma_start(idx_sb[g * 16 : (g + 1) * 16, :], idx_ap)


def _spmm(nc, tc, B, CH, idx_ap, rel_ap, tab, elem, rhs_w, psum_w, iof_t,
          name, flush, skip=False, bufs=3):
    """One-hot-matmul SpMM over 128-dst windows with gather chunks that span
    window boundaries. flush(w, po) consumes each window's PSUM result.
    idx_ap: [16, TOT*8] int16 DRAM AP; rel_ap: [128, TOT] bf16 DRAM AP."""
    total = sum(B)
    with (
        tc.tile_pool(name=f"gg{name}", bufs=bufs) as ggp,
        tc.tile_pool(name=f"gx{name}", bufs=1) as gxp,
        tc.tile_pool(name=f"go{name}", bufs=bufs) as ohp,
        tc.tile_pool(name=f"gp{name}", bufs=2, space="PSUM") as pp,
    ):
        idx_sb = gxp.tile([128, total * 8], I16, tag="gxi")
        _load_idx(nc, idx_sb, idx_ap)
        rel8_sb = gxp.tile([128, total], I8, tag="gxr8")
        nc.sync.dma_start(rel8_sb[:], rel_ap)
        rel_sb = gxp.tile([128, total], BF16, tag="gxr")
        nc.vector.tensor_copy(rel_sb[:], rel8_sb[:])

        gts, ohs = {}, {}
        gb = 0
        for w, Bw in enumerate(B):
            po = pp.tile([128, psum_w], F32, tag="po")
            if skip:
                nc.vector.memset(po[:], 1.0)
                flush(w, po)
                continue
            for b in range(Bw):
                ch, off = divmod(gb, CH)
                if off == 0:
                    cb = min(CH, total - ch * CH)
                    gt = ggp.tile([128, CH, elem], BF16, tag="gg")
                    nc.gpsimd.dma_gather(
                        gt[:, :cb, :], tab[:],
                        idx_sb[:, ch * CH * 8 : (ch * CH + cb) * 8],
                        cb * 128, cb * 128, elem, single_packet=False,
                    )
                    oh = ohp.tile([128, CH, 128], BF16, tag="go")
                    nc.vector.tensor_tensor(
                        oh[:, :cb, :],
                        iof_t[:].rearrange("p (x f) -> p x f", x=1)
                        .broadcast_to([128, cb, 128]),
                        rel_sb[:, ch * CH : ch * CH + cb]
                        .rearrange("p (b x) -> p b x", x=1)
                        .broadcast_to([128, cb, 128]),
                        EQ,
                    )
                    gts[ch], ohs[ch] = gt, oh
                nc.tensor.matmul(
                    po[:], ohs[ch][:, off, :], gts[ch][:, off, 0:rhs_w],
                    start=(b == 0), stop=(b == Bw - 1),
                )
                gb += 1
            flush(w, po)


def _declare(nc, zB, bB):
    ZT, BT = sum(zB), sum(bB)
    T = type("T", (), {})()
    T.xw = nc.dram_tensor("xw", [F, XW], BF16, kind="ExternalInput")
    T.W2cat = nc.dram_tensor("W2cat", [F, C + 1], F32, kind="ExternalInput")
    T.idx_d = nc.dram_tensor(
        "idx", [16, (ZT + BT + NWIN) * 8], I16, kind="ExternalInput"
    )
    T.rel_d = nc.dram_tensor("rel", [128, ZT + BT], I8, kind="ExternalInput")
    T.out_d = nc.dram_tensor("out", [SLICE, C], BF16, kind="ExternalOutput")
    # column offsets into idx/rel for the three index groups
    T.zoff, T.boff, T.gwoff = 0, ZT, ZT + BT

    T.xw_loc = nc.dram_tensor("xw_loc", [F, XW], BF16)
    T.xw_ag = nc.dram_tensor("xw_ag", [P, F, XW], BF16, addr_space="Shared")
    T.g1_tab = nc.dram_tensor("g1_tab", [NPAD, 128], BF16)
    T.hh1_tab = nc.dram_tensor("hh1_tab", [NPAD, OW1], BF16)
    T.g2_tab = nc.dram_tensor("g2_tab", [NPAD, 128], BF16)
    T.hh2_tab = nc.dram_tensor("hh2_tab", [NPAD, 128], BF16)
    T.u1_sl = nc.dram_tensor("u1_sl", [SLICE, H], F32)
    T.u2_sl = nc.dram_tensor("u2_sl", [SLICE, 1], F32)
    T.u1_full = nc.dram_tensor("u1_full", [NPAD, H], F32, addr_space="Shared")
    T.u2_full = nc.dram_tensor("u2_full", [NPAD, 1], F32, addr_space="Shared")
    T.h1T_loc = nc.dram_tensor("h1T_loc", [F, SLICE], F32)
    T.h1T_ag = nc.dram_tensor("h1T_ag", [P, F, SLICE], F32, addr_space="Shared")

    return T


def _emit(nc, tc, T, zB, bB, s=""):
        groups = [list(range(P))]
        # ======= constants generated on device (iota / identity) =======
        with tc.tile_pool(name="const" + s, bufs=1) as cp:
            iof_f = cp.tile([128, 128], F32)     # row-iota 0..127, f32
            nc.gpsimd.iota(iof_f[:], [[1, 128]], channel_multiplier=0,
                           allow_small_or_imprecise_dtypes=True)
            pid_f = cp.tile([128, 128], F32)     # partition index, f32
            nc.gpsimd.iota(pid_f[:], [[0, 128]], channel_multiplier=1,
                           allow_small_or_imprecise_dtypes=True)
            id_t = cp.tile([128, 128], F32)      # identity
            nc.vector.tensor_tensor(id_t[:], iof_f[:], pid_f[:], EQ)
            iof_b = cp.tile([128, 128], BF16)    # row-iota, bf16 (one-hot key)
            nc.vector.tensor_copy(iof_b[:], iof_f[:])

            # broadcast x+W1 across cores on NeuronLink (upload is sharded);
            # collectives cannot read IO tensors, so bounce through DRAM
            nc.sync.dma_start(T.xw_loc[:], T.xw[:])
            nc.gpsimd.collective_compute(
                "AllGather", BYPASS, groups,
                ins=[T.xw_loc[:].opt()], outs=[T.xw_ag[:].opt()],
            )

            # ================= layer 1: dense + tables + z1 =================
            with (
                tc.tile_pool(name="persist" + s, bufs=1) as pp,
                tc.tile_pool(name="small" + s, bufs=3) as sp,
            ):
                xw_view = T.xw_ag.ap().rearrange("s f n -> f s n")
                x_sb = pp.tile([128, P, XSH], BF16)  # full xT, 20.2KB/par
                nc.sync.dma_start(x_sb[:], xw_view[:, :, 0:XSH])
                x_flat = x_sb[:].rearrange("f s n -> f (s n)")
                w1_sb = pp.tile([128, P, W1SH], BF16)
                nc.sync.dma_start(w1_sb[:], xw_view[:, :, XSH:XW])
                w1_flat = w1_sb[:].rearrange("f s n -> f (s n)")
                W1_t = w1_flat[:, 0:OW1]
                W1ar_t = w1_flat[:, OW1 : OW1 + H]

                h_nm = pp.tile([128, NBLK, OW1], BF16)  # 79x512 bf16/par
                g1_nm = pp.tile([128, NBLK, H], F32)
                with (
                    tc.tile_pool(name="ph" + s, bufs=2, space="PSUM") as php,
                    tc.tile_pool(name="psr" + s, bufs=2, space="PSUM") as psrp,
                ):
                    for b in range(NBLK):
                        xt = x_flat[:, b * 128 : (b + 1) * 128]
                        ph = php.tile([128, OW1], F32)
                        nc.tensor.matmul(ph[:], xt, W1_t, start=True, stop=True)
                        psr = psrp.tile([128, H], F32)
                        nc.tensor.matmul(psr[:], xt, W1ar_t, start=True, stop=True)
                        nc.vector.tensor_copy(h_nm[:, b, :], ph[:])
                        nc.scalar.activation(g1_nm[:, b, :], psr[:], EXP)

                with tc.tile_pool(name="stage" + s, bufs=1) as stp:
                    st = stp.tile([128, NBLK, 128], BF16, tag="stage")
                    nc.vector.memset(st[:], 0.0)
                    nc.vector.tensor_copy(
                        st[:, : NBLK - 1, 0:H], g1_nm[:, : NBLK - 1, :]
                    )
                    nv = N - 128 * (NBLK - 1)
                    if nv > 0:
                        nc.vector.tensor_copy(
                            st[0:nv, NBLK - 1, 0:H], g1_nm[0:nv, NBLK - 1, :]
                        )
                    nc.sync.dma_start(
                        T.g1_tab.ap().rearrange("(b p) c -> p b c", p=128), st[:]
                    )

                with tc.tile_pool(name="zu1" + s, bufs=3) as zup:

                    def zflush1(w, po, zup=zup):
                        u_t = zup.tile([128, H], F32, tag="u")
                        nc.vector.reciprocal(u_t[:], po[:, 0:H])
                        rows = min(128, SLICE - 128 * w)
                        nc.sync.dma_start(
                            T.u1_sl[w * 128 : w * 128 + rows, :], u_t[0:rows, :]
                        )

                    _spmm(nc, tc, zB, 32,
                          T.idx_d[:, T.zoff * 8 : (T.zoff + sum(zB)) * 8],
                          T.rel_d[:, T.zoff : T.zoff + sum(zB)],
                          T.g1_tab, 128, 8, 8,
                          iof_b, "z1" + s, zflush1, skip=("z" in SKIP), bufs=2)

                nc.gpsimd.collective_compute(
                    "AllGather", BYPASS, groups,
                    ins=[T.u1_sl[:].opt()], outs=[T.u1_full[0:N, :].opt()],
                )
                zt = sp.tile([NPAD - N, H], F32, tag="zpad")
                nc.vector.memset(zt[:], 0.0)
                nc.sync.dma_start(T.u1_full[N:NPAD, :], zt[:])

                u1_nm = pp.tile([128, NBLK, H], F32)
                nc.sync.dma_start(
                    u1_nm[:], T.u1_full.ap().rearrange("(b p) c -> p b c", p=128)
                )
                with tc.tile_pool(name="hhp" + s, bufs=3) as hhp:
                    for b in range(NBLK):
                        hh = hhp.tile([128, OW1], BF16)
                        for hd in range(H):
                            nc.vector.tensor_scalar(
                                hh[:, hd * F : (hd + 1) * F],
                                h_nm[:, b, hd * F : (hd + 1) * F],
                                u1_nm[:, b, hd : hd + 1],
                                None,
                                MULT,
                            )
                        nc.sync.dma_start(
                            T.hh1_tab.ap().rearrange("(b p) c -> p b c", p=128)[:, b, :],
                            hh[:],
                        )

            # ============ layer 1 aggregation + layer 2 (h_nm freed) ============
            with (
                tc.tile_pool(name="persist2" + s, bufs=1) as pp2,
                tc.tile_pool(name="small2" + s, bufs=3) as sp2,
            ):
                W2cat = pp2.tile([F, C + 1], F32)
                nc.sync.dma_start(W2cat[:], T.W2cat[:])

                h1T_sb = pp2.tile([128, NWIN * 128], F32)

                with (
                    tc.tile_pool(name="gwp" + s, bufs=2) as gwp,
                    tc.tile_pool(name="ptw" + s, bufs=2, space="PSUM") as ptw,
                    tc.tile_pool(name="flush" + s, bufs=2) as flp,
                ):
                    gwi = gwp.tile([128, NWIN * 8], I16, tag="gwi")
                    _load_idx(nc, gwi, T.idx_d[:, T.gwoff * 8 :])
                    gwb = gwp.tile([128, NWIN, 128], BF16, tag="gwb")
                    nc.gpsimd.dma_gather(
                        gwb[:], T.g1_tab[:], gwi[:], NWIN * 128, NWIN * 128, 128,
                        single_packet=False,
                    )
                    gwf = gwp.tile([128, NWIN, 128], F32, tag="gwf")
                    nc.vector.tensor_copy(gwf[:], gwb[:])

                    def flush1(w, po):
                        o_t = flp.tile([128, OW1], F32, tag="o")
                        for hd in range(H):
                            nc.vector.tensor_scalar(
                                o_t[:, hd * F : (hd + 1) * F],
                                po[:, hd * F : (hd + 1) * F],
                                gwf[:, w, hd : hd + 1],
                                None, MULT,
                            )
                        # elu(x) = relu(x) + exp(min(x,0)) - 1 ; h1 = mean_heads
                        neg = flp.tile([128, OW1], F32, tag="neg")
                        nc.vector.tensor_scalar(neg[:], o_t[:], 0.0, None, MIN)
                        ex = flp.tile([128, OW1], F32, tag="ex")
                        nc.scalar.activation(ex[:], neg[:], EXP)
                        rl = flp.tile([128, OW1], F32, tag="rl")
                        nc.vector.tensor_relu(rl[:], o_t[:])
                        su = flp.tile([128, OW1], F32, tag="su")
                        nc.vector.tensor_tensor(su[:], rl[:], ex[:], ADD)
                        t01 = flp.tile([128, F], F32, tag="t01")
                        nc.vector.tensor_tensor(t01[:], su[:, 0:F], su[:, F : 2 * F], ADD)
                        t23 = flp.tile([128, F], F32, tag="t23")
                        nc.vector.tensor_tensor(
                            t23[:], su[:, 2 * F : 3 * F], su[:, 3 * F :], ADD
                        )
                        h1_t = flp.tile([128, F], F32, tag="h1")
                        nc.vector.tensor_tensor(h1_t[:], t01[:], t23[:], ADD)
                        nc.vector.tensor_scalar(h1_t[:], h1_t[:], 0.25, -1.0, MULT, ADD)
                        ptt = ptw.tile([128, 128], F32, tag="ptt")
                        nc.tensor.transpose(ptt[:], h1_t[:], id_t[:])
                        nc.vector.tensor_copy(h1T_sb[:, w * 128 : (w + 1) * 128], ptt[:])

                    _spmm(nc, tc, bB, CHUNK,
                          T.idx_d[:, T.boff * 8 : (T.boff + sum(bB)) * 8],
                          T.rel_d[:, T.boff : T.boff + sum(bB)],
                          T.hh1_tab, OW1,
                          OW1, OW1, iof_b, "a1" + s, flush1, skip=("agg1" in SKIP),
                          bufs=3)

                nc.sync.dma_start(T.h1T_loc[:], h1T_sb[:, 0:SLICE])
                nc.gpsimd.collective_compute(
                    "AllGather", BYPASS, groups,
                    ins=[T.h1T_loc[:].opt()], outs=[T.h1T_ag[:].opt()],
                )
                h1T_full = pp2.tile([128, P, SLICE], F32)
                nc.sync.dma_start(h1T_full[:], T.h1T_ag.ap().rearrange("s f n -> f s n"))
                h1T_flat = h1T_full[:].rearrange("f s n -> f (s n)")

                h2_nm = pp2.tile([128, NBLK, C], F32)
                g2_nm = pp2.tile([128, NBLK, 1], F32)
                with tc.tile_pool(name="ph2" + s, bufs=2, space="PSUM") as ph2p:
                    for b in range(NBLK):
                        nv = max(0, min(128, N - b * 128))
                        if nv < 128:
                            nc.vector.memset(h2_nm[:, b, :], 0.0)
                            nc.vector.memset(g2_nm[:, b, :], 0.0)
                        if nv == 0:
                            continue
                        ph2 = ph2p.tile([128, C + 1], F32)
                        nc.tensor.matmul(
                            ph2[0:nv, :],
                            h1T_flat[:, b * 128 : b * 128 + nv],
                            W2cat[:],
                            start=True,
                            stop=True,
                        )
                        nc.vector.tensor_copy(h2_nm[0:nv, b, :], ph2[0:nv, 0:C])
                        nc.scalar.activation(g2_nm[0:nv, b, :], ph2[0:nv, C : C + 1], EXP)

                with tc.tile_pool(name="stage2" + s, bufs=1) as stp:
                    st = stp.tile([128, NBLK, 128], BF16, tag="stage2")
                    nc.vector.memset(st[:], 0.0)
                    nc.vector.tensor_copy(st[:, :, 0:1], g2_nm[:])
                    nc.sync.dma_start(
                        T.g2_tab.ap().rearrange("(b p) c -> p b c", p=128), st[:]
                    )

                with tc.tile_pool(name="zu2" + s, bufs=3) as zup:

                    def zflush2(w, po, zup=zup):
                        u_t = zup.tile([128, 1], F32, tag="u2")
                        nc.vector.reciprocal(u_t[:], po[:, 0:1])
                        rows = min(128, SLICE - 128 * w)
                        nc.sync.dma_start(
                            T.u2_sl[w * 128 : w * 128 + rows, :], u_t[0:rows, :]
                        )

                    _spmm(nc, tc, zB, 32,
                          T.idx_d[:, T.zoff * 8 : (T.zoff + sum(zB)) * 8],
                          T.rel_d[:, T.zoff : T.zoff + sum(zB)],
                          T.g2_tab, 128, 8, 8,
                          iof_b, "z2" + s, zflush2, skip=("z" in SKIP), bufs=3)

                nc.gpsimd.collective_compute(
                    "AllGather", BYPASS, groups,
                    ins=[T.u2_sl[:].opt()], outs=[T.u2_full[0:N, :].opt()],
                )
                zt2 = sp2.tile([NPAD - N, 1], F32, tag="zpad2")
                nc.vector.memset(zt2[:], 0.0)
                nc.sync.dma_start(T.u2_full[N:NPAD, :], zt2[:])

                u2_nm = pp2.tile([128, NBLK, 1], F32)
                nc.sync.dma_start(
                    u2_nm[:], T.u2_full.ap().rearrange("(b p) c -> p b c", p=128)
                )
                with tc.tile_pool(name="stage3" + s, bufs=1) as stp:
                    st = stp.tile([128, NBLK, 128], BF16, tag="stage3")
                    nc.vector.memset(st[:], 0.0)
                    for b in range(NBLK):
                        nc.vector.tensor_scalar(
                            st[:, b, 0:C], h2_nm[:, b, :], u2_nm[:, b, :], None, MULT
                        )
                    nc.sync.dma_start(
                        T.hh2_tab.ap().rearrange("(b p) c -> p b c", p=128), st[:]
                    )

                with (
                    tc.tile_pool(name="gw2" + s, bufs=2) as gwp,
                    tc.tile_pool(name="fl2" + s, bufs=2) as flp,
                ):
                    gwi = gwp.tile([128, NWIN * 8], I16, tag="gwi2")
                    _load_idx(nc, gwi, T.idx_d[:, T.gwoff * 8 :])
                    gwb = gwp.tile([128, NWIN, 128], BF16, tag="gwb2")
                    nc.gpsimd.dma_gather(
                        gwb[:], T.g2_tab[:], gwi[:], NWIN * 128, NWIN * 128, 128,
                        single_packet=False,
                    )
                    gwf = gwp.tile([128, NWIN, 128], F32, tag="gwf2")
                    nc.vector.tensor_copy(gwf[:], gwb[:])

                    def flush2(w, po):
                        o2 = flp.tile([128, C], BF16, tag="o2")
                        nc.vector.tensor_scalar(
                            o2[:], po[:, 0:C], gwf[:, w, 0:1], None, MULT
                        )
                        rows = min(128, SLICE - 128 * w)
                        nc.sync.dma_start(
                            T.out_d[w * 128 : w * 128 + rows, :], o2[0:rows, :]
                        )

                    _spmm(nc, tc, bB, 32,
                          T.idx_d[:, T.boff * 8 : (T.boff + sum(bB)) * 8],
                          T.rel_d[:, T.boff : T.boff + sum(bB)],
                          T.hh2_tab, 128,
                          C, C, iof_b, "a2" + s, flush2, skip=("agg2" in SKIP),
                          bufs=3)


def _build_program(zB, bB, reps=1):
    nc = bacc.Bacc("TRN2", target_bir_lowering=False, debug=False, num_devices=P)
    T = _declare(nc, zB, bB)
    with tile.TileContext(nc) as tc:
        for r in range(reps):
            _emit(nc, tc, T, zB, bB, s=str(r))
            if reps > 1:
                with tc.tile_critical():
                    nc.all_core_barrier()
    nc.compile()
    # the program is immutable from here on, but the jax lowering re-serializes
    # it on every run_bass_kernel_spmd call (~30ms); memoize the bytes
    cached = nc.to_json_bytes()
    nc.to_json_bytes = lambda: cached
    return nc


def _host_inputs(x, W1, a1, W2, a2):
    x = np.asarray(x, np.float64)
    W1 = np.asarray(W1, np.float64)
    a1 = np.asarray(a1, np.float64)
    W2 = np.asarray(W2, np.float64)
    a2 = np.asarray(a2, np.float64)
    # combined bf16 [F, NPAD + W1PAD]: xT | W1 | w1a | pad, sharded by column
    xw = np.zeros((F, NPAD + W1PAD), np.float64)
    xw[:, :N] = x.T
    xw[:, NPAD : NPAD + OW1] = W1
    # host-precomputed per-head W @ a_r reductions
    xw[:, NPAD + OW1 : NPAD + OW1 + H] = np.einsum(
        "fhg,hg->fh", W1.reshape(F, H, F), a1[:, F:]
    )
    W2cat = np.concatenate([W2, (W2 @ a2[0, C:]).reshape(F, 1)], axis=1)
    return xw.astype(ml_dtypes.bfloat16), W2cat.astype(np.float32)


def build(x, edge_index, W1, a1, W2, a2, reps=1):
    """Build program + per-core input maps. Returns (nc, in_maps)."""
    ei = np.asarray(edge_index)
    row = ei[0].astype(np.int64)
    col = ei[1].astype(np.int64)
    zB, bB, per_core = _build_edge_inputs(row, col)
    nc = _build_program(zB, bB, reps=reps)
    xw, W2cat = _host_inputs(x, W1, a1, W2, a2)
    # shard columns: core k gets xT cols [k*XSH,(k+1)*XSH) + W1cat cols
    # [k*W1SH,(k+1)*W1SH); AllGather + on-device views restore both
    in_maps = [
        {
            **per_core[k],
            "W2cat": W2cat,
            "xw": np.ascontiguousarray(
                np.concatenate(
                    [
                        xw[:, k * XSH : (k + 1) * XSH],
                        xw[:, NPAD + k * W1SH : NPAD + (k + 1) * W1SH],
                    ],
                    axis=1,
                )
            ),
        }
        for k in range(P)
    ]
    return nc, in_maps


def kernel(x, edge_index, W1, a1, W2, a2):
    nc, in_maps = build(x, edge_index, W1, a1, W2, a2)
    res = run_bass_kernel_spmd(nc, in_maps, list(range(P)))
    out = np.concatenate([res.results[k]["out"] for k in range(P)], axis=0)
    return out.astype(np.float32)


# revision 42
# speedup vs baseline: 8.9806x; 1.1527x over previous
"""GAT (2-layer) Trainium2 kernel, SPMD across 8 NeuronCores.

Key algebra: segment softmax keyed by row is shift invariant, so the
(h[row] . a_l) term cancels and attention factorizes:
    alpha_e = g[col_e] * u[row_e],
    g[n] = exp(h[n] . a_r),   u[r] = 1 / sum_{e: row=r} g[col_e]
Each GAT layer then needs only two unweighted sparse ops over the fixed
graph:
    z   = A @ g          (segment-sum keyed by row)   -> u = 1/z
    agg = A^T @ (u * h)  (segment-sum keyed by col)
    out = g * agg
Both are done as: dma_gather of table rows per edge (128 edges/block) +
one-hot matmul (lhsT = one-hot of block-relative destination, built by a
DVE is_equal against an iota tile) accumulating into a PSUM window.

Sharding: z-phase edges by row range, aggregation edges by col range (each
core owns its 1250-node output slice). Cross-core: AllGather of u
([10000,H] f32) and of h1^T (5 MB) between the layers.

Host->device traffic is minimized (the axon tunnel is the wall-clock
bottleneck): x is uploaded sharded as bf16 and AllGathered on device,
W1 is bf16, the W@a_r reductions are host-precomputed, gather indices are
uploaded unreplicated [16, n] and replicated to 128 partitions on device,
one-hot keys are bf16, and iota/identity constants are generated on device.

kernel(**inputs) takes FULL inputs and returns the FULL [10000, 22] output.
"""

import sys

sys.path.insert(0, "/opt/trn_rl_repo")

import numpy as np
import ml_dtypes

# Persistent XLA compilation cache: run_bass_kernel_spmd re-jits a fresh
# closure every call; with the cache enabled the per-call backend compile
# becomes a ~20ms disk hit instead of ~1s.
try:
    import os
    import tempfile

    import jax

    _cache_dir = os.path.join(tempfile.gettempdir(), "jax_comp_cache")
    jax.config.update("jax_compilation_cache_dir", _cache_dir)
    jax.config.update("jax_persistent_cache_min_compile_time_secs", 0)
    jax.config.update("jax_persistent_cache_min_entry_size_bytes", 0)
except Exception:
    pass
try:
    import concourse.bass2jax  # registers the bass_fast_dispatch config state

    jax.config.update("bass_fast_dispatch", True)
except Exception:
    pass

from concourse import bacc, mybir, tile
from concourse.bass_utils import run_bass_kernel_spmd

F32 = mybir.dt.float32
BF16 = mybir.dt.bfloat16
I16 = mybir.dt.int16
I8 = mybir.dt.int8
EXP = mybir.ActivationFunctionType.Exp
EQ = mybir.AluOpType.is_equal
MULT = mybir.AluOpType.mult
ADD = mybir.AluOpType.add
MIN = mybir.AluOpType.min
BYPASS = mybir.AluOpType.bypass

N = 10000
E = 320000
F = 128
H = 4
C = 22
P = 8
SLICE = N // P               # 1250 nodes per core
NWIN = (SLICE + 127) // 128  # 10 windows of <=128 dst/src nodes
NBLK = N // 128 + 1          # 79; always >= 1 pad block so row N is zero
NPAD = NBLK * 128            # 10112; table rows >= N are zero
XSH = NPAD // P              # 1264 xT columns uploaded per core
OW1 = H * F                  # 512
W1PAD = 520                  # W1 (512) + w1a (4) + pad (4), divisible by P
W1SH = W1PAD // P            # 65 W1cat columns uploaded per core
W2PAD = 24                   # W2 (22) + w2a (1) + pad (1)
W2SH = W2PAD // P            # 3 W2cat columns uploaded per core
XW = XSH + W1SH + W2SH       # 1332 combined xT+W1+W2 shard columns
CHUNK = 16                   # layer-1 gather chunk (128-edge blocks)
SKIP = set()                 # debug/timing: {"z", "agg1", "agg2"}


def _configure(n, e, p=8):
    """Shrink sizes for simulator debugging (same program structure)."""
    global N, E, P, SLICE, NWIN, NBLK, NPAD, XSH, XW
    N, E, P = n, e, p
    SLICE = N // P
    NWIN = (SLICE + 127) // 128
    NBLK = N // 128 + 1
    NPAD = NBLK * 128
    XSH = NPAD // P
    XW = XSH + W1SH + W2SH


def _cdiv(a, b):
    return (a + b - 1) // b


def _wrap_idxs(idx):
    """dma_gather index layout: logical i at [i%16, i//16] (16 partitions,
    replicated to 128 on device)."""
    n = idx.shape[0]
    assert n % 16 == 0
    return np.ascontiguousarray(idx.reshape(n // 16, 16).T.astype(np.int16))


def _phase_arrays(key, other, nwin):
    """Group one core's (already core-local) edges by 128-wide key window.
    Returns per-window (rel, other) with rel = key - 128*w."""
    w = key >> 7
    order = np.argsort(w, kind="stable")
    key, other, w = key[order], other[order], w[order]
    out = []
    bounds = np.searchsorted(w, np.arange(nwin + 1))
    for i in range(nwin):
        sl = slice(bounds[i], bounds[i + 1])
        k, o = key[sl] - 128 * i, other[sl]
        so = np.argsort(o, kind="stable")  # sorted gather idx -> HBM locality
        out.append((k[so], o[so]))
    return out


def _build_edge_inputs(row, col):
    zraw, braw = [], []
    for k in range(P):
        base = k * SLICE
        m = (row >= base) & (row < base + SLICE)
        zraw.append(_phase_arrays(row[m] - base, col[m], NWIN))
        m = (col >= base) & (col < base + SLICE)
        braw.append(_phase_arrays(col[m] - base, row[m], NWIN))

    def block_counts(raw):
        return [
            max(_cdiv(max(max(len(raw[k][w][0]) for k in range(P)), 1), 128), 1)
            for w in range(NWIN)
        ]

    zB = block_counts(zraw)
    bB = block_counts(braw)

    def pack(raw, B):
        idx_l, rel_l = [], []
        for w in range(NWIN):
            n = B[w] * 128
            rel = np.zeros(n, np.int32)
            oth = np.full(n, N, np.int32)  # dummy -> zero table row
            r, o = raw[w]
            rel[: len(r)] = r
            oth[: len(o)] = o
            idx_l.append(_wrap_idxs(oth))
            rel_l.append(rel.reshape(B[w], 128).T.astype(np.int8))
        return np.concatenate(idx_l, 1), np.concatenate(rel_l, 1)

    per_core = []
    for k in range(P):
        zidx, zrel = pack(zraw[k], zB)
        bidx, brel = pack(braw[k], bB)
        base = k * SLICE
        gw = []
        for w in range(NWIN):
            nid = base + 128 * w + np.arange(128)
            nid = np.where(nid < base + SLICE, nid, N)
            gw.append(_wrap_idxs(nid))
        per_core.append(
            dict(
                idx=np.ascontiguousarray(
                    np.concatenate([zidx, bidx] + gw, 1)
                ),
                rel=np.ascontiguousarray(np.concatenate([zrel, brel], 1)),
            )
        )
    return zB, bB, per_core


def _load_idx(nc, idx_sb, idx_ap):
    """Replicate an unreplicated [16, X] int16 DRAM index AP to all 128
    SBUF partitions (8 small DMAs)."""
    for g in range(8):
        nc.sync.dma_start(idx_sb[g * 16 : (g + 1) * 16, :], idx_ap)


def _spmm(nc, tc, B, CH, idx_ap, rel_ap, tab, elem, rhs_w, psum_w, iof_t,
          name, flush, skip=False, bufs=3):
    """One-hot-matmul SpMM over 128-dst windows with gather chunks that span
    window boundaries. flush(w, po) consumes each window's PSUM result.
    idx_ap: [16, TOT*8] int16 DRAM AP; rel_ap: [128, TOT] bf16 DRAM AP."""
    total = sum(B)
    with (
        tc.tile_pool(name=f"gg{name}", bufs=bufs) as ggp,
        tc.tile_pool(name=f"gx{name}", bufs=1) as gxp,
        tc.tile_pool(name=f"go{name}", bufs=bufs) as ohp,
        tc.tile_pool(name=f"gp{name}", bufs=2, space="PSUM") as pp,
    ):
        idx_sb = gxp.tile([128, total * 8], I16, tag="gxi")
        _load_idx(nc, idx_sb, idx_ap)
        rel8_sb = gxp.tile([128, total], I8, tag="gxr8")
        nc.sync.dma_start(rel8_sb[:], rel_ap)
        rel_sb = gxp.tile([128, total], BF16, tag="gxr")
        nc.vector.tensor_copy(rel_sb[:], rel8_sb[:])

        gts, ohs = {}, {}
        gb = 0
        for w, Bw in enumerate(B):
            po = pp.tile([128, psum_w], F32, tag="po")
            if skip:
                nc.vector.memset(po[:], 1.0)
                flush(w, po)
                continue
            for b in range(Bw):
                ch, off = divmod(gb, CH)
                if off == 0:
                    cb = min(CH, total - ch * CH)
                    gt = ggp.tile([128, CH, elem], BF16, tag="gg")
                    nc.gpsimd.dma_gather(
                        gt[:, :cb, :], tab[:],
                        idx_sb[:, ch * CH * 8 : (ch * CH + cb) * 8],
                        cb * 128, cb * 128, elem, single_packet=False,
                    )
                    oh = ohp.tile([128, CH, 128], BF16, tag="go")
                    nc.vector.tensor_tensor(
                        oh[:, :cb, :],
                        iof_t[:].rearrange("p (x f) -> p x f", x=1)
                        .broadcast_to([128, cb, 128]),
                        rel_sb[:, ch * CH : ch * CH + cb]
                        .rearrange("p (b x) -> p b x", x=1)
                        .broadcast_to([128, cb, 128]),
                        EQ,
                    )
                    gts[ch], ohs[ch] = gt, oh
                nc.tensor.matmul(
                    po[:], ohs[ch][:, off, :], gts[ch][:, off, 0:rhs_w],
                    start=(b == 0), stop=(b == Bw - 1),
                )
                gb += 1
            flush(w, po)


def _declare(nc, zB, bB):
    ZT, BT = sum(zB), sum(bB)
    T = type("T", (), {})()
    # blob layout per partition row (int8 bytes): xw bf16 | zrel i8 | brel i8
    T.blob = nc.dram_tensor(
        "blob", [128, 2 * XW + ZT + BT], I8, kind="ExternalInput"
    )
    T.idx_d = nc.dram_tensor(
        "idx", [16, (ZT + BT + NWIN) * 8], I16, kind="ExternalInput"
    )
    T.out_d = nc.dram_tensor("out", [SLICE, C], BF16, kind="ExternalOutput")
    # column offsets into idx for the three index groups
    T.zoff, T.boff, T.gwoff = 0, ZT, ZT + BT
    T.ZT, T.BT = ZT, BT

    T.xw_loc = nc.dram_tensor("xw_loc", [F, XW], BF16)
    T.xw_ag = nc.dram_tensor("xw_ag", [P, F, XW], BF16, addr_space="Shared")
    T.g1_tab = nc.dram_tensor("g1_tab", [NPAD, 128], BF16)
    T.hh1_tab = nc.dram_tensor("hh1_tab", [NPAD, OW1], BF16)
    T.g2_tab = nc.dram_tensor("g2_tab", [NPAD, 128], BF16)
    T.hh2_tab = nc.dram_tensor("hh2_tab", [NPAD, 128], BF16)
    T.u1_sl = nc.dram_tensor("u1_sl", [SLICE, H], F32)
    T.u2_sl = nc.dram_tensor("u2_sl", [SLICE, 1], F32)
    T.u1_full = nc.dram_tensor("u1_full", [NPAD, H], F32, addr_space="Shared")
    T.u2_full = nc.dram_tensor("u2_full", [NPAD, 1], F32, addr_space="Shared")
    T.h1T_loc = nc.dram_tensor("h1T_loc", [F, SLICE], F32)
    T.h1T_ag = nc.dram_tensor("h1T_ag", [P, F, SLICE], F32, addr_space="Shared")

    return T


def _emit(nc, tc, T, zB, bB, s=""):
        groups = [list(range(P))]
        # ======= constants generated on device (iota / identity) =======
        with tc.tile_pool(name="const" + s, bufs=1) as cp:
            iof_f = cp.tile([128, 128], F32)     # row-iota 0..127, f32
            nc.gpsimd.iota(iof_f[:], [[1, 128]], channel_multiplier=0,
                           allow_small_or_imprecise_dtypes=True)
            pid_f = cp.tile([128, 128], F32)     # partition index, f32
            nc.gpsimd.iota(pid_f[:], [[0, 128]], channel_multiplier=1,
                           allow_small_or_imprecise_dtypes=True)
            id_t = cp.tile([128, 128], F32)      # identity
            nc.vector.tensor_tensor(id_t[:], iof_f[:], pid_f[:], EQ)
            iof_b = cp.tile([128, 128], BF16)    # row-iota, bf16 (one-hot key)
            nc.vector.tensor_copy(iof_b[:], iof_f[:])

            # broadcast x+W1+W2 across cores on NeuronLink (upload is sharded);
            # collectives cannot read IO tensors, so bounce through DRAM
            nc.sync.dma_start(
                T.xw_loc[:], T.blob.ap()[:, 0 : 2 * XW].bitcast(BF16)
            )
            nc.gpsimd.collective_compute(
                "AllGather", BYPASS, groups,
                ins=[T.xw_loc[:].opt()], outs=[T.xw_ag[:].opt()],
            )

            # ================= layer 1: dense + tables + z1 =================
            with (
                tc.tile_pool(name="persist" + s, bufs=1) as pp,
                tc.tile_pool(name="small" + s, bufs=3) as sp,
            ):
                xw_view = T.xw_ag.ap().rearrange("s f n -> f s n")
                x_sb = pp.tile([128, P, XSH], BF16)  # full xT, 20.2KB/par
                nc.sync.dma_start(x_sb[:], xw_view[:, :, 0:XSH])
                x_flat = x_sb[:].rearrange("f s n -> f (s n)")
                w1_sb = pp.tile([128, P, W1SH], BF16)
                nc.sync.dma_start(w1_sb[:], xw_view[:, :, XSH : XSH + W1SH])
                w1_flat = w1_sb[:].rearrange("f s n -> f (s n)")
                W1_t = w1_flat[:, 0:OW1]
                W1ar_t = w1_flat[:, OW1 : OW1 + H]

                h_nm = pp.tile([128, NBLK, OW1], BF16)  # 79x512 bf16/par
                g1_nm = pp.tile([128, NBLK, H], F32)
                with (
                    tc.tile_pool(name="ph" + s, bufs=2, space="PSUM") as php,
                    tc.tile_pool(name="psr" + s, bufs=2, space="PSUM") as psrp,
                ):
                    for b in range(NBLK):
                        xt = x_flat[:, b * 128 : (b + 1) * 128]
                        ph = php.tile([128, OW1], F32)
                        nc.tensor.matmul(ph[:], xt, W1_t, start=True, stop=True)
                        psr = psrp.tile([128, H], F32)
                        nc.tensor.matmul(psr[:], xt, W1ar_t, start=True, stop=True)
                        nc.vector.tensor_copy(h_nm[:, b, :], ph[:])
                        nc.scalar.activation(g1_nm[:, b, :], psr[:], EXP)

                with tc.tile_pool(name="stage" + s, bufs=1) as stp:
                    st = stp.tile([128, NBLK, 128], BF16, tag="stage")
                    nc.vector.memset(st[:], 0.0)
                    nc.vector.tensor_copy(
                        st[:, : NBLK - 1, 0:H], g1_nm[:, : NBLK - 1, :]
                    )
                    nv = N - 128 * (NBLK - 1)
                    if nv > 0:
                        nc.vector.tensor_copy(
                            st[0:nv, NBLK - 1, 0:H], g1_nm[0:nv, NBLK - 1, :]
                        )
                    nc.sync.dma_start(
                        T.g1_tab.ap().rearrange("(b p) c -> p b c", p=128), st[:]
                    )

                with tc.tile_pool(name="zu1" + s, bufs=3) as zup:

                    def zflush1(w, po, zup=zup):
                        u_t = zup.tile([128, H], F32, tag="u")
                        nc.vector.reciprocal(u_t[:], po[:, 0:H])
                        rows = min(128, SLICE - 128 * w)
                        nc.sync.dma_start(
                            T.u1_sl[w * 128 : w * 128 + rows, :], u_t[0:rows, :]
                        )

                    _spmm(nc, tc, zB, 32,
                          T.idx_d[:, T.zoff * 8 : (T.zoff + sum(zB)) * 8],
                          T.blob.ap()[:, 2 * XW : 2 * XW + T.ZT],
                          T.g1_tab, 128, 8, 8,
                          iof_b, "z1" + s, zflush1, skip=("z" in SKIP), bufs=2)

                nc.gpsimd.collective_compute(
                    "AllGather", BYPASS, groups,
                    ins=[T.u1_sl[:].opt()], outs=[T.u1_full[0:N, :].opt()],
                )
                zt = sp.tile([NPAD - N, H], F32, tag="zpad")
                nc.vector.memset(zt[:], 0.0)
                nc.sync.dma_start(T.u1_full[N:NPAD, :], zt[:])

                u1_nm = pp.tile([128, NBLK, H], F32)
                nc.sync.dma_start(
                    u1_nm[:], T.u1_full.ap().rearrange("(b p) c -> p b c", p=128)
                )
                with tc.tile_pool(name="hhp" + s, bufs=3) as hhp:
                    for b in range(NBLK):
                        hh = hhp.tile([128, OW1], BF16)
                        for hd in range(H):
                            nc.vector.tensor_scalar(
                                hh[:, hd * F : (hd + 1) * F],
                                h_nm[:, b, hd * F : (hd + 1) * F],
                                u1_nm[:, b, hd : hd + 1],
                                None,
                                MULT,
                            )
                        nc.sync.dma_start(
                            T.hh1_tab.ap().rearrange("(b p) c -> p b c", p=128)[:, b, :],
                            hh[:],
                        )

            # ============ layer 1 aggregation + layer 2 (h_nm freed) ============
            with (
                tc.tile_pool(name="persist2" + s, bufs=1) as pp2,
                tc.tile_pool(name="small2" + s, bufs=3) as sp2,
            ):
                w2_sb = pp2.tile([128, P, W2SH], BF16)
                nc.sync.dma_start(
                    w2_sb[:],
                    T.xw_ag.ap().rearrange("s f n -> f s n")[:, :, XSH + W1SH : XW],
                )
                W2cat = pp2.tile([F, C + 1], F32)
                nc.vector.tensor_copy(
                    W2cat[:], w2_sb[:].rearrange("f s n -> f (s n)")[:, 0 : C + 1]
                )

                h1T_sb = pp2.tile([128, NWIN * 128], F32)

                with (
                    tc.tile_pool(name="gwp" + s, bufs=2) as gwp,
                    tc.tile_pool(name="ptw" + s, bufs=2, space="PSUM") as ptw,
                    tc.tile_pool(name="flush" + s, bufs=2) as flp,
                ):
                    gwi = gwp.tile([128, NWIN * 8], I16, tag="gwi")
                    _load_idx(nc, gwi, T.idx_d[:, T.gwoff * 8 :])
                    gwb = gwp.tile([128, NWIN, 128], BF16, tag="gwb")
                    nc.gpsimd.dma_gather(
                        gwb[:], T.g1_tab[:], gwi[:], NWIN * 128, NWIN * 128, 128,
                        single_packet=False,
                    )
                    gwf = gwp.tile([128, NWIN, 128], F32, tag="gwf")
                    nc.vector.tensor_copy(gwf[:], gwb[:])

                    def flush1(w, po):
                        o_t = flp.tile([128, OW1], F32, tag="o")
                        for hd in range(H):
                            nc.vector.tensor_scalar(
                                o_t[:, hd * F : (hd + 1) * F],
                                po[:, hd * F : (hd + 1) * F],
                                gwf[:, w, hd : hd + 1],
                                None, MULT,
                            )
                        # elu(x) = relu(x) + exp(min(x,0)) - 1 ; h1 = mean_heads
                        neg = flp.tile([128, OW1], F32, tag="neg")
                        nc.vector.tensor_scalar(neg[:], o_t[:], 0.0, None, MIN)
                        ex = flp.tile([128, OW1], F32, tag="ex")
                        nc.scalar.activation(ex[:], neg[:], EXP)
                        rl = flp.tile([128, OW1], F32, tag="rl")
                        nc.vector.tensor_relu(rl[:], o_t[:])
                        su = flp.tile([128, OW1], F32, tag="su")
                        nc.vector.tensor_tensor(su[:], rl[:], ex[:], ADD)
                        t01 = flp.tile([128, F], F32, tag="t01")
                        nc.vector.tensor_tensor(t01[:], su[:, 0:F], su[:, F : 2 * F], ADD)
                        t23 = flp.tile([128, F], F32, tag="t23")
                        nc.vector.tensor_tensor(
                            t23[:], su[:, 2 * F : 3 * F], su[:, 3 * F :], ADD
                        )
                        h1_t = flp.tile([128, F], F32, tag="h1")
                        nc.vector.tensor_tensor(h1_t[:], t01[:], t23[:], ADD)
                        nc.vector.tensor_scalar(h1_t[:], h1_t[:], 0.25, -1.0, MULT, ADD)
                        ptt = ptw.tile([128, 128], F32, tag="ptt")
                        nc.tensor.transpose(ptt[:], h1_t[:], id_t[:])
                        nc.vector.tensor_copy(h1T_sb[:, w * 128 : (w + 1) * 128], ptt[:])

                    _spmm(nc, tc, bB, CHUNK,
                          T.idx_d[:, T.boff * 8 : (T.boff + sum(bB)) * 8],
                          T.blob.ap()[:, 2 * XW + T.ZT : 2 * XW + T.ZT + T.BT],
                          T.hh1_tab, OW1,
                          OW1, OW1, iof_b, "a1" + s, flush1, skip=("agg1" in SKIP),
                          bufs=3)

                nc.sync.dma_start(T.h1T_loc[:], h1T_sb[:, 0:SLICE])
                nc.gpsimd.collective_compute(
                    "AllGather", BYPASS, groups,
                    ins=[T.h1T_loc[:].opt()], outs=[T.h1T_ag[:].opt()],
                )
                h1T_full = pp2.tile([128, P, SLICE], F32)
                nc.sync.dma_start(h1T_full[:], T.h1T_ag.ap().rearrange("s f n -> f s n"))
                h1T_flat = h1T_full[:].rearrange("f s n -> f (s n)")

                h2_nm = pp2.tile([128, NBLK, C], F32)
                g2_nm = pp2.tile([128, NBLK, 1], F32)
                with tc.tile_pool(name="ph2" + s, bufs=2, space="PSUM") as ph2p:
                    for b in range(NBLK):
                        nv = max(0, min(128, N - b * 128))
                        if nv < 128:
                            nc.vector.memset(h2_nm[:, b, :], 0.0)
                            nc.vector.memset(g2_nm[:, b, :], 0.0)
                        if nv == 0:
                            continue
                        ph2 = ph2p.tile([128, C + 1], F32)
                        nc.tensor.matmul(
                            ph2[0:nv, :],
                            h1T_flat[:, b * 128 : b * 128 + nv],
                            W2cat[:],
                            start=True,
                            stop=True,
                        )
                        nc.vector.tensor_copy(h2_nm[0:nv, b, :], ph2[0:nv, 0:C])
                        nc.scalar.activation(g2_nm[0:nv, b, :], ph2[0:nv, C : C + 1], EXP)

                with tc.tile_pool(name="stage2" + s, bufs=1) as stp:
                    st = stp.tile([128, NBLK, 128], BF16, tag="stage2")
                    nc.vector.memset(st[:], 0.0)
                    nc.vector.tensor_copy(st[:, :, 0:1], g2_nm[:])
                    nc.sync.dma_start(
                        T.g2_tab.ap().rearrange("(b p) c -> p b c", p=128), st[:]
                    )

                with tc.tile_pool(name="zu2" + s, bufs=3) as zup:

                    def zflush2(w, po, zup=zup):
                        u_t = zup.tile([128, 1], F32, tag="u2")
                        nc.vector.reciprocal(u_t[:], po[:, 0:1])
                        rows = min(128, SLICE - 128 * w)
                        nc.sync.dma_start(
                            T.u2_sl[w * 128 : w * 128 + rows, :], u_t[0:rows, :]
                        )

                    _spmm(nc, tc, zB, 32,
                          T.idx_d[:, T.zoff * 8 : (T.zoff + sum(zB)) * 8],
                          T.blob.ap()[:, 2 * XW : 2 * XW + T.ZT],
                          T.g2_tab, 128, 8, 8,
                          iof_b, "z2" + s, zflush2, skip=("z" in SKIP), bufs=3)

                nc.gpsimd.collective_compute(
                    "AllGather", BYPASS, groups,
                    ins=[T.u2_sl[:].opt()], outs=[T.u2_full[0:N, :].opt()],
                )
                zt2 = sp2.tile([NPAD - N, 1], F32, tag="zpad2")
                nc.vector.memset(zt2[:], 0.0)
                nc.sync.dma_start(T.u2_full[N:NPAD, :], zt2[:])

                u2_nm = pp2.tile([128, NBLK, 1], F32)
                nc.sync.dma_start(
                    u2_nm[:], T.u2_full.ap().rearrange("(b p) c -> p b c", p=128)
                )
                with tc.tile_pool(name="stage3" + s, bufs=1) as stp:
                    st = stp.tile([128, NBLK, 128], BF16, tag="stage3")
                    nc.vector.memset(st[:], 0.0)
                    for b in range(NBLK):
                        nc.vector.tensor_scalar(
                            st[:, b, 0:C], h2_nm[:, b, :], u2_nm[:, b, :], None, MULT
                        )
                    nc.sync.dma_start(
                        T.hh2_tab.ap().rearrange("(b p) c -> p b c", p=128), st[:]
                    )

                with (
                    tc.tile_pool(name="gw2" + s, bufs=2) as gwp,
                    tc.tile_pool(name="fl2" + s, bufs=2) as flp,
                ):
                    gwi = gwp.tile([128, NWIN * 8], I16, tag="gwi2")
                    _load_idx(nc, gwi, T.idx_d[:, T.gwoff * 8 :])
                    gwb = gwp.tile([128, NWIN, 128], BF16, tag="gwb2")
                    nc.gpsimd.dma_gather(
                        gwb[:], T.g2_tab[:], gwi[:], NWIN * 128, NWIN * 128, 128,
                        single_packet=False,
                    )
                    gwf = gwp.tile([128, NWIN, 128], F32, tag="gwf2")
                    nc.vector.tensor_copy(gwf[:], gwb[:])

                    def flush2(w, po):
                        o2 = flp.tile([128, C], BF16, tag="o2")
                        nc.vector.tensor_scalar(
                            o2[:], po[:, 0:C], gwf[:, w, 0:1], None, MULT
                        )
                        rows = min(128, SLICE - 128 * w)
                        nc.sync.dma_start(
                            T.out_d[w * 128 : w * 128 + rows, :], o2[0:rows, :]
                        )

                    _spmm(nc, tc, bB, 32,
                          T.idx_d[:, T.boff * 8 : (T.boff + sum(bB)) * 8],
                          T.blob.ap()[:, 2 * XW + T.ZT : 2 * XW + T.ZT + T.BT],
                          T.hh2_tab, 128,
                          C, C, iof_b, "a2" + s, flush2, skip=("agg2" in SKIP),
                          bufs=3)


def _build_program(zB, bB, reps=1):
    nc = bacc.Bacc("TRN2", target_bir_lowering=False, debug=False, num_devices=P)
    T = _declare(nc, zB, bB)
    with tile.TileContext(nc) as tc:
        for r in range(reps):
            _emit(nc, tc, T, zB, bB, s=str(r))
            if reps > 1:
                with tc.tile_critical():
                    nc.all_core_barrier()
    nc.compile()
    # the program is immutable from here on, but the jax lowering re-serializes
    # it on every run_bass_kernel_spmd call (~30ms); memoize the bytes
    cached = nc.to_json_bytes()
    nc.to_json_bytes = lambda: cached
    return nc


def _host_inputs(x, W1, a1, W2, a2):
    x = np.asarray(x, np.float64)
    W1 = np.asarray(W1, np.float64)
    a1 = np.asarray(a1, np.float64)
    W2 = np.asarray(W2, np.float64)
    a2 = np.asarray(a2, np.float64)
    # combined bf16 [F, NPAD+W1PAD+W2PAD]: xT | W1 | w1a | W2 | w2a | pads
    xw = np.zeros((F, NPAD + W1PAD + W2PAD), np.float64)
    xw[:, :N] = x.T
    xw[:, NPAD : NPAD + OW1] = W1
    # host-precomputed per-head W @ a_r reductions
    xw[:, NPAD + OW1 : NPAD + OW1 + H] = np.einsum(
        "fhg,hg->fh", W1.reshape(F, H, F), a1[:, F:]
    )
    xw[:, NPAD + W1PAD : NPAD + W1PAD + C] = W2
    xw[:, NPAD + W1PAD + C] = W2 @ a2[0, C:]
    return xw.astype(ml_dtypes.bfloat16)


def build(x, edge_index, W1, a1, W2, a2, reps=1):
    """Build program + per-core input maps. Returns (nc, in_maps)."""
    ei = np.asarray(edge_index)
    row = ei[0].astype(np.int64)
    col = ei[1].astype(np.int64)
    zB, bB, per_core = _build_edge_inputs(row, col)
    nc = _build_program(zB, bB, reps=reps)
    xw = _host_inputs(x, W1, a1, W2, a2)
    # shard columns: core k gets xT cols [k*XSH,(k+1)*XSH) + W1cat cols
    # [k*W1SH,(k+1)*W1SH) + W2cat cols [k*W2SH,(k+1)*W2SH); the AllGather +
    # on-device views restore all three. blob = xw bytes | zrel | brel.
    in_maps = []
    for k in range(P):
        xw_k = np.concatenate(
            [
                xw[:, k * XSH : (k + 1) * XSH],
                xw[:, NPAD + k * W1SH : NPAD + (k + 1) * W1SH],
                xw[:, NPAD + W1PAD + k * W2SH : NPAD + W1PAD + (k + 1) * W2SH],
            ],
            axis=1,
        )
        blob = np.concatenate(
            [
                np.ascontiguousarray(xw_k).view(np.int8),
                per_core[k]["rel"],
            ],
            axis=1,
        )
        in_maps.append(
            {
                "idx": per_core[k]["idx"],
                "blob": np.ascontiguousarray(blob),
            }
        )
    return nc, in_maps


def kernel(x, edge_index, W1, a1, W2, a2):
    nc, in_maps = build(x, edge_index, W1, a1, W2, a2)
    res = run_bass_kernel_spmd(nc, in_maps, list(range(P)))
    out = np.concatenate([res.results[k]["out"] for k in range(P)], axis=0)
    return out.astype(np.float32)
